# revision 51
# baseline (speedup 1.0000x reference)
"""Trainium2 Bass kernel for grouped-attention MoE routing.

Math (derived from the nn.Module):
  gate  = softmax(mlp(maxpool(conv(x))) + mlp(avgpool(conv(x))))      (B,45)
  sel   = sorted(top22(mean_b gate))                                  (22,)
  Per expert e with u = x[:, sel[e], :]:
    energy[l,m] = (a_e*u_l + g_e) * u_m   (rank-1; scalars a,g from weights)
    attn = softmax_m(energy);  s_l = sum_m u_m attn[l,m]
    y_l  = P_e*s_l + Q_e;      A[:,sel[e],:] = y * gate[:,sel[e]]
  G = x * A (flat);  return (G, A_flat)

Implementation strategy (v2):
  Launch 1 (gate): bf16 PE conv with bias folded via a 127th ones row and
    the avg-pool folded in as extra matmul columns; both MLP branches run
    through one block-diagonal matmul pair; outputs batched into one DMA.
  Launch 2 (attn): the rank-1 softmax is evaluated without the LxL energy
    tensor.  With w = exp(g_e*u) and phi = a_e*u:
      den(phi_l) = sum_m w_m exp(phi_l u_m) ~= sum_k cd_k phi_l^k W_k
      num(phi_l) = sum_m u_m w_m exp(phi_l u_m) ~= sum_k cn_k phi_l^k W_{k+1}
    where W_k = sum_m w_m u_m^k are on-device moments and cd/cn are host-
    fitted per-expert polynomial coefficients (least squares over the
    empirical tau = phi*u range; numerator fit |u|-weighted).  s = num/den.
    Per-expert degree ladder (2..6) by empirical |tau| range; experts are
    permuted so degree classes are contiguous and the nested Horner only
    touches suffix slices for the high degrees.  Everything bf16, l-major
    (l outer, e inner) so per-(b,e) coefficient broadcasts stay packed.
  Routing (45-float mean-gate reduction) is mediated on host between the
  two launches, equivalent to the all-reduce in the sharding hint.
"""

import math
import numpy as np
from contextlib import ExitStack

import bass_rust
import concourse.bass as bass
import concourse.mybir as mybir
import concourse.tile as tile
from concourse.bass_utils import run_bass_kernel_spmd

_MULTIWAIT_OK = ("InstNoOp", "InstAllEngineBarrier",
                 "InstEventSemaphore", "InstUnconditionalBranch")


def legalize_sync_waits(nc):
    """walrus codegen on this stack rejects >1 sync wait on most
    instructions; hoist extra waits onto same-engine NoOps."""
    for func in nc.m.functions:
        for block in func.blocks:
            il = block.instructions
            out = []
            for inst in il:
                tname = type(inst).__name__
                si = getattr(inst, "sync_info", None)
                waits = list(si.on_wait) if si is not None else []
                if tname not in _MULTIWAIT_OK and len(waits) > 1:
                    for k, w in enumerate(waits):
                        nop = mybir.InstNoOp(
                            name=f"{inst.name}-synop{k}", ins=[], outs=[])
                        nop.engine = inst.engine
                        nop.sync_info = bass_rust.SyncInfo(
                            on_wait=[w], on_update=[])
                        out.append(nop)
                    inst.sync_info = bass_rust.SyncInfo(
                        on_wait=[], on_update=list(inst.sync_info.on_update))
                out.append(inst)
            il.clear()
            il.extend(out)


B, C, L, E = 8192, 45, 21, 22
NCORES = 8
BC = B // NCORES          # rows per core
P = 128                   # SBUF partitions
NT = BC // P              # batch tiles per core
CL = C * L                # 945
EL = E * L                # 462
WF = NT * EL              # 3696 full-shard free width (t, l, e) l-major
F32 = mybir.dt.float32
BF16 = mybir.dt.bfloat16
AF = mybir.ActivationFunctionType
ALU = mybir.AluOpType
AX = mybir.AxisListType

NG = 8                    # conv channel groups (6,6,...,3 channels)
GCH = [list(range(g, min(g + 6, C))) for g in range(0, C, 6)]
NLEV = 8                  # moment levels W_0..W_7 held on device
DEG_MAX = 6


def _ap(base, extra_free):
    """Custom free-dim access pattern on an SBUF tile slice (partition
    dim kept from `base`)."""
    return bass.AP(tensor=base.tensor, offset=base.offset,
                   ap=[base.ap[0]] + extra_free)


def _off(base, extra_free, col_off):
    ap = bass.AP(tensor=base.tensor, offset=base.offset,
                 ap=[base.ap[0]] + extra_free)
    ap.offset = ap.offset + col_off
    return ap


# --------------------------------------------------------------------------
# Launch 1: gating network
# --------------------------------------------------------------------------

def build_gate_program():
    """Gate launch. Host uploads x TRANSPOSED and padded: XT (1024, 1024)
    with rows 0:945 = x[coreshard].T (row = (chan,l)), rows 945:1023 = 0,
    row 1023 = 1 (bias lane).  The conv+avg matmul accumulates chunk-wise
    into 3 PSUM column blocks of 15 channels x 22 outputs; no on-device
    transposes or PSUM->SBUF copies are needed for the conv at all."""
    nc = bass.Bass()
    KC = P * NT                       # 1024 padded contraction rows
    xT = nc.declare_dram_parameter("xT", [KC, KC], BF16, isOutput=False)
    # packed rhs slices for the 12 (chunk, block) matmuls: (128, 12*330)
    wpk = nc.declare_dram_parameter("wpk", [P, 12 * 330], BF16,
                                    isOutput=False)
    w1blk = nc.declare_dram_parameter("w1blk", [P, P], F32, isOutput=False)
    w2blk = nc.declare_dram_parameter("w2blk", [P, 90], F32, isOutput=False)
    ident = nc.declare_dram_parameter("ident", [P, P], F32, isOutput=False)
    gate_o = nc.declare_dram_parameter("gate", [P, NT * C], BF16,
                                       isOutput=True)

    # (chunk, block) pairs: block j covers channels 15j..15j+14 =
    # contraction rows 315j..315j+314, plus the bias lane in chunk 7
    PAIRS = [[0, 1, 2, 7], [2, 3, 4, 7], [4, 5, 6, 7]]

    with tile.TileContext(nc) as tc, ExitStack() as ctx, \
            nc.allow_low_precision(reason="bf16 gate pipeline; 2e-2 tol"):
        singles = ctx.enter_context(tc.tile_pool(name="singles", bufs=1))
        cvps = ctx.enter_context(tc.tile_pool(name="cvps", bufs=2,
                                              space="PSUM"))
        ppps = ctx.enter_context(tc.tile_pool(name="ppps", bufs=1,
                                              space="PSUM"))
        work = ctx.enter_context(tc.tile_pool(name="work", bufs=2))
        small = ctx.enter_context(tc.tile_pool(name="small", bufs=3))

        def dve_const(dram, p, n, dt=BF16):
            raw = singles.tile([p, n], dt, name="raw_" + dram.name)
            nc.sync.dma_start(out=raw, in_=dram[:, :])
            t = singles.tile([p, n], dt, name="sb_" + dram.name)
            nc.vector.tensor_copy(out=t, in_=raw)
            return t

        sb_w = dve_const(wpk, P, 12 * 330)
        sb_w1 = dve_const(w1blk, P, P, F32)
        sb_w2 = dve_const(w2blk, P, 90, F32)
        sb_id = dve_const(ident, P, P, F32)

        # xT loads: per-tile DMA of the (128, 8x128) lhsT panel so tile 0
        # can start early.  lhs[p, c*128+b] = XT[128c+p, 128t+b]
        xb = xT[:, :]
        lhsT = []
        for t in range(NT):
            lt = singles.tile([P, NT * P], BF16, name=f"lhsT{t}")
            ap = bass.AP(tensor=xb.tensor, offset=xb.offset,
                         ap=[[KC, P], [P * KC, NT], [1, P]])
            ap.offset = ap.offset + t * P
            nc.sync.dma_start(out=lt, in_=ap)
            lhsT.append(lt)

        # persistent h tiles: cols 0:90 rewritten each use; 90:128 junk is
        # zeroed once (col 95 = ones lane feeding the layer-1 bias row)
        hb = []
        for i in range(2):
            t = singles.tile([P, P], F32, name=f"hb{i}")
            nc.vector.memset(t[:, 90:P], 0.0)
            nc.vector.memset(t[:, 95:96], 1.0)
            hb.append(t)

        gate_all = singles.tile([P, NT * C], BF16)
        zall = singles.tile([P, NT * C], BF16)

        def conv_block(t):
            lt = lhsT[t]
            cvt = [cvps.tile([P, 330], F32, tag=f"cv{j}", name=f"cv{j}")
                   for j in range(3)]
            for j in range(3):
                for i, c in enumerate(PAIRS[j]):
                    nc.tensor.matmul(
                        cvt[j], lt[:, c * P:(c + 1) * P],
                        sb_w[:, (j * 4 + i) * 330:(j * 4 + i + 1) * 330],
                        start=(i == 0), stop=(i == len(PAIRS[j]) - 1))
            h = hb[t % 2]
            for j in range(3):
                nc.vector.tensor_reduce(
                    out=h[:, 15 * j:15 * j + 15],
                    in_=_ap(cvt[j][:, 0:330], [[22, 15], [1, L]]),
                    axis=AX.X, op=ALU.max)
            # avg lanes (col 21 of each 22-block): 2 on DVE, 1 on Act
            nc.vector.tensor_copy(out=h[:, 45:60],
                                  in_=_off(cvt[0], [[22, 15]], 21))
            nc.scalar.copy(out=h[:, 60:75], in_=_off(cvt[1], [[22, 15]], 21))
            nc.vector.tensor_copy(out=h[:, 75:90],
                                  in_=_off(cvt[2], [[22, 15]], 21))
            return h

        def mlp_block(t, h):
            # both MLP branches through 128x128 f32 blocks; PE transposes
            # with ones lanes via h col 95 and the saturated tanh col 127
            trm = ppps.tile([P, P], F32, tag="trm", name="trm")
            nc.tensor.transpose(trm, h, sb_id)
            hT = work.tile([P, P], F32, tag="hT", name="hT")
            nc.scalar.copy(out=hT, in_=trm)
            pp = ppps.tile([P, P], F32, tag="pp", name="pp")
            nc.tensor.matmul(pp, hT, sb_w1, start=True, stop=True)
            t1 = small.tile([P, P], F32, tag="t1", name="t1")
            nc.scalar.activation(out=t1, in_=pp, func=AF.Tanh)
            trm2 = ppps.tile([P, P], F32, tag="trm", name="trm2")
            nc.tensor.transpose(trm2, t1, sb_id)
            t1T = work.tile([P, P], F32, tag="t1T", name="t1T")
            nc.vector.tensor_copy(out=t1T, in_=trm2)
            p2 = ppps.tile([P, 90], F32, tag="pp", name="p2")
            nc.tensor.matmul(p2, t1T, sb_w2, start=True, stop=True)
            z2 = small.tile([P, 90], BF16, tag="z2", name="z2")
            nc.scalar.activation(out=z2, in_=p2, func=AF.Tanh)
            nc.vector.tensor_add(out=zall[:, t * C:(t + 1) * C],
                                 in0=z2[:, 0:45], in1=z2[:, 45:90])

        # software-pipelined emission: engines issue in program order, so
        # interleave conv(t+1) ahead of mlp(t) to let tiles overlap
        hprev = None
        for t in range(NT + 1):
            if t < NT:
                hcur = conv_block(t)
            if t >= 1:
                mlp_block(t - 1, hprev)
            hprev = hcur

        # batched softmax over all 8 tile blocks (segmented per block)
        zmax = singles.tile([P, NT], F32)
        nc.vector.tensor_reduce(out=zmax, in_=_ap(zall, [[C, NT], [1, C]]),
                                axis=AX.X, op=ALU.max)
        zmax16 = singles.tile([P, NT], BF16)
        nc.vector.tensor_copy(out=zmax16, in_=zmax)
        zsub = singles.tile([P, NT * C], BF16)
        nc.vector.tensor_sub(out=zsub, in0=zall,
                             in1=_ap(zmax16, [[1, NT], [0, C]]))
        eg = singles.tile([P, NT * C], BF16)
        nc.scalar.activation(out=eg, in_=zsub, func=AF.Exp)
        ssum = singles.tile([P, NT], F32)
        nc.vector.tensor_reduce(out=ssum, in_=_ap(eg, [[C, NT], [1, C]]),
                                axis=AX.X, op=ALU.add)
        rs = singles.tile([P, NT], BF16)
        nc.vector.reciprocal(out=rs, in_=ssum)
        nc.vector.tensor_mul(out=gate_all, in0=eg,
                             in1=_ap(rs, [[1, NT], [0, C]]))
        nc.sync.dma_start(out=gate_o[:, :], in_=gate_all)
    legalize_sync_waits(nc)
    return nc


# --------------------------------------------------------------------------
# Launch 2: expert attention via fitted moment polynomials
# --------------------------------------------------------------------------

def build_attn_program(degs):
    """degs: per-expert polynomial degree, sorted ascending (len 22)."""
    degs = list(degs)
    assert degs == sorted(degs)
    dmax = max(degs)
    # suffix start index for "experts with degree > k"
    estart = {k: next((i for i in range(E) if degs[i] > k), E)
              for k in range(dmax)}
    # first expert of each degree class (for acc initialization)
    class_start = {}
    for i, d in enumerate(degs):
        class_start.setdefault(d, i)

    nc = bass.Bass()
    u_d = nc.declare_dram_parameter("u", [BC, EL], BF16, isOutput=False)
    phi_d = nc.declare_dram_parameter("phi", [BC, EL], BF16, isOutput=False)
    gu_d = nc.declare_dram_parameter("gu", [BC, EL], BF16, isOutput=False)
    gpq_d = nc.declare_dram_parameter("gpq", [BC, 2 * E], BF16,
                                      isOutput=False)
    cd_d = nc.declare_dram_parameter("cd", [1, (NLEV - 1) * NT * E], BF16,
                                     isOutput=False)
    cn_d = nc.declare_dram_parameter("cn", [1, (NLEV - 1) * NT * E], BF16,
                                     isOutput=False)
    at_o = nc.declare_dram_parameter("at", [BC, EL], BF16, isOutput=True)
    gt_o = nc.declare_dram_parameter("gt", [BC, EL], BF16, isOutput=True)

    def shard_ap(dram, ncols):
        base = dram[:, :]
        return bass.AP(tensor=base.tensor, offset=base.offset,
                       ap=[[ncols, P], [P * ncols, NT], [1, ncols]])

    with tile.TileContext(nc) as tc, ExitStack() as ctx, \
            nc.allow_low_precision(reason="bf16 attn pipeline; 2e-2 tol"):
        sg = ctx.enter_context(tc.tile_pool(name="sg", bufs=1))

        gu = sg.tile([P, WF], BF16)
        nc.sync.dma_start(out=gu, in_=shard_ap(gu_d, EL))
        u = sg.tile([P, WF], BF16)
        nc.sync.dma_start(out=u, in_=shard_ap(u_d, EL))
        phi = sg.tile([P, WF], BF16)
        nc.sync.dma_start(out=phi, in_=shard_ap(phi_d, EL))
        gpq = sg.tile([P, NT * 2 * E], BF16)
        nc.sync.dma_start(out=gpq, in_=shard_ap(gpq_d, 2 * E))

        def bconst(dram, n, nm):
            base = dram[:, :]
            t = sg.tile([P, n], BF16, name=nm)
            nc.sync.dma_start(
                out=t, in_=bass.AP(tensor=base.tensor, offset=base.offset,
                                   ap=[[0, P], [1, n]]))
            return t

        NC_ = NT * E                      # 176 moment columns per level
        cdB = bconst(cd_d, (NLEV - 1) * NC_, "cdB")
        cnB = bconst(cn_d, (NLEV - 1) * NC_, "cnB")

        Wt = sg.tile([P, NLEV * NC_], BF16)   # moment levels W_0..W_7
        nc.vector.memset(Wt, 0.0)

        # slice helpers (l-major: free = (t, l, e); e innermost)
        def full3(tile_, e0=0, ne=E, coloff=0):
            return _off(tile_, [[EL, NT], [E, L], [1, ne]], coloff + e0)

        def wout(lev, e0=0, ne=E):
            return _off(Wt, [[E, NT], [1, ne]], lev * NC_ + e0)

        def wred_in(src, e0=0, ne=E):
            return _off(src, [[EL, NT], [1, ne], [E, L]], e0)

        def coef(ctile, lev, e0=0, ne=E):
            return _off(ctile, [[E, NT], [0, L], [1, ne]], lev * NC_ + e0)

        # w = exp(gu); moments
        w = sg.tile([P, WF], BF16)
        nc.scalar.activation(out=w, in_=gu, func=AF.Exp)
        nc.vector.tensor_reduce(out=wout(0), in_=wred_in(w), axis=AX.X,
                                op=ALU.add)
        va = sg.tile([P, WF], BF16)
        vb = sg.tile([P, WF], BF16)
        cur, nxt = va, vb
        nc.vector.tensor_mul(out=cur, in0=w, in1=u)
        nc.vector.tensor_reduce(out=wout(1), in_=wred_in(cur), axis=AX.X,
                                op=ALU.add)
        for lev in range(2, NLEV):
            # moment level `lev` is needed by experts with degree >= lev-1
            e0 = estart.get(lev - 2, E)
            ne = E - e0
            if ne <= 0:
                break
            nc.vector.tensor_mul(out=full3(nxt, e0, ne),
                                 in0=full3(cur, e0, ne),
                                 in1=full3(u, e0, ne))
            nc.vector.tensor_reduce(out=wout(lev, e0, ne),
                                    in_=wred_in(nxt, e0, ne),
                                    axis=AX.X, op=ALU.add)
            cur, nxt = nxt, cur

        # Horner coefficient tensors: D_k = W_k*cd_k, N_k = W_{k+1}*cn_k
        Dt = sg.tile([P, (NLEV - 1) * NC_], BF16)
        nc.vector.tensor_mul(out=Dt, in0=Wt[:, 0:(NLEV - 1) * NC_], in1=cdB)
        Nt = sg.tile([P, (NLEV - 1) * NC_], BF16)
        nc.vector.tensor_mul(out=Nt, in0=Wt[:, NC_:NLEV * NC_], in1=cnB)

        # nested mixed-degree Horner (experts sorted by degree ascending)
        accd = sg.tile([P, WF], BF16)
        accn = sg.tile([P, WF], BF16)
        for d, e0 in class_start.items():
            ne = (min([cs for dd, cs in class_start.items() if dd > d],
                      default=E)) - e0
            nc.vector.tensor_copy(out=full3(accd, e0, ne),
                                  in_=coef(Dt, d, e0, ne))
            nc.vector.tensor_copy(out=full3(accn, e0, ne),
                                  in_=coef(Nt, d, e0, ne))
        for k in range(dmax - 1, -1, -1):
            e0 = estart[k]
            ne = E - e0
            nc.vector.tensor_mul(out=full3(accd, e0, ne),
                                 in0=full3(accd, e0, ne),
                                 in1=full3(phi, e0, ne))
            nc.vector.tensor_add(out=full3(accd, e0, ne),
                                 in0=full3(accd, e0, ne),
                                 in1=coef(Dt, k, e0, ne))
            nc.vector.tensor_mul(out=full3(accn, e0, ne),
                                 in0=full3(accn, e0, ne),
                                 in1=full3(phi, e0, ne))
            nc.vector.tensor_add(out=full3(accn, e0, ne),
                                 in0=full3(accn, e0, ne),
                                 in1=coef(Nt, k, e0, ne))

        # s = num/den; at = s*gp + gq; gt = at*u
        # (tail ops split 16/6 experts across DVE and Pool so they overlap)
        rden = sg.tile([P, WF], BF16)
        nc.vector.reciprocal(out=rden, in_=accd)
        s = accn
        nc.vector.tensor_mul(out=s, in0=accn, in1=rden)
        at = accd
        nc.vector.tensor_mul(
            out=at, in0=s,
            in1=_ap(gpq, [[2 * E, NT], [0, L], [1, E]]))
        ESP = 16
        nc.vector.tensor_add(
            out=full3(at, 0, ESP), in0=full3(at, 0, ESP),
            in1=_off(gpq, [[2 * E, NT], [0, L], [1, ESP]], E))
        nc.gpsimd.tensor_add(
            out=full3(at, ESP, E - ESP), in0=full3(at, ESP, E - ESP),
            in1=_off(gpq, [[2 * E, NT], [0, L], [1, E - ESP]], E + ESP))
        gt = va
        nc.gpsimd.tensor_mul(out=full3(gt, ESP, E - ESP),
                             in0=full3(at, ESP, E - ESP),
                             in1=full3(u, ESP, E - ESP))
        nc.vector.tensor_mul(out=full3(gt, 0, ESP),
                             in0=full3(at, 0, ESP),
                             in1=full3(u, 0, ESP))
        nc.sync.dma_start(out=shard_ap(at_o, EL), in_=at)
        nc.sync.dma_start(out=shard_ap(gt_o, EL), in_=gt)
    legalize_sync_waits(nc)
    return nc


# --------------------------------------------------------------------------
# Host-side preparation
# --------------------------------------------------------------------------

def _gate_params(inputs):
    gc_w = inputs["gc_w"].astype(np.float64)
    gc_b = inputs["gc_b"].astype(np.float64)
    KC = P * NT
    # full conv weight: rows = (chan,l) + pad + bias lane, cols = (chan, 22)
    wfull = np.zeros((KC, 990), np.float32)
    wavvec = gc_w.mean(0)
    for i in range(C):
        wfull[i * L:(i + 1) * L, i * 22:i * 22 + L] = gc_w.T
        wfull[i * L:(i + 1) * L, i * 22 + L] = wavvec
        wfull[KC - 1, i * 22:i * 22 + L] = gc_b
        wfull[KC - 1, i * 22 + L] = gc_b.mean()
    PAIRS = [[0, 1, 2, 7], [2, 3, 4, 7], [4, 5, 6, 7]]
    wpk = np.zeros((P, 12 * 330), np.float32)
    for j in range(3):
        for i, c in enumerate(PAIRS[j]):
            wpk[:, (j * 4 + i) * 330:(j * 4 + i + 1) * 330] = \
                wfull[c * P:(c + 1) * P, j * 330:(j + 1) * 330]
    # 128x128 MLP layer-1 block: rows = transposed h cols (0:45 mx, 45:90
    # av, 95 = ones), out cols 0:50 = both branch hiddens, col 127 driven to
    # +30 via the ones row so tanh saturates to an exact 1.0 "ones" lane for
    # layer 2; all other cells zero.
    w1blk = np.zeros((128, 128), np.float32)
    w1blk[0:45, 0:25] = inputs["w1"].T
    w1blk[45:90, 25:50] = inputs["w1"].T
    w1blk[95, 0:25] = inputs["b1"]
    w1blk[95, 25:50] = inputs["b1"]
    w1blk[95, 127] = 30.0
    w2blk = np.zeros((128, 90), np.float32)
    w2blk[0:25, 0:45] = inputs["w2"].T
    w2blk[25:50, 45:90] = inputs["w2"].T
    w2blk[127, 0:45] = inputs["b2"]
    w2blk[127, 45:90] = inputs["b2"]
    return wpk, w1blk, w2blk


def _fit_exp(tsamp, K, wsamp=None, ntail=0.5):
    t = np.asarray(tsamp, np.float64)
    w = np.ones_like(t) if wsamp is None else np.asarray(wsamp, np.float64)
    tm = max(np.abs(t).max(), 1e-3)
    textra = np.linspace(-tm, tm, 64)
    t = np.concatenate([t, textra])
    w = np.concatenate([w, np.full(64, ntail * w.mean())])
    V = np.vander(t, K + 1, increasing=True) * w[:, None]
    c, *_ = np.linalg.lstsq(V, np.exp(t) * w, rcond=None)
    return c


_CACHE = {}


def kernel(**inputs):
    inputs = {k: np.ascontiguousarray(np.asarray(v)) for k, v in
              inputs.items()}
    x = inputs["x"].astype(np.float32)            # (B, C, L)
    import ml_dtypes
    bf = ml_dtypes.bfloat16

    wpk, w1blk, w2blk = _gate_params(inputs)
    cores = list(range(NCORES))
    KC = P * NT

    if "gate" not in _CACHE:
        _CACHE["gate"] = build_gate_program()
    nc1 = _CACHE["gate"]
    maps1 = []
    for i in cores:
        xt = np.zeros((KC, KC), np.float32)
        xt[0:CL, :] = x.reshape(B, CL)[i * BC:(i + 1) * BC].T
        xt[KC - 1, :] = 1.0
        maps1.append({"xT": xt.astype(bf), "wpk": wpk.astype(bf),
                      "w1blk": w1blk, "w2blk": w2blk,
                      "ident": np.eye(P, dtype=np.float32)})
    r1 = run_bass_kernel_spmd(nc1, maps1, cores).results
    # gate tiles come back as (P, NT*C): row p, block t -> batch p + t*P
    gate = np.zeros((B, C), np.float32)
    for i in cores:
        g = np.asarray(r1[i]["gate"]).astype(np.float32)
        gate[i * BC:(i + 1) * BC] = \
            g.reshape(P, NT, C).transpose(1, 0, 2).reshape(BC, C)
    mean_gate = gate.astype(np.float64).mean(0)
    sel = np.sort(np.argsort(-mean_gate, kind="stable")[:E])

    # expert scalars
    wq, bq = inputs["wq"], inputs["bq"]
    wk, bk = inputs["wk"], inputs["bk"]
    wv, bv = inputs["wv"], inputs["bv"]
    wo, bo = inputs["wo"], inputs["bo"]
    alpha = (wq * wk).sum(1).astype(np.float32)
    gamma = (bq * wk).sum(1).astype(np.float32)
    pv = (wo * wv).sum(1).astype(np.float32)
    qv = ((wo * bv).sum(1) + bo).astype(np.float32)

    usel = x[:, sel, :]                            # (B, E, L)
    # per-expert tau range -> degree ladder
    phimax = np.abs(alpha[None, :, None] * usel).max(axis=(0, 2))
    umax = np.abs(usel).max(axis=(0, 2))
    taumax = phimax * umax
    degs_raw = np.where(taumax <= 0.7, 2,
                        np.where(taumax <= 1.2, 3,
                                 np.where(taumax <= 2.0, 4, 6)))
    perm = np.argsort(degs_raw, kind="stable")     # experts by degree asc
    degs = degs_raw[perm]

    # coefficient fits per (permuted) expert
    rng = np.random.RandomState(12345)
    cd = np.zeros((NLEV - 1, E), np.float32)
    cn = np.zeros((NLEV - 1, E), np.float32)
    sub = usel[::16]                               # (B/16, E, L) samples
    for j, e in enumerate(perm):
        K = int(degs[j])
        ue = sub[:, e, :].astype(np.float64)
        tau = (alpha[e] * ue[:, :, None] * ue[:, None, :]).ravel()
        uw = np.abs(np.broadcast_to(ue[:, None, :], ue.shape[:1] + (L, L))
                    ).ravel()
        ss = rng.choice(tau.size, min(40000, tau.size), replace=False)
        cd[0:K + 1, j] = _fit_exp(tau[ss], K)
        cn[0:K + 1, j] = _fit_exp(tau[ss], K, wsamp=uw[ss] + 0.1)

    # device tensors (l-major, expert-permuted)
    uselp = usel[:, perm, :]
    u_lm = np.ascontiguousarray(uselp.transpose(0, 2, 1).reshape(B, EL))
    phi_lm = np.ascontiguousarray(
        (alpha[perm][None, :, None] * uselp).transpose(0, 2, 1)
        .reshape(B, EL))
    gu_lm = np.ascontiguousarray(
        (gamma[perm][None, :, None] * uselp).transpose(0, 2, 1)
        .reshape(B, EL))
    gsel = gate[:, sel][:, perm]
    gp = gsel * pv[perm][None, :]
    gq = gsel * qv[perm][None, :]
    gpq = np.concatenate([gp, gq], 1).astype(np.float32)   # (B, 44)

    NC_ = NT * E
    cd_full = np.tile(cd[:, None, :], (1, NT, 1)).reshape(1, (NLEV - 1) * NC_)
    cn_full = np.tile(cn[:, None, :], (1, NT, 1)).reshape(1, (NLEV - 1) * NC_)

    key = (tuple(int(d) for d in degs),)
    if _CACHE.get("attn_key") != key:
        _CACHE["attn"] = build_attn_program([int(d) for d in degs])
        _CACHE["attn_key"] = key
    nc2 = _CACHE["attn"]
    maps2 = [{"u": u_lm[i * BC:(i + 1) * BC].astype(bf),
              "phi": phi_lm[i * BC:(i + 1) * BC].astype(bf),
              "gu": gu_lm[i * BC:(i + 1) * BC].astype(bf),
              "gpq": gpq[i * BC:(i + 1) * BC].astype(bf),
              "cd": cd_full.astype(bf), "cn": cn_full.astype(bf)}
             for i in cores]
    r2 = run_bass_kernel_spmd(nc2, maps2, cores).results
    at = np.concatenate([np.asarray(r["at"]).astype(np.float32)
                         for r in r2], 0)          # (B, 462) l-major perm
    gt = np.concatenate([np.asarray(r["gt"]).astype(np.float32)
                         for r in r2], 0)

    inv = np.argsort(perm)
    at_e = at.reshape(B, L, E).transpose(0, 2, 1)[:, inv, :]   # (B,E,L)
    gt_e = gt.reshape(B, L, E).transpose(0, 2, 1)[:, inv, :]
    A_full = np.zeros((B, C, L), np.float32)
    G_full = np.zeros((B, C, L), np.float32)
    A_full[:, sel, :] = at_e
    G_full[:, sel, :] = gt_e
    return G_full.reshape(B, CL), A_full.reshape(B, CL)


# revision 53
# speedup vs baseline: 1.0235x; 1.0235x over previous
"""Trainium2 Bass kernel for grouped-attention MoE routing.

Math (derived from the nn.Module):
  gate  = softmax(mlp(maxpool(conv(x))) + mlp(avgpool(conv(x))))      (B,45)
  sel   = sorted(top22(mean_b gate))                                  (22,)
  Per expert e with u = x[:, sel[e], :]:
    energy[l,m] = (a_e*u_l + g_e) * u_m   (rank-1; scalars a,g from weights)
    attn = softmax_m(energy);  s_l = sum_m u_m attn[l,m]
    y_l  = P_e*s_l + Q_e;      A[:,sel[e],:] = y * gate[:,sel[e]]
  G = x * A (flat);  return (G, A_flat)

Implementation strategy (v2):
  Launch 1 (gate): bf16 PE conv with bias folded via a 127th ones row and
    the avg-pool folded in as extra matmul columns; both MLP branches run
    through one block-diagonal matmul pair; outputs batched into one DMA.
  Launch 2 (attn): the rank-1 softmax is evaluated without the LxL energy
    tensor.  With w = exp(g_e*u) and phi = a_e*u:
      den(phi_l) = sum_m w_m exp(phi_l u_m) ~= sum_k cd_k phi_l^k W_k
      num(phi_l) = sum_m u_m w_m exp(phi_l u_m) ~= sum_k cn_k phi_l^k W_{k+1}
    where W_k = sum_m w_m u_m^k are on-device moments and cd/cn are host-
    fitted per-expert polynomial coefficients (least squares over the
    empirical tau = phi*u range; numerator fit |u|-weighted).  s = num/den.
    Per-expert degree ladder (2..6) by empirical |tau| range; experts are
    permuted so degree classes are contiguous and the nested Horner only
    touches suffix slices for the high degrees.  Everything bf16, l-major
    (l outer, e inner) so per-(b,e) coefficient broadcasts stay packed.
  Routing (45-float mean-gate reduction) is mediated on host between the
  two launches, equivalent to the all-reduce in the sharding hint.
"""

import math
import numpy as np
from contextlib import ExitStack

import bass_rust
import concourse.bass as bass
import concourse.mybir as mybir
import concourse.tile as tile
from concourse.bass_utils import run_bass_kernel_spmd

_MULTIWAIT_OK = ("InstNoOp", "InstAllEngineBarrier",
                 "InstEventSemaphore", "InstUnconditionalBranch")


def legalize_sync_waits(nc):
    """walrus codegen on this stack rejects >1 sync wait on most
    instructions; hoist extra waits onto same-engine NoOps."""
    for func in nc.m.functions:
        for block in func.blocks:
            il = block.instructions
            out = []
            for inst in il:
                tname = type(inst).__name__
                si = getattr(inst, "sync_info", None)
                waits = list(si.on_wait) if si is not None else []
                if tname not in _MULTIWAIT_OK and len(waits) > 1:
                    for k, w in enumerate(waits):
                        nop = mybir.InstNoOp(
                            name=f"{inst.name}-synop{k}", ins=[], outs=[])
                        nop.engine = inst.engine
                        nop.sync_info = bass_rust.SyncInfo(
                            on_wait=[w], on_update=[])
                        out.append(nop)
                    inst.sync_info = bass_rust.SyncInfo(
                        on_wait=[], on_update=list(inst.sync_info.on_update))
                out.append(inst)
            il.clear()
            il.extend(out)


B, C, L, E = 8192, 45, 21, 22
NCORES = 8
BC = B // NCORES          # rows per core
P = 128                   # SBUF partitions
NT = BC // P              # batch tiles per core
CL = C * L                # 945
EL = E * L                # 462
WF = NT * EL              # 3696 full-shard free width (t, l, e) l-major
F32 = mybir.dt.float32
BF16 = mybir.dt.bfloat16
AF = mybir.ActivationFunctionType
ALU = mybir.AluOpType
AX = mybir.AxisListType

NG = 8                    # conv channel groups (6,6,...,3 channels)
GCH = [list(range(g, min(g + 6, C))) for g in range(0, C, 6)]
NLEV = 8                  # moment levels W_0..W_7 held on device
DEG_MAX = 6


def _ap(base, extra_free):
    """Custom free-dim access pattern on an SBUF tile slice (partition
    dim kept from `base`)."""
    return bass.AP(tensor=base.tensor, offset=base.offset,
                   ap=[base.ap[0]] + extra_free)


def _off(base, extra_free, col_off):
    ap = bass.AP(tensor=base.tensor, offset=base.offset,
                 ap=[base.ap[0]] + extra_free)
    ap.offset = ap.offset + col_off
    return ap


# --------------------------------------------------------------------------
# Launch 1: gating network
# --------------------------------------------------------------------------

def build_gate_program():
    """Gate launch. Host uploads x TRANSPOSED and padded: XT (1024, 1024)
    with rows 0:945 = x[coreshard].T (row = (chan,l)), rows 945:1023 = 0,
    row 1023 = 1 (bias lane).  The conv+avg matmul accumulates chunk-wise
    into 3 PSUM column blocks of 15 channels x 22 outputs; no on-device
    transposes or PSUM->SBUF copies are needed for the conv at all."""
    nc = bass.Bass()
    KC = P * NT                       # 1024 padded contraction rows
    xT = nc.declare_dram_parameter("xT", [KC, KC], BF16, isOutput=False)
    # packed rhs slices for the 12 (chunk, block) matmuls: (128, 12*330)
    wpk = nc.declare_dram_parameter("wpk", [P, 12 * 330], BF16,
                                    isOutput=False)
    w1blk = nc.declare_dram_parameter("w1blk", [P, P], F32, isOutput=False)
    w2blk = nc.declare_dram_parameter("w2blk", [P, 90], F32, isOutput=False)
    ident = nc.declare_dram_parameter("ident", [P, P], F32, isOutput=False)
    gate_o = nc.declare_dram_parameter("gate", [P, NT * C], BF16,
                                       isOutput=True)

    # (chunk, block) pairs: block j covers channels 15j..15j+14 =
    # contraction rows 315j..315j+314, plus the bias lane in chunk 7
    PAIRS = [[0, 1, 2, 7], [2, 3, 4, 7], [4, 5, 6, 7]]

    with tile.TileContext(nc) as tc, ExitStack() as ctx, \
            nc.allow_low_precision(reason="bf16 gate pipeline; 2e-2 tol"):
        singles = ctx.enter_context(tc.tile_pool(name="singles", bufs=1))
        cvps = ctx.enter_context(tc.tile_pool(name="cvps", bufs=2,
                                              space="PSUM"))
        ppps = ctx.enter_context(tc.tile_pool(name="ppps", bufs=1,
                                              space="PSUM"))
        work = ctx.enter_context(tc.tile_pool(name="work", bufs=2))
        small = ctx.enter_context(tc.tile_pool(name="small", bufs=3))

        def dve_const(dram, p, n, dt=BF16):
            raw = singles.tile([p, n], dt, name="raw_" + dram.name)
            nc.sync.dma_start(out=raw, in_=dram[:, :])
            t = singles.tile([p, n], dt, name="sb_" + dram.name)
            nc.vector.tensor_copy(out=t, in_=raw)
            return t

        sb_w = dve_const(wpk, P, 12 * 330)

        # xT loads: per-tile DMA of the (128, 8x128) lhsT panel so tile 0
        # can start early.  lhs[p, c*128+b] = XT[128c+p, 128t+b]
        xb = xT[:, :]
        lhsT = []

        def load_lhsT(t):
            lt = singles.tile([P, NT * P], BF16, name=f"lhsT{t}")
            ap = bass.AP(tensor=xb.tensor, offset=xb.offset,
                         ap=[[KC, P], [P * KC, NT], [1, P]])
            ap.offset = ap.offset + t * P
            nc.sync.dma_start(out=lt, in_=ap)
            lhsT.append(lt)

        load_lhsT(0)
        load_lhsT(1)
        sb_w1 = dve_const(w1blk, P, P, F32)
        sb_w2 = dve_const(w2blk, P, 90, F32)
        sb_id = dve_const(ident, P, P, F32)
        for t in range(2, NT):
            load_lhsT(t)

        # persistent h tiles: cols 0:90 rewritten each use; 90:128 junk is
        # zeroed once (col 95 = ones lane feeding the layer-1 bias row)
        hb = []
        for i in range(2):
            t = singles.tile([P, P], F32, name=f"hb{i}")
            nc.vector.memset(t[:, 90:P], 0.0)
            nc.vector.memset(t[:, 95:96], 1.0)
            hb.append(t)

        gate_all = singles.tile([P, NT * C], BF16)
        zall = singles.tile([P, NT * C], BF16)

        def conv_block(t):
            lt = lhsT[t]
            cvt = [cvps.tile([P, 330], F32, tag=f"cv{j}", name=f"cv{j}")
                   for j in range(3)]
            for j in range(3):
                for i, c in enumerate(PAIRS[j]):
                    nc.tensor.matmul(
                        cvt[j], lt[:, c * P:(c + 1) * P],
                        sb_w[:, (j * 4 + i) * 330:(j * 4 + i + 1) * 330],
                        start=(i == 0), stop=(i == len(PAIRS[j]) - 1))
            h = hb[t % 2]
            for j in range(3):
                nc.vector.tensor_reduce(
                    out=h[:, 15 * j:15 * j + 15],
                    in_=_ap(cvt[j][:, 0:330], [[22, 15], [1, L]]),
                    axis=AX.X, op=ALU.max)
            # avg lanes (col 21 of each 22-block): 2 on DVE, 1 on Act
            nc.vector.tensor_copy(out=h[:, 45:60],
                                  in_=_off(cvt[0], [[22, 15]], 21))
            nc.scalar.copy(out=h[:, 60:75], in_=_off(cvt[1], [[22, 15]], 21))
            nc.vector.tensor_copy(out=h[:, 75:90],
                                  in_=_off(cvt[2], [[22, 15]], 21))
            return h

        def mlp_block(t, h):
            # both MLP branches through 128x128 f32 blocks; PE transposes
            # with ones lanes via h col 95 and the saturated tanh col 127
            trm = ppps.tile([P, P], F32, tag="trm", name="trm")
            nc.tensor.transpose(trm, h, sb_id)
            hT = work.tile([P, P], F32, tag="hT", name="hT")
            nc.scalar.copy(out=hT, in_=trm)
            pp = ppps.tile([P, P], F32, tag="pp", name="pp")
            nc.tensor.matmul(pp, hT, sb_w1, start=True, stop=True)
            t1 = small.tile([P, P], F32, tag="t1", name="t1")
            nc.scalar.activation(out=t1, in_=pp, func=AF.Tanh)
            trm2 = ppps.tile([P, P], F32, tag="trm", name="trm2")
            nc.tensor.transpose(trm2, t1, sb_id)
            t1T = work.tile([P, P], F32, tag="t1T", name="t1T")
            nc.vector.tensor_copy(out=t1T, in_=trm2)
            p2 = ppps.tile([P, 90], F32, tag="pp", name="p2")
            nc.tensor.matmul(p2, t1T, sb_w2, start=True, stop=True)
            z2 = small.tile([P, 90], BF16, tag="z2", name="z2")
            nc.scalar.activation(out=z2, in_=p2, func=AF.Tanh)
            nc.vector.tensor_add(out=zall[:, t * C:(t + 1) * C],
                                 in0=z2[:, 0:45], in1=z2[:, 45:90])

        # software-pipelined emission: engines issue in program order, so
        # interleave conv(t+1) ahead of mlp(t) to let tiles overlap
        hprev = None
        for t in range(NT + 1):
            if t < NT:
                hcur = conv_block(t)
            if t >= 1:
                mlp_block(t - 1, hprev)
            hprev = hcur

        # batched softmax over all 8 tile blocks (segmented per block)
        zmax = singles.tile([P, NT], F32)
        nc.vector.tensor_reduce(out=zmax, in_=_ap(zall, [[C, NT], [1, C]]),
                                axis=AX.X, op=ALU.max)
        zmax16 = singles.tile([P, NT], BF16)
        nc.vector.tensor_copy(out=zmax16, in_=zmax)
        zsub = singles.tile([P, NT * C], BF16)
        nc.vector.tensor_sub(out=zsub, in0=zall,
                             in1=_ap(zmax16, [[1, NT], [0, C]]))
        eg = singles.tile([P, NT * C], BF16)
        nc.scalar.activation(out=eg, in_=zsub, func=AF.Exp)
        ssum = singles.tile([P, NT], F32)
        nc.vector.tensor_reduce(out=ssum, in_=_ap(eg, [[C, NT], [1, C]]),
                                axis=AX.X, op=ALU.add)
        rs = singles.tile([P, NT], BF16)
        nc.vector.reciprocal(out=rs, in_=ssum)
        nc.vector.tensor_mul(out=gate_all, in0=eg,
                             in1=_ap(rs, [[1, NT], [0, C]]))
        nc.sync.dma_start(out=gate_o[:, :], in_=gate_all)
    legalize_sync_waits(nc)
    return nc


# --------------------------------------------------------------------------
# Launch 2: expert attention via fitted moment polynomials
# --------------------------------------------------------------------------

def build_attn_program(degs):
    """degs: per-expert polynomial degree, sorted ascending (len 22)."""
    degs = list(degs)
    assert degs == sorted(degs)
    dmax = max(degs)
    # suffix start index for "experts with degree > k"
    estart = {k: next((i for i in range(E) if degs[i] > k), E)
              for k in range(dmax)}
    # first expert of each degree class (for acc initialization)
    class_start = {}
    for i, d in enumerate(degs):
        class_start.setdefault(d, i)

    nc = bass.Bass()
    u_d = nc.declare_dram_parameter("u", [BC, EL], BF16, isOutput=False)
    phi_d = nc.declare_dram_parameter("phi", [BC, EL], BF16, isOutput=False)
    gu_d = nc.declare_dram_parameter("gu", [BC, EL], BF16, isOutput=False)
    gpq_d = nc.declare_dram_parameter("gpq", [BC, 2 * E], BF16,
                                      isOutput=False)
    cd_d = nc.declare_dram_parameter("cd", [1, (NLEV - 1) * NT * E], BF16,
                                     isOutput=False)
    cn_d = nc.declare_dram_parameter("cn", [1, (NLEV - 1) * NT * E], BF16,
                                     isOutput=False)
    at_o = nc.declare_dram_parameter("at", [BC, EL], BF16, isOutput=True)
    gt_o = nc.declare_dram_parameter("gt", [BC, EL], BF16, isOutput=True)

    def shard_ap(dram, ncols):
        base = dram[:, :]
        return bass.AP(tensor=base.tensor, offset=base.offset,
                       ap=[[ncols, P], [P * ncols, NT], [1, ncols]])

    with tile.TileContext(nc) as tc, ExitStack() as ctx, \
            nc.allow_low_precision(reason="bf16 attn pipeline; 2e-2 tol"):
        sg = ctx.enter_context(tc.tile_pool(name="sg", bufs=1))

        gu = sg.tile([P, WF], BF16)
        nc.sync.dma_start(out=gu, in_=shard_ap(gu_d, EL))
        u = sg.tile([P, WF], BF16)
        nc.sync.dma_start(out=u, in_=shard_ap(u_d, EL))
        phi = sg.tile([P, WF], BF16)
        nc.sync.dma_start(out=phi, in_=shard_ap(phi_d, EL))
        gpq = sg.tile([P, NT * 2 * E], BF16)
        nc.sync.dma_start(out=gpq, in_=shard_ap(gpq_d, 2 * E))

        def bconst(dram, n, nm):
            base = dram[:, :]
            t = sg.tile([P, n], BF16, name=nm)
            nc.sync.dma_start(
                out=t, in_=bass.AP(tensor=base.tensor, offset=base.offset,
                                   ap=[[0, P], [1, n]]))
            return t

        NC_ = NT * E                      # 176 moment columns per level
        cdB = bconst(cd_d, (NLEV - 1) * NC_, "cdB")
        cnB = bconst(cn_d, (NLEV - 1) * NC_, "cnB")

        Wt = sg.tile([P, NLEV * NC_], BF16)   # moment levels W_0..W_7
        # levels 0..3 are fully written by reduces; only the sliced levels
        # need zeroing (their prefixes are read by the D/N scale muls)
        nc.vector.memset(Wt[:, 4 * NC_:NLEV * NC_], 0.0)

        # slice helpers (l-major: free = (t, l, e); e innermost)
        def full3(tile_, e0=0, ne=E, coloff=0):
            return _off(tile_, [[EL, NT], [E, L], [1, ne]], coloff + e0)

        def wout(lev, e0=0, ne=E):
            return _off(Wt, [[E, NT], [1, ne]], lev * NC_ + e0)

        def wred_in(src, e0=0, ne=E):
            return _off(src, [[EL, NT], [1, ne], [E, L]], e0)

        def coef(ctile, lev, e0=0, ne=E):
            return _off(ctile, [[E, NT], [0, L], [1, ne]], lev * NC_ + e0)

        # w = exp(gu); moments
        w = sg.tile([P, WF], BF16)
        nc.scalar.activation(out=w, in_=gu, func=AF.Exp)
        nc.vector.tensor_reduce(out=wout(0), in_=wred_in(w), axis=AX.X,
                                op=ALU.add)
        va = sg.tile([P, WF], BF16)
        vb = sg.tile([P, WF], BF16)
        cur, nxt = va, vb
        nc.vector.tensor_mul(out=cur, in0=w, in1=u)
        nc.vector.tensor_reduce(out=wout(1), in_=wred_in(cur), axis=AX.X,
                                op=ALU.add)
        for lev in range(2, NLEV):
            # moment level `lev` is needed by experts with degree >= lev-1
            e0 = estart.get(lev - 2, E)
            ne = E - e0
            if ne <= 0:
                break
            nc.vector.tensor_mul(out=full3(nxt, e0, ne),
                                 in0=full3(cur, e0, ne),
                                 in1=full3(u, e0, ne))
            nc.vector.tensor_reduce(out=wout(lev, e0, ne),
                                    in_=wred_in(nxt, e0, ne),
                                    axis=AX.X, op=ALU.add)
            cur, nxt = nxt, cur

        # Horner coefficient tensors: D_k = W_k*cd_k, N_k = W_{k+1}*cn_k
        Dt = sg.tile([P, (NLEV - 1) * NC_], BF16)
        nc.vector.tensor_mul(out=Dt, in0=Wt[:, 0:(NLEV - 1) * NC_], in1=cdB)
        Nt = sg.tile([P, (NLEV - 1) * NC_], BF16)
        nc.vector.tensor_mul(out=Nt, in0=Wt[:, NC_:NLEV * NC_], in1=cnB)

        # nested mixed-degree Horner (experts sorted by degree ascending);
        # a class's accumulator starts life fused into its first step:
        # acc = coef(d)*phi + coef(d-1)
        accd = sg.tile([P, WF], BF16)
        accn = sg.tile([P, WF], BF16)
        for k in range(dmax - 1, -1, -1):
            e0 = estart[k]
            ne = E - e0
            cs = class_start.get(k + 1)
            for acc, Ct in ((accd, Dt), (accn, Nt)):
                if cs is not None:
                    ncs = (min([s for dd, s in class_start.items()
                                if dd > k + 1], default=E)) - cs
                    nc.vector.tensor_mul(out=full3(acc, cs, ncs),
                                         in0=coef(Ct, k + 1, cs, ncs),
                                         in1=full3(phi, cs, ncs))
                    e1 = cs + ncs
                    if E - e1 > 0:
                        nc.vector.tensor_mul(out=full3(acc, e1, E - e1),
                                             in0=full3(acc, e1, E - e1),
                                             in1=full3(phi, e1, E - e1))
                else:
                    nc.vector.tensor_mul(out=full3(acc, e0, ne),
                                         in0=full3(acc, e0, ne),
                                         in1=full3(phi, e0, ne))
                nc.vector.tensor_add(out=full3(acc, e0, ne),
                                     in0=full3(acc, e0, ne),
                                     in1=coef(Ct, k, e0, ne))

        # s = num/den; at = s*gp + gq; gt = at*u
        # (tail ops split 16/6 experts across DVE and Pool so they overlap)
        rden = sg.tile([P, WF], BF16)
        nc.vector.reciprocal(out=rden, in_=accd)
        s = accn
        nc.vector.tensor_mul(out=s, in0=accn, in1=rden)
        at = accd
        nc.vector.tensor_mul(
            out=at, in0=s,
            in1=_ap(gpq, [[2 * E, NT], [0, L], [1, E]]))
        ESP = 16
        nc.vector.tensor_add(
            out=full3(at, 0, ESP), in0=full3(at, 0, ESP),
            in1=_off(gpq, [[2 * E, NT], [0, L], [1, ESP]], E))
        nc.gpsimd.tensor_add(
            out=full3(at, ESP, E - ESP), in0=full3(at, ESP, E - ESP),
            in1=_off(gpq, [[2 * E, NT], [0, L], [1, E - ESP]], E + ESP))
        gt = va
        nc.gpsimd.tensor_mul(out=full3(gt, ESP, E - ESP),
                             in0=full3(at, ESP, E - ESP),
                             in1=full3(u, ESP, E - ESP))
        nc.vector.tensor_mul(out=full3(gt, 0, ESP),
                             in0=full3(at, 0, ESP),
                             in1=full3(u, 0, ESP))
        nc.sync.dma_start(out=shard_ap(at_o, EL), in_=at)
        nc.sync.dma_start(out=shard_ap(gt_o, EL), in_=gt)
    legalize_sync_waits(nc)
    return nc


# --------------------------------------------------------------------------
# Host-side preparation
# --------------------------------------------------------------------------

def _gate_params(inputs):
    gc_w = inputs["gc_w"].astype(np.float64)
    gc_b = inputs["gc_b"].astype(np.float64)
    KC = P * NT
    # full conv weight: rows = (chan,l) + pad + bias lane, cols = (chan, 22)
    wfull = np.zeros((KC, 990), np.float32)
    wavvec = gc_w.mean(0)
    for i in range(C):
        wfull[i * L:(i + 1) * L, i * 22:i * 22 + L] = gc_w.T
        wfull[i * L:(i + 1) * L, i * 22 + L] = wavvec
        wfull[KC - 1, i * 22:i * 22 + L] = gc_b
        wfull[KC - 1, i * 22 + L] = gc_b.mean()
    PAIRS = [[0, 1, 2, 7], [2, 3, 4, 7], [4, 5, 6, 7]]
    wpk = np.zeros((P, 12 * 330), np.float32)
    for j in range(3):
        for i, c in enumerate(PAIRS[j]):
            wpk[:, (j * 4 + i) * 330:(j * 4 + i + 1) * 330] = \
                wfull[c * P:(c + 1) * P, j * 330:(j + 1) * 330]
    # 128x128 MLP layer-1 block: rows = transposed h cols (0:45 mx, 45:90
    # av, 95 = ones), out cols 0:50 = both branch hiddens, col 127 driven to
    # +30 via the ones row so tanh saturates to an exact 1.0 "ones" lane for
    # layer 2; all other cells zero.
    w1blk = np.zeros((128, 128), np.float32)
    w1blk[0:45, 0:25] = inputs["w1"].T
    w1blk[45:90, 25:50] = inputs["w1"].T
    w1blk[95, 0:25] = inputs["b1"]
    w1blk[95, 25:50] = inputs["b1"]
    w1blk[95, 127] = 30.0
    w2blk = np.zeros((128, 90), np.float32)
    w2blk[0:25, 0:45] = inputs["w2"].T
    w2blk[25:50, 45:90] = inputs["w2"].T
    w2blk[127, 0:45] = inputs["b2"]
    w2blk[127, 45:90] = inputs["b2"]
    return wpk, w1blk, w2blk


def _fit_exp(tsamp, K, wsamp=None, ntail=0.5):
    t = np.asarray(tsamp, np.float64)
    w = np.ones_like(t) if wsamp is None else np.asarray(wsamp, np.float64)
    tm = max(np.abs(t).max(), 1e-3)
    textra = np.linspace(-tm, tm, 64)
    t = np.concatenate([t, textra])
    w = np.concatenate([w, np.full(64, ntail * w.mean())])
    V = np.vander(t, K + 1, increasing=True) * w[:, None]
    c, *_ = np.linalg.lstsq(V, np.exp(t) * w, rcond=None)
    return c


_CACHE = {}


def kernel(**inputs):
    inputs = {k: np.ascontiguousarray(np.asarray(v)) for k, v in
              inputs.items()}
    x = inputs["x"].astype(np.float32)            # (B, C, L)
    import ml_dtypes
    bf = ml_dtypes.bfloat16

    wpk, w1blk, w2blk = _gate_params(inputs)
    cores = list(range(NCORES))
    KC = P * NT

    if "gate" not in _CACHE:
        _CACHE["gate"] = build_gate_program()
    nc1 = _CACHE["gate"]
    maps1 = []
    for i in cores:
        xt = np.zeros((KC, KC), np.float32)
        xt[0:CL, :] = x.reshape(B, CL)[i * BC:(i + 1) * BC].T
        xt[KC - 1, :] = 1.0
        maps1.append({"xT": xt.astype(bf), "wpk": wpk.astype(bf),
                      "w1blk": w1blk, "w2blk": w2blk,
                      "ident": np.eye(P, dtype=np.float32)})
    r1 = run_bass_kernel_spmd(nc1, maps1, cores).results
    # gate tiles come back as (P, NT*C): row p, block t -> batch p + t*P
    gate = np.zeros((B, C), np.float32)
    for i in cores:
        g = np.asarray(r1[i]["gate"]).astype(np.float32)
        gate[i * BC:(i + 1) * BC] = \
            g.reshape(P, NT, C).transpose(1, 0, 2).reshape(BC, C)
    mean_gate = gate.astype(np.float64).mean(0)
    sel = np.sort(np.argsort(-mean_gate, kind="stable")[:E])

    # expert scalars
    wq, bq = inputs["wq"], inputs["bq"]
    wk, bk = inputs["wk"], inputs["bk"]
    wv, bv = inputs["wv"], inputs["bv"]
    wo, bo = inputs["wo"], inputs["bo"]
    alpha = (wq * wk).sum(1).astype(np.float32)
    gamma = (bq * wk).sum(1).astype(np.float32)
    pv = (wo * wv).sum(1).astype(np.float32)
    qv = ((wo * bv).sum(1) + bo).astype(np.float32)

    usel = x[:, sel, :]                            # (B, E, L)
    # per-expert tau range -> degree ladder
    phimax = np.abs(alpha[None, :, None] * usel).max(axis=(0, 2))
    umax = np.abs(usel).max(axis=(0, 2))
    taumax = phimax * umax
    degs_raw = np.where(taumax <= 0.7, 2,
                        np.where(taumax <= 1.2, 3,
                                 np.where(taumax <= 2.0, 4, 6)))
    perm = np.argsort(degs_raw, kind="stable")     # experts by degree asc
    degs = degs_raw[perm]

    # coefficient fits per (permuted) expert
    rng = np.random.RandomState(12345)
    cd = np.zeros((NLEV - 1, E), np.float32)
    cn = np.zeros((NLEV - 1, E), np.float32)
    sub = usel[::16]                               # (B/16, E, L) samples
    for j, e in enumerate(perm):
        K = int(degs[j])
        ue = sub[:, e, :].astype(np.float64)
        tau = (alpha[e] * ue[:, :, None] * ue[:, None, :]).ravel()
        uw = np.abs(np.broadcast_to(ue[:, None, :], ue.shape[:1] + (L, L))
                    ).ravel()
        ss = rng.choice(tau.size, min(40000, tau.size), replace=False)
        cd[0:K + 1, j] = _fit_exp(tau[ss], K)
        cn[0:K + 1, j] = _fit_exp(tau[ss], K, wsamp=uw[ss] + 0.1)

    # device tensors (l-major, expert-permuted)
    uselp = usel[:, perm, :]
    u_lm = np.ascontiguousarray(uselp.transpose(0, 2, 1).reshape(B, EL))
    phi_lm = np.ascontiguousarray(
        (alpha[perm][None, :, None] * uselp).transpose(0, 2, 1)
        .reshape(B, EL))
    gu_lm = np.ascontiguousarray(
        (gamma[perm][None, :, None] * uselp).transpose(0, 2, 1)
        .reshape(B, EL))
    gsel = gate[:, sel][:, perm]
    gp = gsel * pv[perm][None, :]
    gq = gsel * qv[perm][None, :]
    gpq = np.concatenate([gp, gq], 1).astype(np.float32)   # (B, 44)

    NC_ = NT * E
    cd_full = np.tile(cd[:, None, :], (1, NT, 1)).reshape(1, (NLEV - 1) * NC_)
    cn_full = np.tile(cn[:, None, :], (1, NT, 1)).reshape(1, (NLEV - 1) * NC_)

    key = (tuple(int(d) for d in degs),)
    if _CACHE.get("attn_key") != key:
        _CACHE["attn"] = build_attn_program([int(d) for d in degs])
        _CACHE["attn_key"] = key
    nc2 = _CACHE["attn"]
    maps2 = [{"u": u_lm[i * BC:(i + 1) * BC].astype(bf),
              "phi": phi_lm[i * BC:(i + 1) * BC].astype(bf),
              "gu": gu_lm[i * BC:(i + 1) * BC].astype(bf),
              "gpq": gpq[i * BC:(i + 1) * BC].astype(bf),
              "cd": cd_full.astype(bf), "cn": cn_full.astype(bf)}
             for i in cores]
    r2 = run_bass_kernel_spmd(nc2, maps2, cores).results
    at = np.concatenate([np.asarray(r["at"]).astype(np.float32)
                         for r in r2], 0)          # (B, 462) l-major perm
    gt = np.concatenate([np.asarray(r["gt"]).astype(np.float32)
                         for r in r2], 0)

    inv = np.argsort(perm)
    at_e = at.reshape(B, L, E).transpose(0, 2, 1)[:, inv, :]   # (B,E,L)
    gt_e = gt.reshape(B, L, E).transpose(0, 2, 1)[:, inv, :]
    A_full = np.zeros((B, C, L), np.float32)
    G_full = np.zeros((B, C, L), np.float32)
    A_full[:, sel, :] = at_e
    G_full[:, sel, :] = gt_e
    return G_full.reshape(B, CL), A_full.reshape(B, CL)


# revision 54
# speedup vs baseline: 1.0259x; 1.0023x over previous
"""Trainium2 Bass kernel for grouped-attention MoE routing.

Math (derived from the nn.Module):
  gate  = softmax(mlp(maxpool(conv(x))) + mlp(avgpool(conv(x))))      (B,45)
  sel   = sorted(top22(mean_b gate))                                  (22,)
  Per expert e with u = x[:, sel[e], :]:
    energy[l,m] = (a_e*u_l + g_e) * u_m   (rank-1; scalars a,g from weights)
    attn = softmax_m(energy);  s_l = sum_m u_m attn[l,m]
    y_l  = P_e*s_l + Q_e;      A[:,sel[e],:] = y * gate[:,sel[e]]
  G = x * A (flat);  return (G, A_flat)

Implementation strategy (v2):
  Launch 1 (gate): bf16 PE conv with bias folded via a 127th ones row and
    the avg-pool folded in as extra matmul columns; both MLP branches run
    through one block-diagonal matmul pair; outputs batched into one DMA.
  Launch 2 (attn): the rank-1 softmax is evaluated without the LxL energy
    tensor.  With w = exp(g_e*u) and phi = a_e*u:
      den(phi_l) = sum_m w_m exp(phi_l u_m) ~= sum_k cd_k phi_l^k W_k
      num(phi_l) = sum_m u_m w_m exp(phi_l u_m) ~= sum_k cn_k phi_l^k W_{k+1}
    where W_k = sum_m w_m u_m^k are on-device moments and cd/cn are host-
    fitted per-expert polynomial coefficients (least squares over the
    empirical tau = phi*u range; numerator fit |u|-weighted).  s = num/den.
    Per-expert degree ladder (2..6) by empirical |tau| range; experts are
    permuted so degree classes are contiguous and the nested Horner only
    touches suffix slices for the high degrees.  Everything bf16, l-major
    (l outer, e inner) so per-(b,e) coefficient broadcasts stay packed.
  Routing (45-float mean-gate reduction) is mediated on host between the
  two launches, equivalent to the all-reduce in the sharding hint.
"""

import math
import numpy as np
from contextlib import ExitStack

import bass_rust
import concourse.bass as bass
import concourse.mybir as mybir
import concourse.tile as tile
from concourse.bass_utils import run_bass_kernel_spmd

_MULTIWAIT_OK = ("InstNoOp", "InstAllEngineBarrier",
                 "InstEventSemaphore", "InstUnconditionalBranch")


def legalize_sync_waits(nc):
    """walrus codegen on this stack rejects >1 sync wait on most
    instructions; hoist extra waits onto same-engine NoOps."""
    for func in nc.m.functions:
        for block in func.blocks:
            il = block.instructions
            out = []
            for inst in il:
                tname = type(inst).__name__
                si = getattr(inst, "sync_info", None)
                waits = list(si.on_wait) if si is not None else []
                if tname not in _MULTIWAIT_OK and len(waits) > 1:
                    for k, w in enumerate(waits):
                        nop = mybir.InstNoOp(
                            name=f"{inst.name}-synop{k}", ins=[], outs=[])
                        nop.engine = inst.engine
                        nop.sync_info = bass_rust.SyncInfo(
                            on_wait=[w], on_update=[])
                        out.append(nop)
                    inst.sync_info = bass_rust.SyncInfo(
                        on_wait=[], on_update=list(inst.sync_info.on_update))
                out.append(inst)
            il.clear()
            il.extend(out)


B, C, L, E = 8192, 45, 21, 22
NCORES = 8
BC = B // NCORES          # rows per core
P = 128                   # SBUF partitions
NT = BC // P              # batch tiles per core
CL = C * L                # 945
EL = E * L                # 462
WF = NT * EL              # 3696 full-shard free width (t, l, e) l-major
F32 = mybir.dt.float32
BF16 = mybir.dt.bfloat16
AF = mybir.ActivationFunctionType
ALU = mybir.AluOpType
AX = mybir.AxisListType

NG = 8                    # conv channel groups (6,6,...,3 channels)
GCH = [list(range(g, min(g + 6, C))) for g in range(0, C, 6)]
NLEV = 8                  # moment levels W_0..W_7 held on device
DEG_MAX = 6


def _ap(base, extra_free):
    """Custom free-dim access pattern on an SBUF tile slice (partition
    dim kept from `base`)."""
    return bass.AP(tensor=base.tensor, offset=base.offset,
                   ap=[base.ap[0]] + extra_free)


def _off(base, extra_free, col_off):
    ap = bass.AP(tensor=base.tensor, offset=base.offset,
                 ap=[base.ap[0]] + extra_free)
    ap.offset = ap.offset + col_off
    return ap


# --------------------------------------------------------------------------
# Launch 1: gating network
# --------------------------------------------------------------------------

def build_gate_program():
    """Gate launch. Host uploads x TRANSPOSED and padded: XT (1024, 1024)
    with rows 0:945 = x[coreshard].T (row = (chan,l)), rows 945:1023 = 0,
    row 1023 = 1 (bias lane).  The conv+avg matmul accumulates chunk-wise
    into 3 PSUM column blocks of 15 channels x 22 outputs; no on-device
    transposes or PSUM->SBUF copies are needed for the conv at all."""
    nc = bass.Bass()
    KC = P * NT                       # 1024 padded contraction rows
    xT = nc.declare_dram_parameter("xT", [KC, KC], BF16, isOutput=False)
    # packed rhs slices for the 12 (chunk, block) matmuls: (128, 12*330)
    wpk = nc.declare_dram_parameter("wpk", [P, 12 * 330], BF16,
                                    isOutput=False)
    w1blk = nc.declare_dram_parameter("w1blk", [P, P], F32, isOutput=False)
    w2blk = nc.declare_dram_parameter("w2blk", [P, 90], F32, isOutput=False)
    ident = nc.declare_dram_parameter("ident", [P, P], F32, isOutput=False)
    gate_o = nc.declare_dram_parameter("gate", [P, NT * C], BF16,
                                       isOutput=True)

    # (chunk, block) pairs: block j covers channels 15j..15j+14 =
    # contraction rows 315j..315j+314, plus the bias lane in chunk 7
    PAIRS = [[0, 1, 2, 7], [2, 3, 4, 7], [4, 5, 6, 7]]

    with tile.TileContext(nc) as tc, ExitStack() as ctx, \
            nc.allow_low_precision(reason="bf16 gate pipeline; 2e-2 tol"):
        singles = ctx.enter_context(tc.tile_pool(name="singles", bufs=1))
        cvps = ctx.enter_context(tc.tile_pool(name="cvps", bufs=2,
                                              space="PSUM"))
        ppps = ctx.enter_context(tc.tile_pool(name="ppps", bufs=1,
                                              space="PSUM"))
        work = ctx.enter_context(tc.tile_pool(name="work", bufs=2))
        small = ctx.enter_context(tc.tile_pool(name="small", bufs=3))

        def dve_const(dram, p, n, dt=BF16):
            raw = singles.tile([p, n], dt, name="raw_" + dram.name)
            nc.sync.dma_start(out=raw, in_=dram[:, :])
            t = singles.tile([p, n], dt, name="sb_" + dram.name)
            nc.vector.tensor_copy(out=t, in_=raw)
            return t

        sb_w = dve_const(wpk, P, 12 * 330)

        # xT loads: per-tile DMA of the (128, 8x128) lhsT panel so tile 0
        # can start early.  lhs[p, c*128+b] = XT[128c+p, 128t+b]
        xb = xT[:, :]
        lhsT = []

        def load_lhsT(t):
            lt = singles.tile([P, NT * P], BF16, name=f"lhsT{t}")
            ap = bass.AP(tensor=xb.tensor, offset=xb.offset,
                         ap=[[KC, P], [P * KC, NT], [1, P]])
            ap.offset = ap.offset + t * P
            nc.sync.dma_start(out=lt, in_=ap)
            lhsT.append(lt)

        load_lhsT(0)
        load_lhsT(1)
        sb_w1 = dve_const(w1blk, P, P, F32)
        sb_w2 = dve_const(w2blk, P, 90, F32)
        sb_id = dve_const(ident, P, P, F32)
        for t in range(2, NT):
            load_lhsT(t)

        # persistent h tiles: cols 0:90 rewritten each use; 90:128 junk is
        # zeroed once (col 95 = ones lane feeding the layer-1 bias row)
        hb = []
        for i in range(2):
            t = singles.tile([P, P], F32, name=f"hb{i}")
            nc.vector.memset(t[:, 90:P], 0.0)
            nc.vector.memset(t[:, 95:96], 1.0)
            hb.append(t)

        gate_all = singles.tile([P, NT * C], BF16)
        zall = singles.tile([P, NT * C], BF16)

        def conv_block(t):
            lt = lhsT[t]
            cvt = [cvps.tile([P, 330], F32, tag=f"cv{j}", name=f"cv{j}")
                   for j in range(3)]
            for j in range(3):
                for i, c in enumerate(PAIRS[j]):
                    nc.tensor.matmul(
                        cvt[j], lt[:, c * P:(c + 1) * P],
                        sb_w[:, (j * 4 + i) * 330:(j * 4 + i + 1) * 330],
                        start=(i == 0), stop=(i == len(PAIRS[j]) - 1))
            h = hb[t % 2]
            for j in range(3):
                nc.vector.tensor_reduce(
                    out=h[:, 15 * j:15 * j + 15],
                    in_=_ap(cvt[j][:, 0:330], [[22, 15], [1, L]]),
                    axis=AX.X, op=ALU.max)
            # avg lanes (col 21 of each 22-block): 2 on DVE, 1 on Act
            nc.vector.tensor_copy(out=h[:, 45:60],
                                  in_=_off(cvt[0], [[22, 15]], 21))
            nc.scalar.copy(out=h[:, 60:75], in_=_off(cvt[1], [[22, 15]], 21))
            nc.vector.tensor_copy(out=h[:, 75:90],
                                  in_=_off(cvt[2], [[22, 15]], 21))
            return h

        def mlp_block(t, h):
            # both MLP branches through 128x128 f32 blocks; PE transposes
            # with ones lanes via h col 95 and the saturated tanh col 127
            trm = ppps.tile([P, P], F32, tag="trm", name="trm")
            nc.tensor.transpose(trm, h, sb_id)
            hT = work.tile([P, P], F32, tag="hT", name="hT")
            nc.scalar.copy(out=hT, in_=trm)
            pp = ppps.tile([P, P], F32, tag="pp", name="pp")
            nc.tensor.matmul(pp, hT, sb_w1, start=True, stop=True)
            t1 = small.tile([P, P], F32, tag="t1", name="t1")
            nc.scalar.activation(out=t1, in_=pp, func=AF.Tanh)
            trm2 = ppps.tile([P, P], F32, tag="trm", name="trm2")
            nc.tensor.transpose(trm2, t1, sb_id)
            t1T = work.tile([P, P], F32, tag="t1T", name="t1T")
            nc.vector.tensor_copy(out=t1T, in_=trm2)
            p2 = ppps.tile([P, 90], F32, tag="pp", name="p2")
            nc.tensor.matmul(p2, t1T, sb_w2, start=True, stop=True)
            z2 = small.tile([P, 90], BF16, tag="z2", name="z2")
            nc.scalar.activation(out=z2, in_=p2, func=AF.Tanh)
            nc.vector.tensor_add(out=zall[:, t * C:(t + 1) * C],
                                 in0=z2[:, 0:45], in1=z2[:, 45:90])

        def softmax_half(lo, nt):
            # segmented softmax over tile blocks [lo, lo+nt) + output DMA
            zs = zall[:, lo * C:(lo + nt) * C]
            zmax = small.tile([P, nt], F32, tag="zmax", name="zmax")
            nc.vector.tensor_reduce(out=zmax, in_=_ap(zs, [[C, nt], [1, C]]),
                                    axis=AX.X, op=ALU.max)
            zmax16 = small.tile([P, nt], BF16, tag="zmax16", name="zmax16")
            nc.vector.tensor_copy(out=zmax16, in_=zmax)
            zsub = work.tile([P, nt * C], BF16, tag="zsub", name="zsub")
            nc.vector.tensor_sub(out=zsub, in0=zs,
                                 in1=_ap(zmax16, [[1, nt], [0, C]]))
            eg = work.tile([P, nt * C], BF16, tag="eg", name="eg")
            nc.scalar.activation(out=eg, in_=zsub, func=AF.Exp)
            ssum = small.tile([P, nt], F32, tag="ssum", name="ssum")
            nc.vector.tensor_reduce(out=ssum, in_=_ap(eg, [[C, nt], [1, C]]),
                                    axis=AX.X, op=ALU.add)
            rs = small.tile([P, nt], BF16, tag="rs", name="rs")
            nc.vector.reciprocal(out=rs, in_=ssum)
            gs = gate_all[:, lo * C:(lo + nt) * C]
            nc.vector.tensor_mul(out=gs, in0=eg,
                                 in1=_ap(rs, [[1, nt], [0, C]]))
            nc.sync.dma_start(out=gate_o[:, lo * C:(lo + nt) * C], in_=gs)

        # software-pipelined emission: engines issue in program order, so
        # interleave conv(t+1) ahead of mlp(t) to let tiles overlap; the
        # first softmax half runs under the back half of the pipeline
        hprev = None
        for t in range(NT + 1):
            if t < NT:
                hcur = conv_block(t)
            if t >= 1:
                mlp_block(t - 1, hprev)
            if t == NT // 2 + 1:
                softmax_half(0, NT // 2)
            hprev = hcur
        softmax_half(NT // 2, NT // 2)
    legalize_sync_waits(nc)
    return nc


# --------------------------------------------------------------------------
# Launch 2: expert attention via fitted moment polynomials
# --------------------------------------------------------------------------

def build_attn_program(degs):
    """degs: per-expert polynomial degree, sorted ascending (len 22)."""
    degs = list(degs)
    assert degs == sorted(degs)
    dmax = max(degs)
    # suffix start index for "experts with degree > k"
    estart = {k: next((i for i in range(E) if degs[i] > k), E)
              for k in range(dmax)}
    # first expert of each degree class (for acc initialization)
    class_start = {}
    for i, d in enumerate(degs):
        class_start.setdefault(d, i)

    nc = bass.Bass()
    u_d = nc.declare_dram_parameter("u", [BC, EL], BF16, isOutput=False)
    phi_d = nc.declare_dram_parameter("phi", [BC, EL], BF16, isOutput=False)
    gu_d = nc.declare_dram_parameter("gu", [BC, EL], BF16, isOutput=False)
    gpq_d = nc.declare_dram_parameter("gpq", [BC, 2 * E], BF16,
                                      isOutput=False)
    cd_d = nc.declare_dram_parameter("cd", [1, (NLEV - 1) * NT * E], BF16,
                                     isOutput=False)
    cn_d = nc.declare_dram_parameter("cn", [1, (NLEV - 1) * NT * E], BF16,
                                     isOutput=False)
    at_o = nc.declare_dram_parameter("at", [BC, EL], BF16, isOutput=True)
    gt_o = nc.declare_dram_parameter("gt", [BC, EL], BF16, isOutput=True)

    def shard_ap(dram, ncols):
        base = dram[:, :]
        return bass.AP(tensor=base.tensor, offset=base.offset,
                       ap=[[ncols, P], [P * ncols, NT], [1, ncols]])

    with tile.TileContext(nc) as tc, ExitStack() as ctx, \
            nc.allow_low_precision(reason="bf16 attn pipeline; 2e-2 tol"):
        sg = ctx.enter_context(tc.tile_pool(name="sg", bufs=1))

        gu = sg.tile([P, WF], BF16)
        nc.sync.dma_start(out=gu, in_=shard_ap(gu_d, EL))
        u = sg.tile([P, WF], BF16)
        nc.sync.dma_start(out=u, in_=shard_ap(u_d, EL))
        phi = sg.tile([P, WF], BF16)
        nc.sync.dma_start(out=phi, in_=shard_ap(phi_d, EL))
        gpq = sg.tile([P, NT * 2 * E], BF16)
        nc.sync.dma_start(out=gpq, in_=shard_ap(gpq_d, 2 * E))

        def bconst(dram, n, nm):
            base = dram[:, :]
            t = sg.tile([P, n], BF16, name=nm)
            nc.sync.dma_start(
                out=t, in_=bass.AP(tensor=base.tensor, offset=base.offset,
                                   ap=[[0, P], [1, n]]))
            return t

        NC_ = NT * E                      # 176 moment columns per level
        cdB = bconst(cd_d, (NLEV - 1) * NC_, "cdB")
        cnB = bconst(cn_d, (NLEV - 1) * NC_, "cnB")

        Wt = sg.tile([P, NLEV * NC_], BF16)   # moment levels W_0..W_7
        # levels 0..3 are fully written by reduces; only the sliced levels
        # need zeroing (their prefixes are read by the D/N scale muls)
        nc.vector.memset(Wt[:, 4 * NC_:NLEV * NC_], 0.0)

        # slice helpers (l-major: free = (t, l, e); e innermost)
        def full3(tile_, e0=0, ne=E, coloff=0):
            return _off(tile_, [[EL, NT], [E, L], [1, ne]], coloff + e0)

        def wout(lev, e0=0, ne=E):
            return _off(Wt, [[E, NT], [1, ne]], lev * NC_ + e0)

        def wred_in(src, e0=0, ne=E):
            return _off(src, [[EL, NT], [1, ne], [E, L]], e0)

        def coef(ctile, lev, e0=0, ne=E):
            return _off(ctile, [[E, NT], [0, L], [1, ne]], lev * NC_ + e0)

        # w = exp(gu); moments
        w = sg.tile([P, WF], BF16)
        nc.scalar.activation(out=w, in_=gu, func=AF.Exp)
        nc.vector.tensor_reduce(out=wout(0), in_=wred_in(w), axis=AX.X,
                                op=ALU.add)
        va = sg.tile([P, WF], BF16)
        vb = sg.tile([P, WF], BF16)
        cur, nxt = va, vb
        nc.vector.tensor_mul(out=cur, in0=w, in1=u)
        nc.vector.tensor_reduce(out=wout(1), in_=wred_in(cur), axis=AX.X,
                                op=ALU.add)
        for lev in range(2, NLEV):
            # moment level `lev` is needed by experts with degree >= lev-1
            e0 = estart.get(lev - 2, E)
            ne = E - e0
            if ne <= 0:
                break
            nc.vector.tensor_mul(out=full3(nxt, e0, ne),
                                 in0=full3(cur, e0, ne),
                                 in1=full3(u, e0, ne))
            nc.vector.tensor_reduce(out=wout(lev, e0, ne),
                                    in_=wred_in(nxt, e0, ne),
                                    axis=AX.X, op=ALU.add)
            cur, nxt = nxt, cur

        # Horner coefficient tensors: D_k = W_k*cd_k, N_k = W_{k+1}*cn_k
        Dt = sg.tile([P, (NLEV - 1) * NC_], BF16)
        nc.vector.tensor_mul(out=Dt, in0=Wt[:, 0:(NLEV - 1) * NC_], in1=cdB)
        Nt = sg.tile([P, (NLEV - 1) * NC_], BF16)
        nc.vector.tensor_mul(out=Nt, in0=Wt[:, NC_:NLEV * NC_], in1=cnB)

        # nested mixed-degree Horner (experts sorted by degree ascending);
        # a class's accumulator starts life fused into its first step:
        # acc = coef(d)*phi + coef(d-1)
        accd = sg.tile([P, WF], BF16)
        accn = sg.tile([P, WF], BF16)
        for k in range(dmax - 1, -1, -1):
            e0 = estart[k]
            ne = E - e0
            cs = class_start.get(k + 1)
            for acc, Ct in ((accd, Dt), (accn, Nt)):
                if cs is not None:
                    ncs = (min([s for dd, s in class_start.items()
                                if dd > k + 1], default=E)) - cs
                    nc.vector.tensor_mul(out=full3(acc, cs, ncs),
                                         in0=coef(Ct, k + 1, cs, ncs),
                                         in1=full3(phi, cs, ncs))
                    e1 = cs + ncs
                    if E - e1 > 0:
                        nc.vector.tensor_mul(out=full3(acc, e1, E - e1),
                                             in0=full3(acc, e1, E - e1),
                                             in1=full3(phi, e1, E - e1))
                else:
                    nc.vector.tensor_mul(out=full3(acc, e0, ne),
                                         in0=full3(acc, e0, ne),
                                         in1=full3(phi, e0, ne))
                nc.vector.tensor_add(out=full3(acc, e0, ne),
                                     in0=full3(acc, e0, ne),
                                     in1=coef(Ct, k, e0, ne))

        # s = num/den; at = s*gp + gq; gt = at*u
        # (tail ops split 16/6 experts across DVE and Pool so they overlap)
        rden = sg.tile([P, WF], BF16)
        nc.vector.reciprocal(out=rden, in_=accd)
        s = accn
        nc.vector.tensor_mul(out=s, in0=accn, in1=rden)
        at = accd
        nc.vector.tensor_mul(
            out=at, in0=s,
            in1=_ap(gpq, [[2 * E, NT], [0, L], [1, E]]))
        ESP = 16
        nc.vector.tensor_add(
            out=full3(at, 0, ESP), in0=full3(at, 0, ESP),
            in1=_off(gpq, [[2 * E, NT], [0, L], [1, ESP]], E))
        nc.gpsimd.tensor_add(
            out=full3(at, ESP, E - ESP), in0=full3(at, ESP, E - ESP),
            in1=_off(gpq, [[2 * E, NT], [0, L], [1, E - ESP]], E + ESP))
        gt = va
        nc.gpsimd.tensor_mul(out=full3(gt, ESP, E - ESP),
                             in0=full3(at, ESP, E - ESP),
                             in1=full3(u, ESP, E - ESP))
        nc.vector.tensor_mul(out=full3(gt, 0, ESP),
                             in0=full3(at, 0, ESP),
                             in1=full3(u, 0, ESP))
        nc.sync.dma_start(out=shard_ap(at_o, EL), in_=at)
        nc.sync.dma_start(out=shard_ap(gt_o, EL), in_=gt)
    legalize_sync_waits(nc)
    return nc


# --------------------------------------------------------------------------
# Host-side preparation
# --------------------------------------------------------------------------

def _gate_params(inputs):
    gc_w = inputs["gc_w"].astype(np.float64)
    gc_b = inputs["gc_b"].astype(np.float64)
    KC = P * NT
    # full conv weight: rows = (chan,l) + pad + bias lane, cols = (chan, 22)
    wfull = np.zeros((KC, 990), np.float32)
    wavvec = gc_w.mean(0)
    for i in range(C):
        wfull[i * L:(i + 1) * L, i * 22:i * 22 + L] = gc_w.T
        wfull[i * L:(i + 1) * L, i * 22 + L] = wavvec
        wfull[KC - 1, i * 22:i * 22 + L] = gc_b
        wfull[KC - 1, i * 22 + L] = gc_b.mean()
    PAIRS = [[0, 1, 2, 7], [2, 3, 4, 7], [4, 5, 6, 7]]
    wpk = np.zeros((P, 12 * 330), np.float32)
    for j in range(3):
        for i, c in enumerate(PAIRS[j]):
            wpk[:, (j * 4 + i) * 330:(j * 4 + i + 1) * 330] = \
                wfull[c * P:(c + 1) * P, j * 330:(j + 1) * 330]
    # 128x128 MLP layer-1 block: rows = transposed h cols (0:45 mx, 45:90
    # av, 95 = ones), out cols 0:50 = both branch hiddens, col 127 driven to
    # +30 via the ones row so tanh saturates to an exact 1.0 "ones" lane for
    # layer 2; all other cells zero.
    w1blk = np.zeros((128, 128), np.float32)
    w1blk[0:45, 0:25] = inputs["w1"].T
    w1blk[45:90, 25:50] = inputs["w1"].T
    w1blk[95, 0:25] = inputs["b1"]
    w1blk[95, 25:50] = inputs["b1"]
    w1blk[95, 127] = 30.0
    w2blk = np.zeros((128, 90), np.float32)
    w2blk[0:25, 0:45] = inputs["w2"].T
    w2blk[25:50, 45:90] = inputs["w2"].T
    w2blk[127, 0:45] = inputs["b2"]
    w2blk[127, 45:90] = inputs["b2"]
    return wpk, w1blk, w2blk


def _fit_exp(tsamp, K, wsamp=None, ntail=0.5):
    t = np.asarray(tsamp, np.float64)
    w = np.ones_like(t) if wsamp is None else np.asarray(wsamp, np.float64)
    tm = max(np.abs(t).max(), 1e-3)
    textra = np.linspace(-tm, tm, 64)
    t = np.concatenate([t, textra])
    w = np.concatenate([w, np.full(64, ntail * w.mean())])
    V = np.vander(t, K + 1, increasing=True) * w[:, None]
    c, *_ = np.linalg.lstsq(V, np.exp(t) * w, rcond=None)
    return c


_CACHE = {}


def kernel(**inputs):
    inputs = {k: np.ascontiguousarray(np.asarray(v)) for k, v in
              inputs.items()}
    x = inputs["x"].astype(np.float32)            # (B, C, L)
    import ml_dtypes
    bf = ml_dtypes.bfloat16

    wpk, w1blk, w2blk = _gate_params(inputs)
    cores = list(range(NCORES))
    KC = P * NT

    if "gate" not in _CACHE:
        _CACHE["gate"] = build_gate_program()
    nc1 = _CACHE["gate"]
    maps1 = []
    for i in cores:
        xt = np.zeros((KC, KC), np.float32)
        xt[0:CL, :] = x.reshape(B, CL)[i * BC:(i + 1) * BC].T
        xt[KC - 1, :] = 1.0
        maps1.append({"xT": xt.astype(bf), "wpk": wpk.astype(bf),
                      "w1blk": w1blk, "w2blk": w2blk,
                      "ident": np.eye(P, dtype=np.float32)})
    r1 = run_bass_kernel_spmd(nc1, maps1, cores).results
    # gate tiles come back as (P, NT*C): row p, block t -> batch p + t*P
    gate = np.zeros((B, C), np.float32)
    for i in cores:
        g = np.asarray(r1[i]["gate"]).astype(np.float32)
        gate[i * BC:(i + 1) * BC] = \
            g.reshape(P, NT, C).transpose(1, 0, 2).reshape(BC, C)
    mean_gate = gate.astype(np.float64).mean(0)
    sel = np.sort(np.argsort(-mean_gate, kind="stable")[:E])

    # expert scalars
    wq, bq = inputs["wq"], inputs["bq"]
    wk, bk = inputs["wk"], inputs["bk"]
    wv, bv = inputs["wv"], inputs["bv"]
    wo, bo = inputs["wo"], inputs["bo"]
    alpha = (wq * wk).sum(1).astype(np.float32)
    gamma = (bq * wk).sum(1).astype(np.float32)
    pv = (wo * wv).sum(1).astype(np.float32)
    qv = ((wo * bv).sum(1) + bo).astype(np.float32)

    usel = x[:, sel, :]                            # (B, E, L)
    # per-expert tau range -> degree ladder
    phimax = np.abs(alpha[None, :, None] * usel).max(axis=(0, 2))
    umax = np.abs(usel).max(axis=(0, 2))
    taumax = phimax * umax
    degs_raw = np.where(taumax <= 0.7, 2,
                        np.where(taumax <= 1.2, 3,
                                 np.where(taumax <= 2.0, 4, 6)))
    perm = np.argsort(degs_raw, kind="stable")     # experts by degree asc
    degs = degs_raw[perm]

    # coefficient fits per (permuted) expert
    rng = np.random.RandomState(12345)
    cd = np.zeros((NLEV - 1, E), np.float32)
    cn = np.zeros((NLEV - 1, E), np.float32)
    sub = usel[::16]                               # (B/16, E, L) samples
    for j, e in enumerate(perm):
        K = int(degs[j])
        ue = sub[:, e, :].astype(np.float64)
        tau = (alpha[e] * ue[:, :, None] * ue[:, None, :]).ravel()
        uw = np.abs(np.broadcast_to(ue[:, None, :], ue.shape[:1] + (L, L))
                    ).ravel()
        ss = rng.choice(tau.size, min(40000, tau.size), replace=False)
        cd[0:K + 1, j] = _fit_exp(tau[ss], K)
        cn[0:K + 1, j] = _fit_exp(tau[ss], K, wsamp=uw[ss] + 0.1)

    # device tensors (l-major, expert-permuted)
    uselp = usel[:, perm, :]
    u_lm = np.ascontiguousarray(uselp.transpose(0, 2, 1).reshape(B, EL))
    phi_lm = np.ascontiguousarray(
        (alpha[perm][None, :, None] * uselp).transpose(0, 2, 1)
        .reshape(B, EL))
    gu_lm = np.ascontiguousarray(
        (gamma[perm][None, :, None] * uselp).transpose(0, 2, 1)
        .reshape(B, EL))
    gsel = gate[:, sel][:, perm]
    gp = gsel * pv[perm][None, :]
    gq = gsel * qv[perm][None, :]
    gpq = np.concatenate([gp, gq], 1).astype(np.float32)   # (B, 44)

    NC_ = NT * E
    cd_full = np.tile(cd[:, None, :], (1, NT, 1)).reshape(1, (NLEV - 1) * NC_)
    cn_full = np.tile(cn[:, None, :], (1, NT, 1)).reshape(1, (NLEV - 1) * NC_)

    key = (tuple(int(d) for d in degs),)
    if _CACHE.get("attn_key") != key:
        _CACHE["attn"] = build_attn_program([int(d) for d in degs])
        _CACHE["attn_key"] = key
    nc2 = _CACHE["attn"]
    maps2 = [{"u": u_lm[i * BC:(i + 1) * BC].astype(bf),
              "phi": phi_lm[i * BC:(i + 1) * BC].astype(bf),
              "gu": gu_lm[i * BC:(i + 1) * BC].astype(bf),
              "gpq": gpq[i * BC:(i + 1) * BC].astype(bf),
              "cd": cd_full.astype(bf), "cn": cn_full.astype(bf)}
             for i in cores]
    r2 = run_bass_kernel_spmd(nc2, maps2, cores).results
    at = np.concatenate([np.asarray(r["at"]).astype(np.float32)
                         for r in r2], 0)          # (B, 462) l-major perm
    gt = np.concatenate([np.asarray(r["gt"]).astype(np.float32)
                         for r in r2], 0)

    inv = np.argsort(perm)
    at_e = at.reshape(B, L, E).transpose(0, 2, 1)[:, inv, :]   # (B,E,L)
    gt_e = gt.reshape(B, L, E).transpose(0, 2, 1)[:, inv, :]
    A_full = np.zeros((B, C, L), np.float32)
    G_full = np.zeros((B, C, L), np.float32)
    A_full[:, sel, :] = at_e
    G_full[:, sel, :] = gt_e
    return G_full.reshape(B, CL), A_full.reshape(B, CL)


# revision 56
# speedup vs baseline: 1.0568x; 1.0302x over previous
"""Trainium2 Bass kernel for grouped-attention MoE routing.

Math (derived from the nn.Module):
  gate  = softmax(mlp(maxpool(conv(x))) + mlp(avgpool(conv(x))))      (B,45)
  sel   = sorted(top22(mean_b gate))                                  (22,)
  Per expert e with u = x[:, sel[e], :]:
    energy[l,m] = (a_e*u_l + g_e) * u_m   (rank-1; scalars a,g from weights)
    attn = softmax_m(energy);  s_l = sum_m u_m attn[l,m]
    y_l  = P_e*s_l + Q_e;      A[:,sel[e],:] = y * gate[:,sel[e]]
  G = x * A (flat);  return (G, A_flat)

Implementation strategy (v2):
  Launch 1 (gate): bf16 PE conv with bias folded via a 127th ones row and
    the avg-pool folded in as extra matmul columns; both MLP branches run
    through one block-diagonal matmul pair; outputs batched into one DMA.
  Launch 2 (attn): the rank-1 softmax is evaluated without the LxL energy
    tensor.  With w = exp(g_e*u) and phi = a_e*u:
      den(phi_l) = sum_m w_m exp(phi_l u_m) ~= sum_k cd_k phi_l^k W_k
      num(phi_l) = sum_m u_m w_m exp(phi_l u_m) ~= sum_k cn_k phi_l^k W_{k+1}
    where W_k = sum_m w_m u_m^k are on-device moments and cd/cn are host-
    fitted per-expert polynomial coefficients (least squares over the
    empirical tau = phi*u range; numerator fit |u|-weighted).  s = num/den.
    Per-expert degree ladder (2..6) by empirical |tau| range; experts are
    permuted so degree classes are contiguous and the nested Horner only
    touches suffix slices for the high degrees.  Everything bf16, l-major
    (l outer, e inner) so per-(b,e) coefficient broadcasts stay packed.
  Routing (45-float mean-gate reduction) is mediated on host between the
  two launches, equivalent to the all-reduce in the sharding hint.
"""

import math
import numpy as np
from contextlib import ExitStack

import bass_rust
import concourse.bass as bass
import concourse.mybir as mybir
import concourse.tile as tile
from concourse.bass_utils import run_bass_kernel_spmd

_MULTIWAIT_OK = ("InstNoOp", "InstAllEngineBarrier",
                 "InstEventSemaphore", "InstUnconditionalBranch")


def legalize_sync_waits(nc):
    """walrus codegen on this stack rejects >1 sync wait on most
    instructions; hoist extra waits onto same-engine NoOps."""
    for func in nc.m.functions:
        for block in func.blocks:
            il = block.instructions
            out = []
            for inst in il:
                tname = type(inst).__name__
                si = getattr(inst, "sync_info", None)
                waits = list(si.on_wait) if si is not None else []
                if tname not in _MULTIWAIT_OK and len(waits) > 1:
                    for k, w in enumerate(waits):
                        nop = mybir.InstNoOp(
                            name=f"{inst.name}-synop{k}", ins=[], outs=[])
                        nop.engine = inst.engine
                        nop.sync_info = bass_rust.SyncInfo(
                            on_wait=[w], on_update=[])
                        out.append(nop)
                    inst.sync_info = bass_rust.SyncInfo(
                        on_wait=[], on_update=list(inst.sync_info.on_update))
                out.append(inst)
            il.clear()
            il.extend(out)


B, C, L, E = 8192, 45, 21, 22
NCORES = 8
BC = B // NCORES          # rows per core
P = 128                   # SBUF partitions
NT = BC // P              # batch tiles per core
CL = C * L                # 945
EL = E * L                # 462
WF = NT * EL              # 3696 full-shard free width (t, l, e) l-major
F32 = mybir.dt.float32
BF16 = mybir.dt.bfloat16
AF = mybir.ActivationFunctionType
ALU = mybir.AluOpType
AX = mybir.AxisListType

NG = 8                    # conv channel groups (6,6,...,3 channels)
GCH = [list(range(g, min(g + 6, C))) for g in range(0, C, 6)]
NLEV = 8                  # moment levels W_0..W_7 held on device
DEG_MAX = 6


def _ap(base, extra_free):
    """Custom free-dim access pattern on an SBUF tile slice (partition
    dim kept from `base`)."""
    return bass.AP(tensor=base.tensor, offset=base.offset,
                   ap=[base.ap[0]] + extra_free)


def _off(base, extra_free, col_off):
    ap = bass.AP(tensor=base.tensor, offset=base.offset,
                 ap=[base.ap[0]] + extra_free)
    ap.offset = ap.offset + col_off
    return ap


# --------------------------------------------------------------------------
# Launch 1: gating network
# --------------------------------------------------------------------------

def build_gate_program():
    """Gate launch. Host uploads x TRANSPOSED and padded: XT (1024, 1024)
    with rows 0:945 = x[coreshard].T (row = (chan,l)), rows 945:1023 = 0,
    row 1023 = 1 (bias lane).  The conv+avg matmul accumulates chunk-wise
    into 3 PSUM column blocks of 15 channels x 22 outputs; no on-device
    transposes or PSUM->SBUF copies are needed for the conv at all."""
    nc = bass.Bass()
    KC = P * NT                       # 1024 padded contraction rows
    xT = nc.declare_dram_parameter("xT", [KC, KC], BF16, isOutput=False)
    # packed rhs slices for the 12 (chunk, block) matmuls: (128, 12*330)
    wpk = nc.declare_dram_parameter("wpk", [P, 12 * 330], BF16,
                                    isOutput=False)
    w1blk = nc.declare_dram_parameter("w1blk", [P, P], F32, isOutput=False)
    w2blk = nc.declare_dram_parameter("w2blk", [P, 90], F32, isOutput=False)
    ident = nc.declare_dram_parameter("ident", [P, P], F32, isOutput=False)
    gate_o = nc.declare_dram_parameter("gate", [P, NT * C], BF16,
                                       isOutput=True)

    # (chunk, block) pairs: block j covers channels 15j..15j+14 =
    # contraction rows 315j..315j+314, plus the bias lane in chunk 7
    PAIRS = [[0, 1, 2, 7], [2, 3, 4, 7], [4, 5, 6, 7]]

    with tile.TileContext(nc) as tc, ExitStack() as ctx, \
            nc.allow_low_precision(reason="bf16 gate pipeline; 2e-2 tol"):
        singles = ctx.enter_context(tc.tile_pool(name="singles", bufs=1))
        cvps = ctx.enter_context(tc.tile_pool(name="cvps", bufs=2,
                                              space="PSUM"))
        ppps = ctx.enter_context(tc.tile_pool(name="ppps", bufs=1,
                                              space="PSUM"))
        work = ctx.enter_context(tc.tile_pool(name="work", bufs=2))
        small = ctx.enter_context(tc.tile_pool(name="small", bufs=3))

        def dve_const(dram, p, n, dt=BF16):
            raw = singles.tile([p, n], dt, name="raw_" + dram.name)
            nc.sync.dma_start(out=raw, in_=dram[:, :])
            t = singles.tile([p, n], dt, name="sb_" + dram.name)
            nc.vector.tensor_copy(out=t, in_=raw)
            return t

        sb_w = dve_const(wpk, P, 12 * 330)

        # xT loads: per-tile DMA of the (128, 8x128) lhsT panel so tile 0
        # can start early.  lhs[p, c*128+b] = XT[128c+p, 128t+b]
        xb = xT[:, :]
        lhsT = []

        def load_lhsT(t):
            lt = singles.tile([P, NT * P], BF16, name=f"lhsT{t}")
            ap = bass.AP(tensor=xb.tensor, offset=xb.offset,
                         ap=[[KC, P], [P * KC, NT], [1, P]])
            ap.offset = ap.offset + t * P
            nc.sync.dma_start(out=lt, in_=ap)
            lhsT.append(lt)

        load_lhsT(0)
        load_lhsT(1)
        sb_w1 = dve_const(w1blk, P, P, F32)
        sb_w2 = dve_const(w2blk, P, 90, F32)
        sb_id = dve_const(ident, P, P, F32)
        for t in range(2, NT):
            load_lhsT(t)

        # persistent h tiles: cols 0:90 rewritten each use; 90:128 junk is
        # zeroed once (col 95 = ones lane feeding the layer-1 bias row)
        hb = []
        for i in range(2):
            t = singles.tile([P, P], F32, name=f"hb{i}")
            nc.vector.memset(t[:, 90:P], 0.0)
            nc.vector.memset(t[:, 95:96], 1.0)
            hb.append(t)

        gate_all = singles.tile([P, NT * C], BF16)
        zall = singles.tile([P, NT * C], BF16)

        def conv_block(t):
            lt = lhsT[t]
            cvt = [cvps.tile([P, 330], F32, tag=f"cv{j}", name=f"cv{j}")
                   for j in range(3)]
            for j in range(3):
                for i, c in enumerate(PAIRS[j]):
                    nc.tensor.matmul(
                        cvt[j], lt[:, c * P:(c + 1) * P],
                        sb_w[:, (j * 4 + i) * 330:(j * 4 + i + 1) * 330],
                        start=(i == 0), stop=(i == len(PAIRS[j]) - 1))
            h = hb[t % 2]
            for j in range(3):
                nc.vector.tensor_reduce(
                    out=h[:, 15 * j:15 * j + 15],
                    in_=_ap(cvt[j][:, 0:330], [[22, 15], [1, L]]),
                    axis=AX.X, op=ALU.max)
            # avg lanes (col 21 of each 22-block): 2 on DVE, 1 on Act
            nc.vector.tensor_copy(out=h[:, 45:60],
                                  in_=_off(cvt[0], [[22, 15]], 21))
            nc.scalar.copy(out=h[:, 60:75], in_=_off(cvt[1], [[22, 15]], 21))
            nc.vector.tensor_copy(out=h[:, 75:90],
                                  in_=_off(cvt[2], [[22, 15]], 21))
            return h

        def mlp_block(t, h):
            # both MLP branches through 128x128 f32 blocks; PE transposes
            # with ones lanes via h col 95 and the saturated tanh col 127
            trm = ppps.tile([P, P], F32, tag="trm", name="trm")
            nc.tensor.transpose(trm, h, sb_id)
            hT = work.tile([P, P], F32, tag="hT", name="hT")
            nc.scalar.copy(out=hT, in_=trm)
            pp = ppps.tile([P, P], F32, tag="pp", name="pp")
            nc.tensor.matmul(pp, hT, sb_w1, start=True, stop=True)
            t1 = small.tile([P, P], F32, tag="t1", name="t1")
            nc.scalar.activation(out=t1, in_=pp, func=AF.Tanh)
            trm2 = ppps.tile([P, P], F32, tag="trm", name="trm2")
            nc.tensor.transpose(trm2, t1, sb_id)
            t1T = work.tile([P, P], F32, tag="t1T", name="t1T")
            nc.vector.tensor_copy(out=t1T, in_=trm2)
            p2 = ppps.tile([P, 90], F32, tag="pp", name="p2")
            nc.tensor.matmul(p2, t1T, sb_w2, start=True, stop=True)
            z2 = small.tile([P, 90], BF16, tag="z2", name="z2")
            nc.scalar.activation(out=z2, in_=p2, func=AF.Tanh)
            nc.vector.tensor_add(out=zall[:, t * C:(t + 1) * C],
                                 in0=z2[:, 0:45], in1=z2[:, 45:90])

        def softmax_half(lo, nt):
            # segmented softmax over tile blocks [lo, lo+nt) + output DMA
            zs = zall[:, lo * C:(lo + nt) * C]
            zmax = small.tile([P, nt], F32, tag="zmax", name="zmax")
            nc.vector.tensor_reduce(out=zmax, in_=_ap(zs, [[C, nt], [1, C]]),
                                    axis=AX.X, op=ALU.max)
            zmax16 = small.tile([P, nt], BF16, tag="zmax16", name="zmax16")
            nc.vector.tensor_copy(out=zmax16, in_=zmax)
            zsub = work.tile([P, nt * C], BF16, tag="zsub", name="zsub")
            nc.vector.tensor_sub(out=zsub, in0=zs,
                                 in1=_ap(zmax16, [[1, nt], [0, C]]))
            eg = work.tile([P, nt * C], BF16, tag="eg", name="eg")
            nc.scalar.activation(out=eg, in_=zsub, func=AF.Exp)
            ssum = small.tile([P, nt], F32, tag="ssum", name="ssum")
            nc.vector.tensor_reduce(out=ssum, in_=_ap(eg, [[C, nt], [1, C]]),
                                    axis=AX.X, op=ALU.add)
            rs = small.tile([P, nt], BF16, tag="rs", name="rs")
            nc.vector.reciprocal(out=rs, in_=ssum)
            gs = gate_all[:, lo * C:(lo + nt) * C]
            nc.vector.tensor_mul(out=gs, in0=eg,
                                 in1=_ap(rs, [[1, nt], [0, C]]))
            nc.sync.dma_start(out=gate_o[:, lo * C:(lo + nt) * C], in_=gs)

        # software-pipelined emission: engines issue in program order, so
        # interleave conv(t+1) ahead of mlp(t) to let tiles overlap; the
        # first softmax half runs under the back half of the pipeline
        hprev = None
        for t in range(NT + 1):
            if t < NT:
                hcur = conv_block(t)
            if t >= 1:
                mlp_block(t - 1, hprev)
            if t == NT // 2 + 1:
                softmax_half(0, NT // 2)
            hprev = hcur
        softmax_half(NT // 2, NT // 2)
    legalize_sync_waits(nc)
    return nc


# --------------------------------------------------------------------------
# Launch 2: expert attention via fitted moment polynomials
# --------------------------------------------------------------------------

def build_attn_program(degs):
    """degs: per-expert polynomial degree, sorted ascending (len 22)."""
    degs = list(degs)
    assert degs == sorted(degs)
    dmax = max(degs)
    # suffix start index for "experts with degree > k"
    estart = {k: next((i for i in range(E) if degs[i] > k), E)
              for k in range(dmax)}
    # first expert of each degree class (for acc initialization)
    class_start = {}
    for i, d in enumerate(degs):
        class_start.setdefault(d, i)

    nc = bass.Bass()
    u_d = nc.declare_dram_parameter("u", [BC, EL], BF16, isOutput=False)
    phi_d = nc.declare_dram_parameter("phi", [BC, EL], BF16, isOutput=False)
    gu_d = nc.declare_dram_parameter("gu", [BC, EL], BF16, isOutput=False)
    gpq_d = nc.declare_dram_parameter("gpq", [BC, 2 * E], BF16,
                                      isOutput=False)
    cd_d = nc.declare_dram_parameter("cd", [1, (NLEV - 1) * NT * E], BF16,
                                     isOutput=False)
    cn_d = nc.declare_dram_parameter("cn", [1, (NLEV - 1) * NT * E], BF16,
                                     isOutput=False)
    at_o = nc.declare_dram_parameter("at", [BC, EL], BF16, isOutput=True)
    gt_o = nc.declare_dram_parameter("gt", [BC, EL], BF16, isOutput=True)

    def shard_ap(dram, ncols):
        base = dram[:, :]
        return bass.AP(tensor=base.tensor, offset=base.offset,
                       ap=[[ncols, P], [P * ncols, NT], [1, ncols]])

    with tile.TileContext(nc) as tc, ExitStack() as ctx, \
            nc.allow_low_precision(reason="bf16 attn pipeline; 2e-2 tol"):
        sg = ctx.enter_context(tc.tile_pool(name="sg", bufs=1))

        gu = sg.tile([P, WF], BF16)
        nc.sync.dma_start(out=gu, in_=shard_ap(gu_d, EL))
        u = sg.tile([P, WF], BF16)
        nc.sync.dma_start(out=u, in_=shard_ap(u_d, EL))
        phi = sg.tile([P, WF], BF16)
        nc.sync.dma_start(out=phi, in_=shard_ap(phi_d, EL))
        gpq = sg.tile([P, NT * 2 * E], BF16)
        nc.sync.dma_start(out=gpq, in_=shard_ap(gpq_d, 2 * E))

        def bconst(dram, n, nm):
            base = dram[:, :]
            t = sg.tile([P, n], BF16, name=nm)
            nc.sync.dma_start(
                out=t, in_=bass.AP(tensor=base.tensor, offset=base.offset,
                                   ap=[[0, P], [1, n]]))
            return t

        NC_ = NT * E                      # 176 moment columns per level
        cdB = bconst(cd_d, (NLEV - 1) * NC_, "cdB")
        cnB = bconst(cn_d, (NLEV - 1) * NC_, "cnB")

        Wt = sg.tile([P, NLEV * NC_], BF16)   # moment levels W_0..W_7
        # levels 0..3 are fully written by reduces; only the sliced levels
        # need zeroing (their prefixes are read by the D/N scale muls)
        nc.vector.memset(Wt[:, 4 * NC_:NLEV * NC_], 0.0)

        # slice helpers (l-major: free = (t, l, e); e innermost)
        def full3(tile_, e0=0, ne=E, coloff=0):
            return _off(tile_, [[EL, NT], [E, L], [1, ne]], coloff + e0)

        def wout(lev, e0=0, ne=E):
            return _off(Wt, [[E, NT], [1, ne]], lev * NC_ + e0)

        def wred_in(src, e0=0, ne=E):
            return _off(src, [[EL, NT], [1, ne], [E, L]], e0)

        def coef(ctile, lev, e0=0, ne=E):
            return _off(ctile, [[E, NT], [0, L], [1, ne]], lev * NC_ + e0)

        # w = exp(gu); moments
        w = sg.tile([P, WF], BF16)
        nc.scalar.activation(out=w, in_=gu, func=AF.Exp)
        nc.vector.tensor_reduce(out=wout(0), in_=wred_in(w), axis=AX.X,
                                op=ALU.add)
        va = sg.tile([P, WF], BF16)
        vb = sg.tile([P, WF], BF16)
        cur, nxt = va, vb
        nc.vector.tensor_mul(out=cur, in0=w, in1=u)
        nc.vector.tensor_reduce(out=wout(1), in_=wred_in(cur), axis=AX.X,
                                op=ALU.add)
        for lev in range(2, NLEV):
            # moment level `lev` is needed by experts with degree >= lev-1
            e0 = estart.get(lev - 2, E)
            ne = E - e0
            if ne <= 0:
                break
            nc.vector.tensor_mul(out=full3(nxt, e0, ne),
                                 in0=full3(cur, e0, ne),
                                 in1=full3(u, e0, ne))
            nc.vector.tensor_reduce(out=wout(lev, e0, ne),
                                    in_=wred_in(nxt, e0, ne),
                                    axis=AX.X, op=ALU.add)
            cur, nxt = nxt, cur

        # Horner coefficient tensors: D_k = W_k*cd_k, N_k = W_{k+1}*cn_k
        Dt = sg.tile([P, (NLEV - 1) * NC_], BF16)
        nc.vector.tensor_mul(out=Dt, in0=Wt[:, 0:(NLEV - 1) * NC_], in1=cdB)
        Nt = sg.tile([P, (NLEV - 1) * NC_], BF16)
        nc.vector.tensor_mul(out=Nt, in0=Wt[:, NC_:NLEV * NC_], in1=cnB)

        # nested mixed-degree Horner (experts sorted by degree ascending);
        # a class's accumulator starts life fused into its first step:
        # acc = coef(d)*phi + coef(d-1)
        accd = sg.tile([P, WF], BF16)
        accn = sg.tile([P, WF], BF16)
        for k in range(dmax - 1, -1, -1):
            e0 = estart[k]
            ne = E - e0
            cs = class_start.get(k + 1)
            for acc, Ct in ((accd, Dt), (accn, Nt)):
                if cs is not None:
                    ncs = (min([s for dd, s in class_start.items()
                                if dd > k + 1], default=E)) - cs
                    nc.vector.tensor_mul(out=full3(acc, cs, ncs),
                                         in0=coef(Ct, k + 1, cs, ncs),
                                         in1=full3(phi, cs, ncs))
                    e1 = cs + ncs
                    if E - e1 > 0:
                        nc.vector.tensor_mul(out=full3(acc, e1, E - e1),
                                             in0=full3(acc, e1, E - e1),
                                             in1=full3(phi, e1, E - e1))
                else:
                    nc.vector.tensor_mul(out=full3(acc, e0, ne),
                                         in0=full3(acc, e0, ne),
                                         in1=full3(phi, e0, ne))
                nc.vector.tensor_add(out=full3(acc, e0, ne),
                                     in0=full3(acc, e0, ne),
                                     in1=coef(Ct, k, e0, ne))

        # s = num/den; at = s*gp + gq; gt = at*u
        # (tail ops split 16/6 experts across DVE and Pool so they overlap)
        rden = sg.tile([P, WF], BF16)
        nc.vector.reciprocal(out=rden, in_=accd)
        ESP = 16
        s = accn
        nc.vector.tensor_mul(out=full3(s, 0, ESP), in0=full3(accn, 0, ESP),
                             in1=full3(rden, 0, ESP))
        nc.gpsimd.tensor_mul(out=full3(s, ESP, E - ESP),
                             in0=full3(accn, ESP, E - ESP),
                             in1=full3(rden, ESP, E - ESP))
        at = accd
        nc.vector.tensor_mul(
            out=full3(at, 0, ESP), in0=full3(s, 0, ESP),
            in1=_ap(gpq, [[2 * E, NT], [0, L], [1, ESP]]))
        nc.gpsimd.tensor_mul(
            out=full3(at, ESP, E - ESP), in0=full3(s, ESP, E - ESP),
            in1=_off(gpq, [[2 * E, NT], [0, L], [1, E - ESP]], ESP))
        nc.vector.tensor_add(
            out=full3(at, 0, ESP), in0=full3(at, 0, ESP),
            in1=_off(gpq, [[2 * E, NT], [0, L], [1, ESP]], E))
        nc.gpsimd.tensor_add(
            out=full3(at, ESP, E - ESP), in0=full3(at, ESP, E - ESP),
            in1=_off(gpq, [[2 * E, NT], [0, L], [1, E - ESP]], E + ESP))
        gt = va
        nc.gpsimd.tensor_mul(out=full3(gt, ESP, E - ESP),
                             in0=full3(at, ESP, E - ESP),
                             in1=full3(u, ESP, E - ESP))
        nc.vector.tensor_mul(out=full3(gt, 0, ESP),
                             in0=full3(at, 0, ESP),
                             in1=full3(u, 0, ESP))
        nc.sync.dma_start(out=shard_ap(at_o, EL), in_=at)
        nc.sync.dma_start(out=shard_ap(gt_o, EL), in_=gt)
    legalize_sync_waits(nc)
    return nc


# --------------------------------------------------------------------------
# Host-side preparation
# --------------------------------------------------------------------------

def _gate_params(inputs):
    gc_w = inputs["gc_w"].astype(np.float64)
    gc_b = inputs["gc_b"].astype(np.float64)
    KC = P * NT
    # full conv weight: rows = (chan,l) + pad + bias lane, cols = (chan, 22)
    wfull = np.zeros((KC, 990), np.float32)
    wavvec = gc_w.mean(0)
    for i in range(C):
        wfull[i * L:(i + 1) * L, i * 22:i * 22 + L] = gc_w.T
        wfull[i * L:(i + 1) * L, i * 22 + L] = wavvec
        wfull[KC - 1, i * 22:i * 22 + L] = gc_b
        wfull[KC - 1, i * 22 + L] = gc_b.mean()
    PAIRS = [[0, 1, 2, 7], [2, 3, 4, 7], [4, 5, 6, 7]]
    wpk = np.zeros((P, 12 * 330), np.float32)
    for j in range(3):
        for i, c in enumerate(PAIRS[j]):
            wpk[:, (j * 4 + i) * 330:(j * 4 + i + 1) * 330] = \
                wfull[c * P:(c + 1) * P, j * 330:(j + 1) * 330]
    # 128x128 MLP layer-1 block: rows = transposed h cols (0:45 mx, 45:90
    # av, 95 = ones), out cols 0:50 = both branch hiddens, col 127 driven to
    # +30 via the ones row so tanh saturates to an exact 1.0 "ones" lane for
    # layer 2; all other cells zero.
    w1blk = np.zeros((128, 128), np.float32)
    w1blk[0:45, 0:25] = inputs["w1"].T
    w1blk[45:90, 25:50] = inputs["w1"].T
    w1blk[95, 0:25] = inputs["b1"]
    w1blk[95, 25:50] = inputs["b1"]
    w1blk[95, 127] = 30.0
    w2blk = np.zeros((128, 90), np.float32)
    w2blk[0:25, 0:45] = inputs["w2"].T
    w2blk[25:50, 45:90] = inputs["w2"].T
    w2blk[127, 0:45] = inputs["b2"]
    w2blk[127, 45:90] = inputs["b2"]
    return wpk, w1blk, w2blk


def _fit_exp(tsamp, K, wsamp=None, ntail=0.5):
    t = np.asarray(tsamp, np.float64)
    w = np.ones_like(t) if wsamp is None else np.asarray(wsamp, np.float64)
    tm = max(np.abs(t).max(), 1e-3)
    textra = np.linspace(-tm, tm, 64)
    t = np.concatenate([t, textra])
    w = np.concatenate([w, np.full(64, ntail * w.mean())])
    V = np.vander(t, K + 1, increasing=True) * w[:, None]
    c, *_ = np.linalg.lstsq(V, np.exp(t) * w, rcond=None)
    return c


_CACHE = {}


def kernel(**inputs):
    inputs = {k: np.ascontiguousarray(np.asarray(v)) for k, v in
              inputs.items()}
    x = inputs["x"].astype(np.float32)            # (B, C, L)
    import ml_dtypes
    bf = ml_dtypes.bfloat16

    wpk, w1blk, w2blk = _gate_params(inputs)
    cores = list(range(NCORES))
    KC = P * NT

    if "gate" not in _CACHE:
        _CACHE["gate"] = build_gate_program()
    nc1 = _CACHE["gate"]
    maps1 = []
    for i in cores:
        xt = np.zeros((KC, KC), np.float32)
        xt[0:CL, :] = x.reshape(B, CL)[i * BC:(i + 1) * BC].T
        xt[KC - 1, :] = 1.0
        maps1.append({"xT": xt.astype(bf), "wpk": wpk.astype(bf),
                      "w1blk": w1blk, "w2blk": w2blk,
                      "ident": np.eye(P, dtype=np.float32)})
    r1 = run_bass_kernel_spmd(nc1, maps1, cores).results
    # gate tiles come back as (P, NT*C): row p, block t -> batch p + t*P
    gate = np.zeros((B, C), np.float32)
    for i in cores:
        g = np.asarray(r1[i]["gate"]).astype(np.float32)
        gate[i * BC:(i + 1) * BC] = \
            g.reshape(P, NT, C).transpose(1, 0, 2).reshape(BC, C)
    mean_gate = gate.astype(np.float64).mean(0)
    sel = np.sort(np.argsort(-mean_gate, kind="stable")[:E])

    # expert scalars
    wq, bq = inputs["wq"], inputs["bq"]
    wk, bk = inputs["wk"], inputs["bk"]
    wv, bv = inputs["wv"], inputs["bv"]
    wo, bo = inputs["wo"], inputs["bo"]
    alpha = (wq * wk).sum(1).astype(np.float32)
    gamma = (bq * wk).sum(1).astype(np.float32)
    pv = (wo * wv).sum(1).astype(np.float32)
    qv = ((wo * bv).sum(1) + bo).astype(np.float32)

    usel = x[:, sel, :]                            # (B, E, L)
    # per-expert tau range -> degree ladder
    phimax = np.abs(alpha[None, :, None] * usel).max(axis=(0, 2))
    umax = np.abs(usel).max(axis=(0, 2))
    taumax = phimax * umax
    degs_raw = np.where(taumax <= 0.35, 1,
                        np.where(taumax <= 0.7, 2,
                                 np.where(taumax <= 1.2, 3,
                                          np.where(taumax <= 2.0, 4, 6))))
    perm = np.argsort(degs_raw, kind="stable")     # experts by degree asc
    degs = degs_raw[perm]

    # coefficient fits per (permuted) expert
    rng = np.random.RandomState(12345)
    cd = np.zeros((NLEV - 1, E), np.float32)
    cn = np.zeros((NLEV - 1, E), np.float32)
    sub = usel[::16]                               # (B/16, E, L) samples
    for j, e in enumerate(perm):
        K = int(degs[j])
        ue = sub[:, e, :].astype(np.float64)
        tau = (alpha[e] * ue[:, :, None] * ue[:, None, :]).ravel()
        uw = np.abs(np.broadcast_to(ue[:, None, :], ue.shape[:1] + (L, L))
                    ).ravel()
        ss = rng.choice(tau.size, min(40000, tau.size), replace=False)
        cd[0:K + 1, j] = _fit_exp(tau[ss], K)
        cn[0:K + 1, j] = _fit_exp(tau[ss], K, wsamp=uw[ss] + 0.1)

    # device tensors (l-major, expert-permuted)
    uselp = usel[:, perm, :]
    u_lm = np.ascontiguousarray(uselp.transpose(0, 2, 1).reshape(B, EL))
    phi_lm = np.ascontiguousarray(
        (alpha[perm][None, :, None] * uselp).transpose(0, 2, 1)
        .reshape(B, EL))
    gu_lm = np.ascontiguousarray(
        (gamma[perm][None, :, None] * uselp).transpose(0, 2, 1)
        .reshape(B, EL))
    gsel = gate[:, sel][:, perm]
    gp = gsel * pv[perm][None, :]
    gq = gsel * qv[perm][None, :]
    gpq = np.concatenate([gp, gq], 1).astype(np.float32)   # (B, 44)

    NC_ = NT * E
    cd_full = np.tile(cd[:, None, :], (1, NT, 1)).reshape(1, (NLEV - 1) * NC_)
    cn_full = np.tile(cn[:, None, :], (1, NT, 1)).reshape(1, (NLEV - 1) * NC_)

    key = (tuple(int(d) for d in degs),)
    if _CACHE.get("attn_key") != key:
        _CACHE["attn"] = build_attn_program([int(d) for d in degs])
        _CACHE["attn_key"] = key
    nc2 = _CACHE["attn"]
    maps2 = [{"u": u_lm[i * BC:(i + 1) * BC].astype(bf),
              "phi": phi_lm[i * BC:(i + 1) * BC].astype(bf),
              "gu": gu_lm[i * BC:(i + 1) * BC].astype(bf),
              "gpq": gpq[i * BC:(i + 1) * BC].astype(bf),
              "cd": cd_full.astype(bf), "cn": cn_full.astype(bf)}
             for i in cores]
    r2 = run_bass_kernel_spmd(nc2, maps2, cores).results
    at = np.concatenate([np.asarray(r["at"]).astype(np.float32)
                         for r in r2], 0)          # (B, 462) l-major perm
    gt = np.concatenate([np.asarray(r["gt"]).astype(np.float32)
                         for r in r2], 0)

    inv = np.argsort(perm)
    at_e = at.reshape(B, L, E).transpose(0, 2, 1)[:, inv, :]   # (B,E,L)
    gt_e = gt.reshape(B, L, E).transpose(0, 2, 1)[:, inv, :]
    A_full = np.zeros((B, C, L), np.float32)
    G_full = np.zeros((B, C, L), np.float32)
    A_full[:, sel, :] = at_e
    G_full[:, sel, :] = gt_e
    return G_full.reshape(B, CL), A_full.reshape(B, CL)


# revision 59
# speedup vs baseline: 1.1638x; 1.1012x over previous
"""Trainium2 Bass kernel for grouped-attention MoE routing.

Math (derived from the nn.Module):
  gate  = softmax(mlp(maxpool(conv(x))) + mlp(avgpool(conv(x))))      (B,45)
  sel   = sorted(top22(mean_b gate))                                  (22,)
  Per expert e with u = x[:, sel[e], :]:
    energy[l,m] = (a_e*u_l + g_e) * u_m   (rank-1; scalars a,g from weights)
    attn = softmax_m(energy);  s_l = sum_m u_m attn[l,m]
    y_l  = P_e*s_l + Q_e;      A[:,sel[e],:] = y * gate[:,sel[e]]
  G = x * A (flat);  return (G, A_flat)

Implementation strategy (v2):
  Launch 1 (gate): bf16 PE conv with bias folded via a 127th ones row and
    the avg-pool folded in as extra matmul columns; both MLP branches run
    through one block-diagonal matmul pair; outputs batched into one DMA.
  Launch 2 (attn): the rank-1 softmax is evaluated without the LxL energy
    tensor.  With w = exp(g_e*u) and phi = a_e*u:
      den(phi_l) = sum_m w_m exp(phi_l u_m) ~= sum_k cd_k phi_l^k W_k
      num(phi_l) = sum_m u_m w_m exp(phi_l u_m) ~= sum_k cn_k phi_l^k W_{k+1}
    where W_k = sum_m w_m u_m^k are on-device moments and cd/cn are host-
    fitted per-expert polynomial coefficients (least squares over the
    empirical tau = phi*u range; numerator fit |u|-weighted).  s = num/den.
    Per-expert degree ladder (2..6) by empirical |tau| range; experts are
    permuted so degree classes are contiguous and the nested Horner only
    touches suffix slices for the high degrees.  Everything bf16, l-major
    (l outer, e inner) so per-(b,e) coefficient broadcasts stay packed.
  Routing (45-float mean-gate reduction) is mediated on host between the
  two launches, equivalent to the all-reduce in the sharding hint.
"""

import math
import numpy as np
from contextlib import ExitStack

import bass_rust
import concourse.bass as bass
import concourse.mybir as mybir
import concourse.tile as tile
from concourse.bass_utils import run_bass_kernel_spmd

_MULTIWAIT_OK = ("InstNoOp", "InstAllEngineBarrier",
                 "InstEventSemaphore", "InstUnconditionalBranch")


def legalize_sync_waits(nc):
    """walrus codegen on this stack rejects >1 sync wait on most
    instructions; hoist extra waits onto same-engine NoOps."""
    for func in nc.m.functions:
        for block in func.blocks:
            il = block.instructions
            out = []
            for inst in il:
                tname = type(inst).__name__
                si = getattr(inst, "sync_info", None)
                waits = list(si.on_wait) if si is not None else []
                if tname not in _MULTIWAIT_OK and len(waits) > 1:
                    for k, w in enumerate(waits):
                        nop = mybir.InstNoOp(
                            name=f"{inst.name}-synop{k}", ins=[], outs=[])
                        nop.engine = inst.engine
                        nop.sync_info = bass_rust.SyncInfo(
                            on_wait=[w], on_update=[])
                        out.append(nop)
                    inst.sync_info = bass_rust.SyncInfo(
                        on_wait=[], on_update=list(inst.sync_info.on_update))
                out.append(inst)
            il.clear()
            il.extend(out)


B, C, L, E = 8192, 45, 21, 22
NCORES = 8
BC = B // NCORES          # rows per core
P = 128                   # SBUF partitions
NT = BC // P              # batch tiles per core
CL = C * L                # 945
EL = E * L                # 462
WF = NT * EL              # 3696 full-shard free width (t, l, e) l-major
F32 = mybir.dt.float32
BF16 = mybir.dt.bfloat16
AF = mybir.ActivationFunctionType
ALU = mybir.AluOpType
AX = mybir.AxisListType

NG = 8                    # conv channel groups (6,6,...,3 channels)
GCH = [list(range(g, min(g + 6, C))) for g in range(0, C, 6)]
NLEV = 8                  # moment levels W_0..W_7 held on device
DEG_MAX = 6


def _ap(base, extra_free):
    """Custom free-dim access pattern on an SBUF tile slice (partition
    dim kept from `base`)."""
    return bass.AP(tensor=base.tensor, offset=base.offset,
                   ap=[base.ap[0]] + extra_free)


def _off(base, extra_free, col_off):
    ap = bass.AP(tensor=base.tensor, offset=base.offset,
                 ap=[base.ap[0]] + extra_free)
    ap.offset = ap.offset + col_off
    return ap


# --------------------------------------------------------------------------
# Launch 1: gating network
# --------------------------------------------------------------------------

def build_gate_program():
    """Gate launch. Host uploads x TRANSPOSED and padded: XT (1024, 1024)
    with rows 0:945 = x[coreshard].T (row = (chan,l)), rows 945:1023 = 0,
    row 1023 = 1 (bias lane).  The conv+avg matmul accumulates chunk-wise
    into 3 PSUM column blocks of 15 channels x 22 outputs; no on-device
    transposes or PSUM->SBUF copies are needed for the conv at all."""
    nc = bass.Bass()
    KC = P * NT                       # 1024 padded contraction rows
    xT = nc.declare_dram_parameter("xT", [KC, KC], BF16, isOutput=False)
    # packed rhs slices for the 12 (chunk, block) matmuls: (128, 12*330)
    wpk = nc.declare_dram_parameter("wpk", [P, 12 * 330], BF16,
                                    isOutput=False)
    w1blk = nc.declare_dram_parameter("w1blk", [P, P], F32, isOutput=False)
    w2blk = nc.declare_dram_parameter("w2blk", [P, 90], F32, isOutput=False)
    ident = nc.declare_dram_parameter("ident", [P, P], F32, isOutput=False)
    gate_o = nc.declare_dram_parameter("gate", [P, NT * C], BF16,
                                       isOutput=True)

    # (chunk, block) pairs: block j covers channels 15j..15j+14 =
    # contraction rows 315j..315j+314, plus the bias lane in chunk 7
    PAIRS = [[0, 1, 2, 7], [2, 3, 4, 7], [4, 5, 6, 7]]

    with tile.TileContext(nc) as tc, ExitStack() as ctx, \
            nc.allow_low_precision(reason="bf16 gate pipeline; 2e-2 tol"):
        singles = ctx.enter_context(tc.tile_pool(name="singles", bufs=1))
        cvps = ctx.enter_context(tc.tile_pool(name="cvps", bufs=2,
                                              space="PSUM"))
        ppps = ctx.enter_context(tc.tile_pool(name="ppps", bufs=1,
                                              space="PSUM"))
        work = ctx.enter_context(tc.tile_pool(name="work", bufs=2))
        small = ctx.enter_context(tc.tile_pool(name="small", bufs=3))

        def dve_const(dram, p, n, dt=BF16):
            raw = singles.tile([p, n], dt, name="raw_" + dram.name)
            nc.sync.dma_start(out=raw, in_=dram[:, :])
            t = singles.tile([p, n], dt, name="sb_" + dram.name)
            nc.vector.tensor_copy(out=t, in_=raw)
            return t

        sb_w = dve_const(wpk, P, 12 * 330)

        # xT loads: per-tile DMA of the (128, 8x128) lhsT panel so tile 0
        # can start early.  lhs[p, c*128+b] = XT[128c+p, 128t+b]
        xb = xT[:, :]
        lhsT = []

        def load_lhsT(t):
            lt = singles.tile([P, NT * P], BF16, name=f"lhsT{t}")
            ap = bass.AP(tensor=xb.tensor, offset=xb.offset,
                         ap=[[KC, P], [P * KC, NT], [1, P]])
            ap.offset = ap.offset + t * P
            nc.sync.dma_start(out=lt, in_=ap)
            lhsT.append(lt)

        load_lhsT(0)
        load_lhsT(1)
        sb_w1 = dve_const(w1blk, P, P, F32)
        sb_w2 = dve_const(w2blk, P, 90, F32)
        sb_id = dve_const(ident, P, P, F32)
        for t in range(2, NT):
            load_lhsT(t)

        # persistent h tiles: cols 0:90 rewritten each use; 90:128 junk is
        # zeroed once (col 95 = ones lane feeding the layer-1 bias row)
        hb = []
        for i in range(2):
            t = singles.tile([P, P], F32, name=f"hb{i}")
            nc.vector.memset(t[:, 90:P], 0.0)
            nc.vector.memset(t[:, 95:96], 1.0)
            hb.append(t)

        gate_all = singles.tile([P, NT * C], BF16)
        zall = singles.tile([P, NT * C], BF16)

        def conv_block(t):
            lt = lhsT[t]
            cvt = [cvps.tile([P, 330], F32, tag=f"cv{j}", name=f"cv{j}")
                   for j in range(3)]
            for j in range(3):
                for i, c in enumerate(PAIRS[j]):
                    nc.tensor.matmul(
                        cvt[j], lt[:, c * P:(c + 1) * P],
                        sb_w[:, (j * 4 + i) * 330:(j * 4 + i + 1) * 330],
                        start=(i == 0), stop=(i == len(PAIRS[j]) - 1))
            h = hb[t % 2]
            for j in range(3):
                nc.vector.tensor_reduce(
                    out=h[:, 15 * j:15 * j + 15],
                    in_=_ap(cvt[j][:, 0:330], [[22, 15], [1, L]]),
                    axis=AX.X, op=ALU.max)
            # avg lanes (col 21 of each 22-block): 2 on DVE, 1 on Act
            nc.vector.tensor_copy(out=h[:, 45:60],
                                  in_=_off(cvt[0], [[22, 15]], 21))
            nc.scalar.copy(out=h[:, 60:75], in_=_off(cvt[1], [[22, 15]], 21))
            nc.vector.tensor_copy(out=h[:, 75:90],
                                  in_=_off(cvt[2], [[22, 15]], 21))
            return h

        def mlp_block(t, h):
            # both MLP branches through 128x128 f32 blocks; PE transposes
            # with ones lanes via h col 95 and the saturated tanh col 127
            trm = ppps.tile([P, P], F32, tag="trm", name="trm")
            nc.tensor.transpose(trm, h, sb_id)
            hT = work.tile([P, P], F32, tag="hT", name="hT")
            nc.scalar.copy(out=hT, in_=trm)
            pp = ppps.tile([P, P], F32, tag="pp", name="pp")
            nc.tensor.matmul(pp, hT, sb_w1, start=True, stop=True)
            t1 = small.tile([P, P], F32, tag="t1", name="t1")
            nc.scalar.activation(out=t1, in_=pp, func=AF.Tanh)
            trm2 = ppps.tile([P, P], F32, tag="trm", name="trm2")
            nc.tensor.transpose(trm2, t1, sb_id)
            t1T = work.tile([P, P], F32, tag="t1T", name="t1T")
            nc.vector.tensor_copy(out=t1T, in_=trm2)
            p2 = ppps.tile([P, 90], F32, tag="pp", name="p2")
            nc.tensor.matmul(p2, t1T, sb_w2, start=True, stop=True)
            z2 = small.tile([P, 90], BF16, tag="z2", name="z2")
            nc.scalar.activation(out=z2, in_=p2, func=AF.Tanh)
            nc.vector.tensor_add(out=zall[:, t * C:(t + 1) * C],
                                 in0=z2[:, 0:45], in1=z2[:, 45:90])

        def softmax_half(lo, nt):
            # segmented softmax over tile blocks [lo, lo+nt) + output DMA
            zs = zall[:, lo * C:(lo + nt) * C]
            zmax = small.tile([P, nt], F32, tag="zmax", name="zmax")
            nc.vector.tensor_reduce(out=zmax, in_=_ap(zs, [[C, nt], [1, C]]),
                                    axis=AX.X, op=ALU.max)
            zmax16 = small.tile([P, nt], BF16, tag="zmax16", name="zmax16")
            nc.vector.tensor_copy(out=zmax16, in_=zmax)
            zsub = work.tile([P, nt * C], BF16, tag="zsub", name="zsub")
            nc.vector.tensor_sub(out=zsub, in0=zs,
                                 in1=_ap(zmax16, [[1, nt], [0, C]]))
            eg = work.tile([P, nt * C], BF16, tag="eg", name="eg")
            nc.scalar.activation(out=eg, in_=zsub, func=AF.Exp)
            ssum = small.tile([P, nt], F32, tag="ssum", name="ssum")
            nc.vector.tensor_reduce(out=ssum, in_=_ap(eg, [[C, nt], [1, C]]),
                                    axis=AX.X, op=ALU.add)
            rs = small.tile([P, nt], BF16, tag="rs", name="rs")
            nc.vector.reciprocal(out=rs, in_=ssum)
            gs = gate_all[:, lo * C:(lo + nt) * C]
            nc.vector.tensor_mul(out=gs, in0=eg,
                                 in1=_ap(rs, [[1, nt], [0, C]]))
            nc.sync.dma_start(out=gate_o[:, lo * C:(lo + nt) * C], in_=gs)

        # software-pipelined emission: engines issue in program order, so
        # interleave conv(t+1) ahead of mlp(t) to let tiles overlap; the
        # first softmax half runs under the back half of the pipeline
        hprev = None
        for t in range(NT + 1):
            if t < NT:
                hcur = conv_block(t)
            if t >= 1:
                mlp_block(t - 1, hprev)
            if t == NT // 2 + 1:
                softmax_half(0, NT // 2)
            hprev = hcur
        softmax_half(NT // 2, NT // 2)
    legalize_sync_waits(nc)
    return nc


# --------------------------------------------------------------------------
# Launch 2: expert attention via fitted moment polynomials
# --------------------------------------------------------------------------

def build_attn_program(degs):
    """degs: per-expert polynomial degree, sorted ascending (len 22)."""
    degs = list(degs)
    assert degs == sorted(degs)
    dmax = max(degs)
    # suffix start index for "experts with degree > k"
    estart = {k: next((i for i in range(E) if degs[i] > k), E)
              for k in range(dmax)}
    # first expert of each degree class (for acc initialization)
    class_start = {}
    for i, d in enumerate(degs):
        class_start.setdefault(d, i)

    nc = bass.Bass()
    u_d = nc.declare_dram_parameter("u", [BC, EL], BF16, isOutput=False)
    phi_d = nc.declare_dram_parameter("phi", [BC, EL], BF16, isOutput=False)
    # e-major transposed u and g*u: 4 chunks of 128 DRAM rows; chunk c rows
    # 0:126 = transposed rows [126c,126c+126) (row = 21*e_perm + l), rest 0
    uT_d = nc.declare_dram_parameter("uT", [4 * P, BC], BF16, isOutput=False)
    guT_d = nc.declare_dram_parameter("guT", [4 * P, BC], BF16,
                                      isOutput=False)
    ind_d = nc.declare_dram_parameter("ind", [P, 4 * E], BF16, isOutput=False)
    gpq_d = nc.declare_dram_parameter("gpq", [BC, 2 * E], BF16,
                                      isOutput=False)
    cd_d = nc.declare_dram_parameter("cd", [1, (NLEV - 1) * NT * E], BF16,
                                     isOutput=False)
    cn_d = nc.declare_dram_parameter("cn", [1, (NLEV - 1) * NT * E], BF16,
                                     isOutput=False)
    at_o = nc.declare_dram_parameter("at", [BC, EL], BF16, isOutput=True)
    gt_o = nc.declare_dram_parameter("gt", [BC, EL], BF16, isOutput=True)

    def shard_ap(dram, ncols):
        base = dram[:, :]
        return bass.AP(tensor=base.tensor, offset=base.offset,
                       ap=[[ncols, P], [P * ncols, NT], [1, ncols]])

    with tile.TileContext(nc) as tc, ExitStack() as ctx, \
            nc.allow_low_precision(reason="bf16 attn pipeline; 2e-2 tol"):
        sg = ctx.enter_context(tc.tile_pool(name="sg", bufs=1))
        wpool = ctx.enter_context(tc.tile_pool(name="wpool", bufs=1,
                                               space="PSUM"))

        guT = sg.tile([P, 4 * BC], BF16)
        gb = guT_d[:, :]
        nc.sync.dma_start(
            out=guT, in_=bass.AP(tensor=gb.tensor, offset=gb.offset,
                                 ap=[[BC, P], [P * BC, 4], [1, BC]]))
        uT = sg.tile([P, 4 * BC], BF16)
        ub = uT_d[:, :]
        nc.sync.dma_start(
            out=uT, in_=bass.AP(tensor=ub.tensor, offset=ub.offset,
                                ap=[[BC, P], [P * BC, 4], [1, BC]]))
        u = sg.tile([P, WF], BF16)
        nc.sync.dma_start(out=u, in_=shard_ap(u_d, EL))
        phi = sg.tile([P, WF], BF16)
        nc.sync.dma_start(out=phi, in_=shard_ap(phi_d, EL))
        indr = sg.tile([P, 4 * E], BF16)
        nc.sync.dma_start(out=indr, in_=ind_d[:, :])
        ind = sg.tile([P, 4 * E], BF16)
        nc.vector.tensor_copy(out=ind, in_=indr)
        gpq = sg.tile([P, NT * 2 * E], BF16)
        nc.sync.dma_start(out=gpq, in_=shard_ap(gpq_d, 2 * E))

        def bconst(dram, n, nm):
            base = dram[:, :]
            t = sg.tile([P, n], BF16, name=nm)
            nc.sync.dma_start(
                out=t, in_=bass.AP(tensor=base.tensor, offset=base.offset,
                                   ap=[[0, P], [1, n]]))
            return t

        NC_ = NT * E
        cdB = bconst(cd_d, (NLEV - 1) * NC_, "cdB")
        cnB = bconst(cn_d, (NLEV - 1) * NC_, "cnB")

        # Wt layout: col = t*176 + lev*22 + e
        Wt = sg.tile([P, NT * NLEV * E], BF16)

        # slice helpers (l-major: free = (t, l, e); e innermost)
        def full3(tile_, e0=0, ne=E, coloff=0):
            return _off(tile_, [[EL, NT], [E, L], [1, ne]], coloff + e0)

        def coef(ctile, lev, e0=0, ne=E):
            return _off(ctile, [[(NLEV - 1) * E, NT], [0, L], [1, ne]],
                        lev * E + e0)

        # transposed chain: wT = exp(guT); vT_k = vT_{k-1}*uT; chunk c of the
        # free dim holds transposed rows [126c,126c+126) for batch cols
        wT = sg.tile([P, 4 * BC], BF16)
        nc.scalar.activation(out=wT, in_=guT, func=AF.Exp)
        vTa = sg.tile([P, 4 * BC], BF16)
        vTb = sg.tile([P, 4 * BC], BF16)

        # chunk start per level: experts with deg >= lev-1 live in chunks
        # >= estart[lev-2]//6 (extra experts in a straddling chunk are
        # harmless: their cd/cn consts are zero)
        def cstart(lev):
            if lev <= 2:
                return 0
            e0 = estart.get(lev - 2, E)
            return 4 if e0 >= E else e0 // 6

        wps = [wpool.tile([P, NLEV * E], F32, tag=f"wp{t}", name=f"wp{t}")
               for t in range(NT)]

        def moments(lev, vt):
            c0 = cstart(lev)
            for t in range(NT):
                for c in range(c0, 4):
                    nc.tensor.matmul(
                        wps[t][:, lev * E:(lev + 1) * E],
                        vt[0:126, c * BC + t * P:c * BC + (t + 1) * P],
                        ind[0:126, c * E:(c + 1) * E],
                        start=(c == c0), stop=(c == 3))

        moments(0, wT)
        cur, nxt = vTa, vTb
        nc.vector.tensor_mul(out=cur, in0=wT, in1=uT)
        moments(1, cur)
        for lev in range(2, NLEV):
            c0 = cstart(lev)
            if c0 >= 4:
                break
            off = c0 * BC
            nc.vector.tensor_mul(out=nxt[:, off:4 * BC],
                                 in0=cur[:, off:4 * BC],
                                 in1=uT[:, off:4 * BC])
            moments(lev, nxt)
            cur, nxt = nxt, cur

        # PSUM -> Wt copies, one per tile block (DVE/Act split)
        for t in range(NT):
            dst = Wt[:, t * NLEV * E:(t + 1) * NLEV * E]
            if t % 2 == 0:
                nc.vector.tensor_copy(out=dst, in_=wps[t])
            else:
                nc.scalar.copy(out=dst, in_=wps[t])

        # Horner coefficient tensors: D_k = W_k*cd_k, N_k = W_{k+1}*cn_k
        NL1 = (NLEV - 1) * E
        Dt = sg.tile([P, NT * NL1], BF16)
        nc.vector.tensor_mul(out=_ap(Dt, [[NL1, NT], [1, NL1]]),
                             in0=_ap(Wt, [[NLEV * E, NT], [1, NL1]]),
                             in1=cdB)
        Nt = sg.tile([P, NT * NL1], BF16)
        nc.vector.tensor_mul(out=_ap(Nt, [[NL1, NT], [1, NL1]]),
                             in0=_off(Wt, [[NLEV * E, NT], [1, NL1]], E),
                             in1=cnB)

        # nested mixed-degree Horner (experts sorted by degree ascending);
        # a class's accumulator starts life fused into its first step:
        # acc = coef(d)*phi + coef(d-1)
        accd = sg.tile([P, WF], BF16)
        accn = sg.tile([P, WF], BF16)
        for k in range(dmax - 1, -1, -1):
            e0 = estart[k]
            ne = E - e0
            cs = class_start.get(k + 1)
            for acc, Ct in ((accd, Dt), (accn, Nt)):
                if cs is not None:
                    ncs = (min([s for dd, s in class_start.items()
                                if dd > k + 1], default=E)) - cs
                    nc.vector.tensor_mul(out=full3(acc, cs, ncs),
                                         in0=coef(Ct, k + 1, cs, ncs),
                                         in1=full3(phi, cs, ncs))
                    e1 = cs + ncs
                    if E - e1 > 0:
                        nc.vector.tensor_mul(out=full3(acc, e1, E - e1),
                                             in0=full3(acc, e1, E - e1),
                                             in1=full3(phi, e1, E - e1))
                else:
                    nc.vector.tensor_mul(out=full3(acc, e0, ne),
                                         in0=full3(acc, e0, ne),
                                         in1=full3(phi, e0, ne))
                nc.vector.tensor_add(out=full3(acc, e0, ne),
                                     in0=full3(acc, e0, ne),
                                     in1=coef(Ct, k, e0, ne))

        # s = num/den; at = s*gp + gq; gt = at*u
        # (tail ops split 16/6 experts across DVE and Pool so they overlap)
        rden = sg.tile([P, WF], BF16)
        nc.vector.reciprocal(out=rden, in_=accd)
        ESP = 16
        s = accn
        nc.vector.tensor_mul(out=full3(s, 0, ESP), in0=full3(accn, 0, ESP),
                             in1=full3(rden, 0, ESP))
        nc.gpsimd.tensor_mul(out=full3(s, ESP, E - ESP),
                             in0=full3(accn, ESP, E - ESP),
                             in1=full3(rden, ESP, E - ESP))
        at = accd
        nc.vector.tensor_mul(
            out=full3(at, 0, ESP), in0=full3(s, 0, ESP),
            in1=_ap(gpq, [[2 * E, NT], [0, L], [1, ESP]]))
        nc.gpsimd.tensor_mul(
            out=full3(at, ESP, E - ESP), in0=full3(s, ESP, E - ESP),
            in1=_off(gpq, [[2 * E, NT], [0, L], [1, E - ESP]], ESP))
        nc.vector.tensor_add(
            out=full3(at, 0, ESP), in0=full3(at, 0, ESP),
            in1=_off(gpq, [[2 * E, NT], [0, L], [1, ESP]], E))
        nc.gpsimd.tensor_add(
            out=full3(at, ESP, E - ESP), in0=full3(at, ESP, E - ESP),
            in1=_off(gpq, [[2 * E, NT], [0, L], [1, E - ESP]], E + ESP))
        gt = rden
        nc.gpsimd.tensor_mul(out=full3(gt, ESP, E - ESP),
                             in0=full3(at, ESP, E - ESP),
                             in1=full3(u, ESP, E - ESP))
        nc.vector.tensor_mul(out=full3(gt, 0, ESP),
                             in0=full3(at, 0, ESP),
                             in1=full3(u, 0, ESP))
        nc.sync.dma_start(out=shard_ap(at_o, EL), in_=at)
        nc.sync.dma_start(out=shard_ap(gt_o, EL), in_=gt)
    legalize_sync_waits(nc)
    return nc


# --------------------------------------------------------------------------
# Host-side preparation
# --------------------------------------------------------------------------

def _gate_params(inputs):
    gc_w = inputs["gc_w"].astype(np.float64)
    gc_b = inputs["gc_b"].astype(np.float64)
    KC = P * NT
    # full conv weight: rows = (chan,l) + pad + bias lane, cols = (chan, 22)
    wfull = np.zeros((KC, 990), np.float32)
    wavvec = gc_w.mean(0)
    for i in range(C):
        wfull[i * L:(i + 1) * L, i * 22:i * 22 + L] = gc_w.T
        wfull[i * L:(i + 1) * L, i * 22 + L] = wavvec
        wfull[KC - 1, i * 22:i * 22 + L] = gc_b
        wfull[KC - 1, i * 22 + L] = gc_b.mean()
    PAIRS = [[0, 1, 2, 7], [2, 3, 4, 7], [4, 5, 6, 7]]
    wpk = np.zeros((P, 12 * 330), np.float32)
    for j in range(3):
        for i, c in enumerate(PAIRS[j]):
            wpk[:, (j * 4 + i) * 330:(j * 4 + i + 1) * 330] = \
                wfull[c * P:(c + 1) * P, j * 330:(j + 1) * 330]
    # 128x128 MLP layer-1 block: rows = transposed h cols (0:45 mx, 45:90
    # av, 95 = ones), out cols 0:50 = both branch hiddens, col 127 driven to
    # +30 via the ones row so tanh saturates to an exact 1.0 "ones" lane for
    # layer 2; all other cells zero.
    w1blk = np.zeros((128, 128), np.float32)
    w1blk[0:45, 0:25] = inputs["w1"].T
    w1blk[45:90, 25:50] = inputs["w1"].T
    w1blk[95, 0:25] = inputs["b1"]
    w1blk[95, 25:50] = inputs["b1"]
    w1blk[95, 127] = 30.0
    w2blk = np.zeros((128, 90), np.float32)
    w2blk[0:25, 0:45] = inputs["w2"].T
    w2blk[25:50, 45:90] = inputs["w2"].T
    w2blk[127, 0:45] = inputs["b2"]
    w2blk[127, 45:90] = inputs["b2"]
    return wpk, w1blk, w2blk


def _fit_exp(tsamp, K, wsamp=None, ntail=0.5):
    t = np.asarray(tsamp, np.float64)
    w = np.ones_like(t) if wsamp is None else np.asarray(wsamp, np.float64)
    tm = max(np.abs(t).max(), 1e-3)
    textra = np.linspace(-tm, tm, 64)
    t = np.concatenate([t, textra])
    w = np.concatenate([w, np.full(64, ntail * w.mean())])
    V = np.vander(t, K + 1, increasing=True) * w[:, None]
    c, *_ = np.linalg.lstsq(V, np.exp(t) * w, rcond=None)
    return c


_CACHE = {}


def kernel(**inputs):
    inputs = {k: np.ascontiguousarray(np.asarray(v)) for k, v in
              inputs.items()}
    x = inputs["x"].astype(np.float32)            # (B, C, L)
    import ml_dtypes
    bf = ml_dtypes.bfloat16

    wpk, w1blk, w2blk = _gate_params(inputs)
    cores = list(range(NCORES))
    KC = P * NT

    if "gate" not in _CACHE:
        _CACHE["gate"] = build_gate_program()
    nc1 = _CACHE["gate"]
    maps1 = []
    for i in cores:
        xt = np.zeros((KC, KC), np.float32)
        xt[0:CL, :] = x.reshape(B, CL)[i * BC:(i + 1) * BC].T
        xt[KC - 1, :] = 1.0
        maps1.append({"xT": xt.astype(bf), "wpk": wpk.astype(bf),
                      "w1blk": w1blk, "w2blk": w2blk,
                      "ident": np.eye(P, dtype=np.float32)})
    r1 = run_bass_kernel_spmd(nc1, maps1, cores).results
    # gate tiles come back as (P, NT*C): row p, block t -> batch p + t*P
    gate = np.zeros((B, C), np.float32)
    for i in cores:
        g = np.asarray(r1[i]["gate"]).astype(np.float32)
        gate[i * BC:(i + 1) * BC] = \
            g.reshape(P, NT, C).transpose(1, 0, 2).reshape(BC, C)
    mean_gate = gate.astype(np.float64).mean(0)
    sel = np.sort(np.argsort(-mean_gate, kind="stable")[:E])

    # expert scalars
    wq, bq = inputs["wq"], inputs["bq"]
    wk, bk = inputs["wk"], inputs["bk"]
    wv, bv = inputs["wv"], inputs["bv"]
    wo, bo = inputs["wo"], inputs["bo"]
    alpha = (wq * wk).sum(1).astype(np.float32)
    gamma = (bq * wk).sum(1).astype(np.float32)
    pv = (wo * wv).sum(1).astype(np.float32)
    qv = ((wo * bv).sum(1) + bo).astype(np.float32)

    usel = x[:, sel, :]                            # (B, E, L)
    # per-expert tau range -> degree ladder
    phimax = np.abs(alpha[None, :, None] * usel).max(axis=(0, 2))
    umax = np.abs(usel).max(axis=(0, 2))
    taumax = phimax * umax
    degs_raw = np.where(taumax <= 0.35, 1,
                        np.where(taumax <= 0.7, 2,
                                 np.where(taumax <= 1.2, 3,
                                          np.where(taumax <= 2.0, 4, 6))))
    perm = np.argsort(degs_raw, kind="stable")     # experts by degree asc
    degs = degs_raw[perm]

    # coefficient fits per (permuted) expert
    rng = np.random.RandomState(12345)
    cd = np.zeros((NLEV - 1, E), np.float32)
    cn = np.zeros((NLEV - 1, E), np.float32)
    sub = usel[::16]                               # (B/16, E, L) samples
    for j, e in enumerate(perm):
        K = int(degs[j])
        ue = sub[:, e, :].astype(np.float64)
        tau = (alpha[e] * ue[:, :, None] * ue[:, None, :]).ravel()
        uw = np.abs(np.broadcast_to(ue[:, None, :], ue.shape[:1] + (L, L))
                    ).ravel()
        ss = rng.choice(tau.size, min(40000, tau.size), replace=False)
        cd[0:K + 1, j] = _fit_exp(tau[ss], K)
        cn[0:K + 1, j] = _fit_exp(tau[ss], K, wsamp=uw[ss] + 0.1)

    # device tensors (l-major, expert-permuted)
    uselp = usel[:, perm, :]
    u_lm = np.ascontiguousarray(uselp.transpose(0, 2, 1).reshape(B, EL))
    phi_lm = np.ascontiguousarray(
        (alpha[perm][None, :, None] * uselp).transpose(0, 2, 1)
        .reshape(B, EL))
    gusel = gamma[perm][None, :, None] * uselp                  # (B,E,L)

    def emaj_chunks(arr):
        # (B,E,L) -> e-major transposed (462,B) in 4 x 128-row DRAM chunks
        tr = arr.transpose(1, 2, 0).reshape(EL, B)
        out = np.zeros((4 * P, B), np.float32)
        for c in range(4):
            r0, r1 = 126 * c, min(126 * (c + 1), EL)
            out[128 * c:128 * c + (r1 - r0)] = tr[r0:r1]
        return out

    uT_pad = emaj_chunks(uselp)
    guT_pad = emaj_chunks(gusel)
    ind = np.zeros((P, 4 * E), np.float32)
    for c in range(4):
        for p_ in range(126):
            r = 126 * c + p_
            if r < EL:
                ind[p_, c * E + r // L] = 1.0
    gsel = gate[:, sel][:, perm]
    gp = gsel * pv[perm][None, :]
    gq = gsel * qv[perm][None, :]
    gpq = np.concatenate([gp, gq], 1).astype(np.float32)   # (B, 44)

    NC_ = NT * E
    # coefficient layout (t, lev, e) to match the per-tile moment blocks
    cd_full = np.tile(cd[None, :, :], (NT, 1, 1)).reshape(1, (NLEV - 1) * NC_)
    cn_full = np.tile(cn[None, :, :], (NT, 1, 1)).reshape(1, (NLEV - 1) * NC_)

    key = (tuple(int(d) for d in degs),)
    if _CACHE.get("attn_key") != key:
        _CACHE["attn"] = build_attn_program([int(d) for d in degs])
        _CACHE["attn_key"] = key
    nc2 = _CACHE["attn"]
    maps2 = [{"u": u_lm[i * BC:(i + 1) * BC].astype(bf),
              "phi": phi_lm[i * BC:(i + 1) * BC].astype(bf),
              "uT": uT_pad[:, i * BC:(i + 1) * BC].astype(bf),
              "guT": guT_pad[:, i * BC:(i + 1) * BC].astype(bf),
              "ind": ind.astype(bf),
              "gpq": gpq[i * BC:(i + 1) * BC].astype(bf),
              "cd": cd_full.astype(bf), "cn": cn_full.astype(bf)}
             for i in cores]
    r2 = run_bass_kernel_spmd(nc2, maps2, cores).results
    at = np.concatenate([np.asarray(r["at"]).astype(np.float32)
                         for r in r2], 0)          # (B, 462) l-major perm
    gt = np.concatenate([np.asarray(r["gt"]).astype(np.float32)
                         for r in r2], 0)

    inv = np.argsort(perm)
    at_e = at.reshape(B, L, E).transpose(0, 2, 1)[:, inv, :]   # (B,E,L)
    gt_e = gt.reshape(B, L, E).transpose(0, 2, 1)[:, inv, :]
    A_full = np.zeros((B, C, L), np.float32)
    G_full = np.zeros((B, C, L), np.float32)
    A_full[:, sel, :] = at_e
    G_full[:, sel, :] = gt_e
    return G_full.reshape(B, CL), A_full.reshape(B, CL)


# revision 60
# speedup vs baseline: 1.1876x; 1.0205x over previous
"""Trainium2 Bass kernel for grouped-attention MoE routing.

Math (derived from the nn.Module):
  gate  = softmax(mlp(maxpool(conv(x))) + mlp(avgpool(conv(x))))      (B,45)
  sel   = sorted(top22(mean_b gate))                                  (22,)
  Per expert e with u = x[:, sel[e], :]:
    energy[l,m] = (a_e*u_l + g_e) * u_m   (rank-1; scalars a,g from weights)
    attn = softmax_m(energy);  s_l = sum_m u_m attn[l,m]
    y_l  = P_e*s_l + Q_e;      A[:,sel[e],:] = y * gate[:,sel[e]]
  G = x * A (flat);  return (G, A_flat)

Implementation strategy (v2):
  Launch 1 (gate): bf16 PE conv with bias folded via a 127th ones row and
    the avg-pool folded in as extra matmul columns; both MLP branches run
    through one block-diagonal matmul pair; outputs batched into one DMA.
  Launch 2 (attn): the rank-1 softmax is evaluated without the LxL energy
    tensor.  With w = exp(g_e*u) and phi = a_e*u:
      den(phi_l) = sum_m w_m exp(phi_l u_m) ~= sum_k cd_k phi_l^k W_k
      num(phi_l) = sum_m u_m w_m exp(phi_l u_m) ~= sum_k cn_k phi_l^k W_{k+1}
    where W_k = sum_m w_m u_m^k are on-device moments and cd/cn are host-
    fitted per-expert polynomial coefficients (least squares over the
    empirical tau = phi*u range; numerator fit |u|-weighted).  s = num/den.
    Per-expert degree ladder (2..6) by empirical |tau| range; experts are
    permuted so degree classes are contiguous and the nested Horner only
    touches suffix slices for the high degrees.  Everything bf16, l-major
    (l outer, e inner) so per-(b,e) coefficient broadcasts stay packed.
  Routing (45-float mean-gate reduction) is mediated on host between the
  two launches, equivalent to the all-reduce in the sharding hint.
"""

import math
import numpy as np
from contextlib import ExitStack

import bass_rust
import concourse.bass as bass
import concourse.mybir as mybir
import concourse.tile as tile
from concourse.bass_utils import run_bass_kernel_spmd

_MULTIWAIT_OK = ("InstNoOp", "InstAllEngineBarrier",
                 "InstEventSemaphore", "InstUnconditionalBranch")


def legalize_sync_waits(nc):
    """walrus codegen on this stack rejects >1 sync wait on most
    instructions; hoist extra waits onto same-engine NoOps."""
    for func in nc.m.functions:
        for block in func.blocks:
            il = block.instructions
            out = []
            for inst in il:
                tname = type(inst).__name__
                si = getattr(inst, "sync_info", None)
                waits = list(si.on_wait) if si is not None else []
                if tname not in _MULTIWAIT_OK and len(waits) > 1:
                    for k, w in enumerate(waits):
                        nop = mybir.InstNoOp(
                            name=f"{inst.name}-synop{k}", ins=[], outs=[])
                        nop.engine = inst.engine
                        nop.sync_info = bass_rust.SyncInfo(
                            on_wait=[w], on_update=[])
                        out.append(nop)
                    inst.sync_info = bass_rust.SyncInfo(
                        on_wait=[], on_update=list(inst.sync_info.on_update))
                out.append(inst)
            il.clear()
            il.extend(out)


B, C, L, E = 8192, 45, 21, 22
NCORES = 8
BC = B // NCORES          # rows per core
P = 128                   # SBUF partitions
NT = BC // P              # batch tiles per core
CL = C * L                # 945
EL = E * L                # 462
WF = NT * EL              # 3696 full-shard free width (t, l, e) l-major
F32 = mybir.dt.float32
BF16 = mybir.dt.bfloat16
AF = mybir.ActivationFunctionType
ALU = mybir.AluOpType
AX = mybir.AxisListType

NG = 8                    # conv channel groups (6,6,...,3 channels)
GCH = [list(range(g, min(g + 6, C))) for g in range(0, C, 6)]
NLEV = 8                  # moment levels W_0..W_7 held on device
DEG_MAX = 6


def _ap(base, extra_free):
    """Custom free-dim access pattern on an SBUF tile slice (partition
    dim kept from `base`)."""
    return bass.AP(tensor=base.tensor, offset=base.offset,
                   ap=[base.ap[0]] + extra_free)


def _off(base, extra_free, col_off):
    ap = bass.AP(tensor=base.tensor, offset=base.offset,
                 ap=[base.ap[0]] + extra_free)
    ap.offset = ap.offset + col_off
    return ap


# --------------------------------------------------------------------------
# Launch 1: gating network
# --------------------------------------------------------------------------

def build_gate_program():
    """Gate launch. Host uploads x TRANSPOSED and padded: XT (1024, 1024)
    with rows 0:945 = x[coreshard].T (row = (chan,l)), rows 945:1023 = 0,
    row 1023 = 1 (bias lane).  The conv+avg matmul accumulates chunk-wise
    into 3 PSUM column blocks of 15 channels x 22 outputs; no on-device
    transposes or PSUM->SBUF copies are needed for the conv at all."""
    nc = bass.Bass()
    KC = P * NT                       # 1024 padded contraction rows
    xT = nc.declare_dram_parameter("xT", [KC, KC], BF16, isOutput=False)
    # packed rhs slices for the 12 (chunk, block) matmuls: (128, 12*330)
    wpk = nc.declare_dram_parameter("wpk", [P, 12 * 330], BF16,
                                    isOutput=False)
    w1blk = nc.declare_dram_parameter("w1blk", [P, P], F32, isOutput=False)
    w2blk = nc.declare_dram_parameter("w2blk", [P, 90], F32, isOutput=False)
    ident = nc.declare_dram_parameter("ident", [P, P], F32, isOutput=False)
    gate_o = nc.declare_dram_parameter("gate", [P, NT * C], BF16,
                                       isOutput=True)

    # (chunk, block) pairs: block j covers channels 15j..15j+14 =
    # contraction rows 315j..315j+314, plus the bias lane in chunk 7
    PAIRS = [[0, 1, 2, 7], [2, 3, 4, 7], [4, 5, 6, 7]]

    with tile.TileContext(nc) as tc, ExitStack() as ctx, \
            nc.allow_low_precision(reason="bf16 gate pipeline; 2e-2 tol"):
        singles = ctx.enter_context(tc.tile_pool(name="singles", bufs=1))
        cvps = ctx.enter_context(tc.tile_pool(name="cvps", bufs=2,
                                              space="PSUM"))
        ppps = ctx.enter_context(tc.tile_pool(name="ppps", bufs=1,
                                              space="PSUM"))
        work = ctx.enter_context(tc.tile_pool(name="work", bufs=2))
        small = ctx.enter_context(tc.tile_pool(name="small", bufs=3))

        def dve_const(dram, p, n, dt=BF16):
            raw = singles.tile([p, n], dt, name="raw_" + dram.name)
            nc.sync.dma_start(out=raw, in_=dram[:, :])
            t = singles.tile([p, n], dt, name="sb_" + dram.name)
            nc.vector.tensor_copy(out=t, in_=raw)
            return t

        sb_w = dve_const(wpk, P, 12 * 330)

        # xT loads: per-tile DMA of the (128, 8x128) lhsT panel so tile 0
        # can start early.  lhs[p, c*128+b] = XT[128c+p, 128t+b]
        xb = xT[:, :]
        lhsT = []

        def load_lhsT(t):
            lt = singles.tile([P, NT * P], BF16, name=f"lhsT{t}")
            ap = bass.AP(tensor=xb.tensor, offset=xb.offset,
                         ap=[[KC, P], [P * KC, NT], [1, P]])
            ap.offset = ap.offset + t * P
            nc.sync.dma_start(out=lt, in_=ap)
            lhsT.append(lt)

        load_lhsT(0)
        load_lhsT(1)
        sb_w1 = dve_const(w1blk, P, P, F32)
        sb_w2 = dve_const(w2blk, P, 90, F32)
        sb_id = dve_const(ident, P, P, F32)
        for t in range(2, NT):
            load_lhsT(t)

        # persistent h tiles: cols 0:90 rewritten each use; 90:128 junk is
        # zeroed once (col 95 = ones lane feeding the layer-1 bias row)
        hb = []
        for i in range(2):
            t = singles.tile([P, P], F32, name=f"hb{i}")
            nc.vector.memset(t[:, 90:P], 0.0)
            nc.vector.memset(t[:, 95:96], 1.0)
            hb.append(t)

        gate_all = singles.tile([P, NT * C], BF16)
        zall = singles.tile([P, NT * C], BF16)

        def conv_block(t):
            lt = lhsT[t]
            cvt = [cvps.tile([P, 330], F32, tag=f"cv{j}", name=f"cv{j}")
                   for j in range(3)]
            for j in range(3):
                for i, c in enumerate(PAIRS[j]):
                    nc.tensor.matmul(
                        cvt[j], lt[:, c * P:(c + 1) * P],
                        sb_w[:, (j * 4 + i) * 330:(j * 4 + i + 1) * 330],
                        start=(i == 0), stop=(i == len(PAIRS[j]) - 1))
            h = hb[t % 2]
            for j in range(3):
                nc.vector.tensor_reduce(
                    out=h[:, 15 * j:15 * j + 15],
                    in_=_ap(cvt[j][:, 0:330], [[22, 15], [1, L]]),
                    axis=AX.X, op=ALU.max)
            # avg lanes (col 21 of each 22-block): 2 on DVE, 1 on Act
            nc.vector.tensor_copy(out=h[:, 45:60],
                                  in_=_off(cvt[0], [[22, 15]], 21))
            nc.scalar.copy(out=h[:, 60:75], in_=_off(cvt[1], [[22, 15]], 21))
            nc.vector.tensor_copy(out=h[:, 75:90],
                                  in_=_off(cvt[2], [[22, 15]], 21))
            return h

        def mlp_block(t, h):
            # both MLP branches through 128x128 f32 blocks; PE transposes
            # with ones lanes via h col 95 and the saturated tanh col 127
            trm = ppps.tile([P, P], F32, tag="trm", name="trm")
            nc.tensor.transpose(trm, h, sb_id)
            hT = work.tile([P, P], F32, tag="hT", name="hT")
            nc.scalar.copy(out=hT, in_=trm)
            pp = ppps.tile([P, P], F32, tag="pp", name="pp")
            nc.tensor.matmul(pp, hT, sb_w1, start=True, stop=True)
            t1 = small.tile([P, P], F32, tag="t1", name="t1")
            nc.scalar.activation(out=t1, in_=pp, func=AF.Tanh)
            trm2 = ppps.tile([P, P], F32, tag="trm", name="trm2")
            nc.tensor.transpose(trm2, t1, sb_id)
            t1T = work.tile([P, P], F32, tag="t1T", name="t1T")
            nc.vector.tensor_copy(out=t1T, in_=trm2)
            p2 = ppps.tile([P, 90], F32, tag="pp", name="p2")
            nc.tensor.matmul(p2, t1T, sb_w2, start=True, stop=True)
            z2 = small.tile([P, 90], BF16, tag="z2", name="z2")
            nc.scalar.activation(out=z2, in_=p2, func=AF.Tanh)
            nc.vector.tensor_add(out=zall[:, t * C:(t + 1) * C],
                                 in0=z2[:, 0:45], in1=z2[:, 45:90])

        def softmax_half(lo, nt):
            # segmented softmax over tile blocks [lo, lo+nt) + output DMA
            zs = zall[:, lo * C:(lo + nt) * C]
            zmax = small.tile([P, nt], F32, tag="zmax", name="zmax")
            nc.vector.tensor_reduce(out=zmax, in_=_ap(zs, [[C, nt], [1, C]]),
                                    axis=AX.X, op=ALU.max)
            zmax16 = small.tile([P, nt], BF16, tag="zmax16", name="zmax16")
            nc.vector.tensor_copy(out=zmax16, in_=zmax)
            zsub = work.tile([P, nt * C], BF16, tag="zsub", name="zsub")
            nc.vector.tensor_sub(out=zsub, in0=zs,
                                 in1=_ap(zmax16, [[1, nt], [0, C]]))
            eg = work.tile([P, nt * C], BF16, tag="eg", name="eg")
            nc.scalar.activation(out=eg, in_=zsub, func=AF.Exp)
            ssum = small.tile([P, nt], F32, tag="ssum", name="ssum")
            nc.vector.tensor_reduce(out=ssum, in_=_ap(eg, [[C, nt], [1, C]]),
                                    axis=AX.X, op=ALU.add)
            rs = small.tile([P, nt], BF16, tag="rs", name="rs")
            nc.vector.reciprocal(out=rs, in_=ssum)
            gs = gate_all[:, lo * C:(lo + nt) * C]
            nc.vector.tensor_mul(out=gs, in0=eg,
                                 in1=_ap(rs, [[1, nt], [0, C]]))
            nc.sync.dma_start(out=gate_o[:, lo * C:(lo + nt) * C], in_=gs)

        # software-pipelined emission: engines issue in program order, so
        # interleave conv(t+1) ahead of mlp(t) to let tiles overlap; the
        # first softmax half runs under the back half of the pipeline
        hprev = None
        for t in range(NT + 1):
            if t < NT:
                hcur = conv_block(t)
            if t >= 1:
                mlp_block(t - 1, hprev)
            if t == NT // 2 + 1:
                softmax_half(0, NT // 2)
            hprev = hcur
        softmax_half(NT // 2, NT // 2)
    legalize_sync_waits(nc)
    return nc


# --------------------------------------------------------------------------
# Launch 2: expert attention via fitted moment polynomials
# --------------------------------------------------------------------------

def build_attn_program(degs):
    """degs: per-expert polynomial degree, sorted ascending (len 22)."""
    degs = list(degs)
    assert degs == sorted(degs)
    dmax = max(degs)
    # suffix start index for "experts with degree > k"
    estart = {k: next((i for i in range(E) if degs[i] > k), E)
              for k in range(dmax)}
    # first expert of each degree class (for acc initialization)
    class_start = {}
    for i, d in enumerate(degs):
        class_start.setdefault(d, i)

    nc = bass.Bass()
    u_d = nc.declare_dram_parameter("u", [BC, EL], BF16, isOutput=False)
    phi_d = nc.declare_dram_parameter("phi", [BC, EL], BF16, isOutput=False)
    # e-major transposed u and g*u: 4 chunks of 128 DRAM rows; chunk c rows
    # 0:126 = transposed rows [126c,126c+126) (row = 21*e_perm + l), rest 0
    uT_d = nc.declare_dram_parameter("uT", [4 * P, BC], BF16, isOutput=False)
    guT_d = nc.declare_dram_parameter("guT", [4 * P, BC], BF16,
                                      isOutput=False)
    ind_d = nc.declare_dram_parameter("ind", [P, 4 * E], BF16, isOutput=False)
    gpq_d = nc.declare_dram_parameter("gpq", [BC, 2 * E], BF16,
                                      isOutput=False)
    # per-row numerator coefficient scales: gpk[b, lev*22+e] = gp[b,e]*cn[lev,e]
    gpk_d = nc.declare_dram_parameter("gpk", [BC, (NLEV - 1) * E], BF16,
                                      isOutput=False)
    cd_d = nc.declare_dram_parameter("cd", [1, (NLEV - 1) * NT * E], BF16,
                                     isOutput=False)
    at_o = nc.declare_dram_parameter("at", [BC, EL], BF16, isOutput=True)
    gt_o = nc.declare_dram_parameter("gt", [BC, EL], BF16, isOutput=True)

    def shard_ap(dram, ncols):
        base = dram[:, :]
        return bass.AP(tensor=base.tensor, offset=base.offset,
                       ap=[[ncols, P], [P * ncols, NT], [1, ncols]])

    with tile.TileContext(nc) as tc, ExitStack() as ctx, \
            nc.allow_low_precision(reason="bf16 attn pipeline; 2e-2 tol"):
        sg = ctx.enter_context(tc.tile_pool(name="sg", bufs=1))
        wpool = ctx.enter_context(tc.tile_pool(name="wpool", bufs=1,
                                               space="PSUM"))

        guT = sg.tile([P, 4 * BC], BF16)
        gb = guT_d[:, :]
        nc.sync.dma_start(
            out=guT, in_=bass.AP(tensor=gb.tensor, offset=gb.offset,
                                 ap=[[BC, P], [P * BC, 4], [1, BC]]))
        uT = sg.tile([P, 4 * BC], BF16)
        ub = uT_d[:, :]
        nc.sync.dma_start(
            out=uT, in_=bass.AP(tensor=ub.tensor, offset=ub.offset,
                                ap=[[BC, P], [P * BC, 4], [1, BC]]))
        u = sg.tile([P, WF], BF16)
        nc.sync.dma_start(out=u, in_=shard_ap(u_d, EL))
        phi = sg.tile([P, WF], BF16)
        nc.sync.dma_start(out=phi, in_=shard_ap(phi_d, EL))
        indr = sg.tile([P, 4 * E], BF16)
        nc.sync.dma_start(out=indr, in_=ind_d[:, :])
        ind = sg.tile([P, 4 * E], BF16)
        nc.vector.tensor_copy(out=ind, in_=indr)
        gpq = sg.tile([P, NT * 2 * E], BF16)
        nc.sync.dma_start(out=gpq, in_=shard_ap(gpq_d, 2 * E))

        def bconst(dram, n, nm):
            base = dram[:, :]
            t = sg.tile([P, n], BF16, name=nm)
            nc.sync.dma_start(
                out=t, in_=bass.AP(tensor=base.tensor, offset=base.offset,
                                   ap=[[0, P], [1, n]]))
            return t

        NC_ = NT * E
        cdB = bconst(cd_d, (NLEV - 1) * NC_, "cdB")
        gpk = sg.tile([P, NT * (NLEV - 1) * E], BF16)
        nc.sync.dma_start(out=gpk, in_=shard_ap(gpk_d, (NLEV - 1) * E))

        # Wt layout: col = t*176 + lev*22 + e
        Wt = sg.tile([P, NT * NLEV * E], BF16)

        # slice helpers (l-major: free = (t, l, e); e innermost)
        def full3(tile_, e0=0, ne=E, coloff=0):
            return _off(tile_, [[EL, NT], [E, L], [1, ne]], coloff + e0)

        def coef(ctile, lev, e0=0, ne=E):
            return _off(ctile, [[(NLEV - 1) * E, NT], [0, L], [1, ne]],
                        lev * E + e0)

        # transposed chain: wT = exp(guT); vT_k = vT_{k-1}*uT; chunk c of the
        # free dim holds transposed rows [126c,126c+126) for batch cols
        wT = sg.tile([P, 4 * BC], BF16)
        nc.scalar.activation(out=wT, in_=guT, func=AF.Exp)
        vTa = sg.tile([P, 4 * BC], BF16)
        vTb = sg.tile([P, 4 * BC], BF16)

        # chunk start per level: experts with deg >= lev-1 live in chunks
        # >= estart[lev-2]//6 (extra experts in a straddling chunk are
        # harmless: their cd/cn consts are zero)
        def cstart(lev):
            if lev <= 2:
                return 0
            e0 = estart.get(lev - 2, E)
            return 4 if e0 >= E else e0 // 6

        wps = [wpool.tile([P, NLEV * E], F32, tag=f"wp{t}", name=f"wp{t}")
               for t in range(NT)]

        def moments(lev, vt):
            c0 = cstart(lev)
            for t in range(NT):
                for c in range(c0, 4):
                    nc.tensor.matmul(
                        wps[t][:, lev * E:(lev + 1) * E],
                        vt[0:126, c * BC + t * P:c * BC + (t + 1) * P],
                        ind[0:126, c * E:(c + 1) * E],
                        start=(c == c0), stop=(c == 3))

        moments(0, wT)
        cur, nxt = vTa, vTb
        nc.vector.tensor_mul(out=cur, in0=wT, in1=uT)
        moments(1, cur)
        for lev in range(2, NLEV):
            c0 = cstart(lev)
            if c0 >= 4:
                break
            off = c0 * BC
            nc.vector.tensor_mul(out=nxt[:, off:4 * BC],
                                 in0=cur[:, off:4 * BC],
                                 in1=uT[:, off:4 * BC])
            moments(lev, nxt)
            cur, nxt = nxt, cur

        # PSUM -> Wt copies, one per tile block (DVE/Act split)
        for t in range(NT):
            dst = Wt[:, t * NLEV * E:(t + 1) * NLEV * E]
            if t % 2 == 0:
                nc.vector.tensor_copy(out=dst, in_=wps[t])
            else:
                nc.scalar.copy(out=dst, in_=wps[t])

        # Horner coefficient tensors: D_k = W_k*cd_k, N_k = W_{k+1}*cn_k
        NL1 = (NLEV - 1) * E
        Dt = sg.tile([P, NT * NL1], BF16)
        nc.vector.tensor_mul(out=_ap(Dt, [[NL1, NT], [1, NL1]]),
                             in0=_ap(Wt, [[NLEV * E, NT], [1, NL1]]),
                             in1=cdB)
        Nt = sg.tile([P, NT * NL1], BF16)
        nc.vector.tensor_mul(out=_ap(Nt, [[NL1, NT], [1, NL1]]),
                             in0=_off(Wt, [[NLEV * E, NT], [1, NL1]], E),
                             in1=gpk)

        # nested mixed-degree Horner (experts sorted by degree ascending);
        # a class's accumulator starts life fused into its first step:
        # acc = coef(d)*phi + coef(d-1)
        accd = sg.tile([P, WF], BF16)
        accn = sg.tile([P, WF], BF16)
        for k in range(dmax - 1, -1, -1):
            e0 = estart[k]
            ne = E - e0
            cs = class_start.get(k + 1)
            for acc, Ct in ((accd, Dt), (accn, Nt)):
                if cs is not None:
                    ncs = (min([s for dd, s in class_start.items()
                                if dd > k + 1], default=E)) - cs
                    nc.vector.tensor_mul(out=full3(acc, cs, ncs),
                                         in0=coef(Ct, k + 1, cs, ncs),
                                         in1=full3(phi, cs, ncs))
                    e1 = cs + ncs
                    if E - e1 > 0:
                        nc.vector.tensor_mul(out=full3(acc, e1, E - e1),
                                             in0=full3(acc, e1, E - e1),
                                             in1=full3(phi, e1, E - e1))
                else:
                    nc.vector.tensor_mul(out=full3(acc, e0, ne),
                                         in0=full3(acc, e0, ne),
                                         in1=full3(phi, e0, ne))
                nc.vector.tensor_add(out=full3(acc, e0, ne),
                                     in0=full3(acc, e0, ne),
                                     in1=coef(Ct, k, e0, ne))

        # s = num/den; at = s*gp + gq; gt = at*u
        # (tail ops split 16/6 experts across DVE and Pool so they overlap)
        rden = sg.tile([P, WF], BF16)
        nc.vector.reciprocal(out=rden, in_=accd)
        ESP = 16
        at = accn
        nc.vector.tensor_mul(out=full3(at, 0, ESP), in0=full3(accn, 0, ESP),
                             in1=full3(rden, 0, ESP))
        nc.gpsimd.tensor_mul(out=full3(at, ESP, E - ESP),
                             in0=full3(accn, ESP, E - ESP),
                             in1=full3(rden, ESP, E - ESP))
        nc.vector.tensor_add(
            out=full3(at, 0, ESP), in0=full3(at, 0, ESP),
            in1=_off(gpq, [[2 * E, NT], [0, L], [1, ESP]], E))
        nc.gpsimd.tensor_add(
            out=full3(at, ESP, E - ESP), in0=full3(at, ESP, E - ESP),
            in1=_off(gpq, [[2 * E, NT], [0, L], [1, E - ESP]], E + ESP))
        gt = rden
        nc.gpsimd.tensor_mul(out=full3(gt, ESP, E - ESP),
                             in0=full3(at, ESP, E - ESP),
                             in1=full3(u, ESP, E - ESP))
        nc.vector.tensor_mul(out=full3(gt, 0, ESP),
                             in0=full3(at, 0, ESP),
                             in1=full3(u, 0, ESP))
        nc.sync.dma_start(out=shard_ap(at_o, EL), in_=at)
        nc.sync.dma_start(out=shard_ap(gt_o, EL), in_=gt)
    legalize_sync_waits(nc)
    return nc


# --------------------------------------------------------------------------
# Host-side preparation
# --------------------------------------------------------------------------

def _gate_params(inputs):
    gc_w = inputs["gc_w"].astype(np.float64)
    gc_b = inputs["gc_b"].astype(np.float64)
    KC = P * NT
    # full conv weight: rows = (chan,l) + pad + bias lane, cols = (chan, 22)
    wfull = np.zeros((KC, 990), np.float32)
    wavvec = gc_w.mean(0)
    for i in range(C):
        wfull[i * L:(i + 1) * L, i * 22:i * 22 + L] = gc_w.T
        wfull[i * L:(i + 1) * L, i * 22 + L] = wavvec
        wfull[KC - 1, i * 22:i * 22 + L] = gc_b
        wfull[KC - 1, i * 22 + L] = gc_b.mean()
    PAIRS = [[0, 1, 2, 7], [2, 3, 4, 7], [4, 5, 6, 7]]
    wpk = np.zeros((P, 12 * 330), np.float32)
    for j in range(3):
        for i, c in enumerate(PAIRS[j]):
            wpk[:, (j * 4 + i) * 330:(j * 4 + i + 1) * 330] = \
                wfull[c * P:(c + 1) * P, j * 330:(j + 1) * 330]
    # 128x128 MLP layer-1 block: rows = transposed h cols (0:45 mx, 45:90
    # av, 95 = ones), out cols 0:50 = both branch hiddens, col 127 driven to
    # +30 via the ones row so tanh saturates to an exact 1.0 "ones" lane for
    # layer 2; all other cells zero.
    w1blk = np.zeros((128, 128), np.float32)
    w1blk[0:45, 0:25] = inputs["w1"].T
    w1blk[45:90, 25:50] = inputs["w1"].T
    w1blk[95, 0:25] = inputs["b1"]
    w1blk[95, 25:50] = inputs["b1"]
    w1blk[95, 127] = 30.0
    w2blk = np.zeros((128, 90), np.float32)
    w2blk[0:25, 0:45] = inputs["w2"].T
    w2blk[25:50, 45:90] = inputs["w2"].T
    w2blk[127, 0:45] = inputs["b2"]
    w2blk[127, 45:90] = inputs["b2"]
    return wpk, w1blk, w2blk


def _fit_exp(tsamp, K, wsamp=None, ntail=0.5):
    t = np.asarray(tsamp, np.float64)
    w = np.ones_like(t) if wsamp is None else np.asarray(wsamp, np.float64)
    tm = max(np.abs(t).max(), 1e-3)
    textra = np.linspace(-tm, tm, 64)
    t = np.concatenate([t, textra])
    w = np.concatenate([w, np.full(64, ntail * w.mean())])
    V = np.vander(t, K + 1, increasing=True) * w[:, None]
    c, *_ = np.linalg.lstsq(V, np.exp(t) * w, rcond=None)
    return c


_CACHE = {}


def kernel(**inputs):
    inputs = {k: np.ascontiguousarray(np.asarray(v)) for k, v in
              inputs.items()}
    x = inputs["x"].astype(np.float32)            # (B, C, L)
    import ml_dtypes
    bf = ml_dtypes.bfloat16

    wpk, w1blk, w2blk = _gate_params(inputs)
    cores = list(range(NCORES))
    KC = P * NT

    if "gate" not in _CACHE:
        _CACHE["gate"] = build_gate_program()
    nc1 = _CACHE["gate"]
    maps1 = []
    for i in cores:
        xt = np.zeros((KC, KC), np.float32)
        xt[0:CL, :] = x.reshape(B, CL)[i * BC:(i + 1) * BC].T
        xt[KC - 1, :] = 1.0
        maps1.append({"xT": xt.astype(bf), "wpk": wpk.astype(bf),
                      "w1blk": w1blk, "w2blk": w2blk,
                      "ident": np.eye(P, dtype=np.float32)})
    r1 = run_bass_kernel_spmd(nc1, maps1, cores).results
    # gate tiles come back as (P, NT*C): row p, block t -> batch p + t*P
    gate = np.zeros((B, C), np.float32)
    for i in cores:
        g = np.asarray(r1[i]["gate"]).astype(np.float32)
        gate[i * BC:(i + 1) * BC] = \
            g.reshape(P, NT, C).transpose(1, 0, 2).reshape(BC, C)
    mean_gate = gate.astype(np.float64).mean(0)
    sel = np.sort(np.argsort(-mean_gate, kind="stable")[:E])

    # expert scalars
    wq, bq = inputs["wq"], inputs["bq"]
    wk, bk = inputs["wk"], inputs["bk"]
    wv, bv = inputs["wv"], inputs["bv"]
    wo, bo = inputs["wo"], inputs["bo"]
    alpha = (wq * wk).sum(1).astype(np.float32)
    gamma = (bq * wk).sum(1).astype(np.float32)
    pv = (wo * wv).sum(1).astype(np.float32)
    qv = ((wo * bv).sum(1) + bo).astype(np.float32)

    usel = x[:, sel, :]                            # (B, E, L)
    # per-expert tau range -> degree ladder
    phimax = np.abs(alpha[None, :, None] * usel).max(axis=(0, 2))
    umax = np.abs(usel).max(axis=(0, 2))
    taumax = phimax * umax
    degs_raw = np.where(taumax <= 0.35, 1,
                        np.where(taumax <= 0.7, 2,
                                 np.where(taumax <= 1.2, 3,
                                          np.where(taumax <= 2.0, 4, 6))))
    perm = np.argsort(degs_raw, kind="stable")     # experts by degree asc
    degs = degs_raw[perm]

    # coefficient fits per (permuted) expert
    rng = np.random.RandomState(12345)
    cd = np.zeros((NLEV - 1, E), np.float32)
    cn = np.zeros((NLEV - 1, E), np.float32)
    sub = usel[::16]                               # (B/16, E, L) samples
    for j, e in enumerate(perm):
        K = int(degs[j])
        ue = sub[:, e, :].astype(np.float64)
        tau = (alpha[e] * ue[:, :, None] * ue[:, None, :]).ravel()
        uw = np.abs(np.broadcast_to(ue[:, None, :], ue.shape[:1] + (L, L))
                    ).ravel()
        ss = rng.choice(tau.size, min(40000, tau.size), replace=False)
        cd[0:K + 1, j] = _fit_exp(tau[ss], K)
        cn[0:K + 1, j] = _fit_exp(tau[ss], K, wsamp=uw[ss] + 0.1)

    # device tensors (l-major, expert-permuted)
    uselp = usel[:, perm, :]
    u_lm = np.ascontiguousarray(uselp.transpose(0, 2, 1).reshape(B, EL))
    phi_lm = np.ascontiguousarray(
        (alpha[perm][None, :, None] * uselp).transpose(0, 2, 1)
        .reshape(B, EL))
    gusel = gamma[perm][None, :, None] * uselp                  # (B,E,L)

    def emaj_chunks(arr):
        # (B,E,L) -> e-major transposed (462,B) in 4 x 128-row DRAM chunks
        tr = arr.transpose(1, 2, 0).reshape(EL, B)
        out = np.zeros((4 * P, B), np.float32)
        for c in range(4):
            r0, r1 = 126 * c, min(126 * (c + 1), EL)
            out[128 * c:128 * c + (r1 - r0)] = tr[r0:r1]
        return out

    uT_pad = emaj_chunks(uselp)
    guT_pad = emaj_chunks(gusel)
    ind = np.zeros((P, 4 * E), np.float32)
    for c in range(4):
        for p_ in range(126):
            r = 126 * c + p_
            if r < EL:
                ind[p_, c * E + r // L] = 1.0
    gsel = gate[:, sel][:, perm]
    gp = gsel * pv[perm][None, :]
    gq = gsel * qv[perm][None, :]
    gpq = np.concatenate([gp, gq], 1).astype(np.float32)   # (B, 44)
    # fold gp into the numerator coefficient scales per row
    gpk = (gp[:, None, :] * cn[None, :, :]).reshape(
        B, (NLEV - 1) * E).astype(np.float32)

    NC_ = NT * E
    # coefficient layout (t, lev, e) to match the per-tile moment blocks
    cd_full = np.tile(cd[None, :, :], (NT, 1, 1)).reshape(1, (NLEV - 1) * NC_)

    key = (tuple(int(d) for d in degs),)
    if _CACHE.get("attn_key") != key:
        _CACHE["attn"] = build_attn_program([int(d) for d in degs])
        _CACHE["attn_key"] = key
    nc2 = _CACHE["attn"]
    maps2 = [{"u": u_lm[i * BC:(i + 1) * BC].astype(bf),
              "phi": phi_lm[i * BC:(i + 1) * BC].astype(bf),
              "uT": uT_pad[:, i * BC:(i + 1) * BC].astype(bf),
              "guT": guT_pad[:, i * BC:(i + 1) * BC].astype(bf),
              "ind": ind.astype(bf),
              "gpq": gpq[i * BC:(i + 1) * BC].astype(bf),
              "gpk": gpk[i * BC:(i + 1) * BC].astype(bf),
              "cd": cd_full.astype(bf)}
             for i in cores]
    r2 = run_bass_kernel_spmd(nc2, maps2, cores).results
    at = np.concatenate([np.asarray(r["at"]).astype(np.float32)
                         for r in r2], 0)          # (B, 462) l-major perm
    gt = np.concatenate([np.asarray(r["gt"]).astype(np.float32)
                         for r in r2], 0)

    inv = np.argsort(perm)
    at_e = at.reshape(B, L, E).transpose(0, 2, 1)[:, inv, :]   # (B,E,L)
    gt_e = gt.reshape(B, L, E).transpose(0, 2, 1)[:, inv, :]
    A_full = np.zeros((B, C, L), np.float32)
    G_full = np.zeros((B, C, L), np.float32)
    A_full[:, sel, :] = at_e
    G_full[:, sel, :] = gt_e
    return G_full.reshape(B, CL), A_full.reshape(B, CL)


# revision 61
# speedup vs baseline: 1.1939x; 1.0053x over previous
"""Trainium2 Bass kernel for grouped-attention MoE routing.

Math (derived from the nn.Module):
  gate  = softmax(mlp(maxpool(conv(x))) + mlp(avgpool(conv(x))))      (B,45)
  sel   = sorted(top22(mean_b gate))                                  (22,)
  Per expert e with u = x[:, sel[e], :]:
    energy[l,m] = (a_e*u_l + g_e) * u_m   (rank-1; scalars a,g from weights)
    attn = softmax_m(energy);  s_l = sum_m u_m attn[l,m]
    y_l  = P_e*s_l + Q_e;      A[:,sel[e],:] = y * gate[:,sel[e]]
  G = x * A (flat);  return (G, A_flat)

Implementation strategy (v2):
  Launch 1 (gate): bf16 PE conv with bias folded via a 127th ones row and
    the avg-pool folded in as extra matmul columns; both MLP branches run
    through one block-diagonal matmul pair; outputs batched into one DMA.
  Launch 2 (attn): the rank-1 softmax is evaluated without the LxL energy
    tensor.  With w = exp(g_e*u) and phi = a_e*u:
      den(phi_l) = sum_m w_m exp(phi_l u_m) ~= sum_k cd_k phi_l^k W_k
      num(phi_l) = sum_m u_m w_m exp(phi_l u_m) ~= sum_k cn_k phi_l^k W_{k+1}
    where W_k = sum_m w_m u_m^k are on-device moments and cd/cn are host-
    fitted per-expert polynomial coefficients (least squares over the
    empirical tau = phi*u range; numerator fit |u|-weighted).  s = num/den.
    Per-expert degree ladder (2..6) by empirical |tau| range; experts are
    permuted so degree classes are contiguous and the nested Horner only
    touches suffix slices for the high degrees.  Everything bf16, l-major
    (l outer, e inner) so per-(b,e) coefficient broadcasts stay packed.
  Routing (45-float mean-gate reduction) is mediated on host between the
  two launches, equivalent to the all-reduce in the sharding hint.
"""

import math
import numpy as np
from contextlib import ExitStack

import bass_rust
import concourse.bass as bass
import concourse.mybir as mybir
import concourse.tile as tile
from concourse.bass_utils import run_bass_kernel_spmd

_MULTIWAIT_OK = ("InstNoOp", "InstAllEngineBarrier",
                 "InstEventSemaphore", "InstUnconditionalBranch")


def legalize_sync_waits(nc):
    """walrus codegen on this stack rejects >1 sync wait on most
    instructions; hoist extra waits onto same-engine NoOps."""
    for func in nc.m.functions:
        for block in func.blocks:
            il = block.instructions
            out = []
            for inst in il:
                tname = type(inst).__name__
                si = getattr(inst, "sync_info", None)
                waits = list(si.on_wait) if si is not None else []
                if tname not in _MULTIWAIT_OK and len(waits) > 1:
                    for k, w in enumerate(waits):
                        nop = mybir.InstNoOp(
                            name=f"{inst.name}-synop{k}", ins=[], outs=[])
                        nop.engine = inst.engine
                        nop.sync_info = bass_rust.SyncInfo(
                            on_wait=[w], on_update=[])
                        out.append(nop)
                    inst.sync_info = bass_rust.SyncInfo(
                        on_wait=[], on_update=list(inst.sync_info.on_update))
                out.append(inst)
            il.clear()
            il.extend(out)


B, C, L, E = 8192, 45, 21, 22
NCORES = 8
BC = B // NCORES          # rows per core
P = 128                   # SBUF partitions
NT = BC // P              # batch tiles per core
CL = C * L                # 945
EL = E * L                # 462
WF = NT * EL              # 3696 full-shard free width (t, l, e) l-major
F32 = mybir.dt.float32
BF16 = mybir.dt.bfloat16
AF = mybir.ActivationFunctionType
ALU = mybir.AluOpType
AX = mybir.AxisListType

NG = 8                    # conv channel groups (6,6,...,3 channels)
GCH = [list(range(g, min(g + 6, C))) for g in range(0, C, 6)]
NLEV = 8                  # moment levels W_0..W_7 held on device
DEG_MAX = 6


def _ap(base, extra_free):
    """Custom free-dim access pattern on an SBUF tile slice (partition
    dim kept from `base`)."""
    return bass.AP(tensor=base.tensor, offset=base.offset,
                   ap=[base.ap[0]] + extra_free)


def _off(base, extra_free, col_off):
    ap = bass.AP(tensor=base.tensor, offset=base.offset,
                 ap=[base.ap[0]] + extra_free)
    ap.offset = ap.offset + col_off
    return ap


# --------------------------------------------------------------------------
# Launch 1: gating network
# --------------------------------------------------------------------------

def build_gate_program():
    """Gate launch. Host uploads x TRANSPOSED and padded: XT (1024, 1024)
    with rows 0:945 = x[coreshard].T (row = (chan,l)), rows 945:1023 = 0,
    row 1023 = 1 (bias lane).  The conv+avg matmul accumulates chunk-wise
    into 3 PSUM column blocks of 15 channels x 22 outputs; no on-device
    transposes or PSUM->SBUF copies are needed for the conv at all."""
    nc = bass.Bass()
    KC = P * NT                       # 1024 padded contraction rows
    xT = nc.declare_dram_parameter("xT", [KC, KC], BF16, isOutput=False)
    # packed rhs slices for the 12 (chunk, block) matmuls: (128, 12*330)
    wpk = nc.declare_dram_parameter("wpk", [P, 12 * 330], BF16,
                                    isOutput=False)
    w1blk = nc.declare_dram_parameter("w1blk", [P, P], F32, isOutput=False)
    w2blk = nc.declare_dram_parameter("w2blk", [P, 90], F32, isOutput=False)
    ident = nc.declare_dram_parameter("ident", [P, P], F32, isOutput=False)
    gate_o = nc.declare_dram_parameter("gate", [P, NT * C], BF16,
                                       isOutput=True)

    # (chunk, block) pairs: block j covers channels 15j..15j+14 =
    # contraction rows 315j..315j+314, plus the bias lane in chunk 7
    PAIRS = [[0, 1, 2, 7], [2, 3, 4, 7], [4, 5, 6, 7]]

    with tile.TileContext(nc) as tc, ExitStack() as ctx, \
            nc.allow_low_precision(reason="bf16 gate pipeline; 2e-2 tol"):
        singles = ctx.enter_context(tc.tile_pool(name="singles", bufs=1))
        cvps = ctx.enter_context(tc.tile_pool(name="cvps", bufs=2,
                                              space="PSUM"))
        ppps = ctx.enter_context(tc.tile_pool(name="ppps", bufs=1,
                                              space="PSUM"))
        work = ctx.enter_context(tc.tile_pool(name="work", bufs=2))
        small = ctx.enter_context(tc.tile_pool(name="small", bufs=3))

        def dve_const(dram, p, n, dt=BF16):
            raw = singles.tile([p, n], dt, name="raw_" + dram.name)
            nc.sync.dma_start(out=raw, in_=dram[:, :])
            t = singles.tile([p, n], dt, name="sb_" + dram.name)
            nc.vector.tensor_copy(out=t, in_=raw)
            return t

        sb_w = dve_const(wpk, P, 12 * 330)

        # xT loads: per-tile DMA of the (128, 8x128) lhsT panel so tile 0
        # can start early.  lhs[p, c*128+b] = XT[128c+p, 128t+b]
        xb = xT[:, :]
        lhsT = []

        def load_lhsT(t):
            lt = singles.tile([P, NT * P], BF16, name=f"lhsT{t}")
            ap = bass.AP(tensor=xb.tensor, offset=xb.offset,
                         ap=[[KC, P], [P * KC, NT], [1, P]])
            ap.offset = ap.offset + t * P
            nc.sync.dma_start(out=lt, in_=ap)
            lhsT.append(lt)

        load_lhsT(0)
        load_lhsT(1)
        sb_w1 = dve_const(w1blk, P, P, F32)
        sb_w2 = dve_const(w2blk, P, 90, F32)
        sb_id = dve_const(ident, P, P, F32)
        for t in range(2, NT):
            load_lhsT(t)

        # persistent h tiles: cols 0:90 rewritten each use; 90:128 junk is
        # zeroed once (col 95 = ones lane feeding the layer-1 bias row)
        hb = []
        for i in range(2):
            t = singles.tile([P, P], F32, name=f"hb{i}")
            nc.vector.memset(t[:, 90:P], 0.0)
            nc.vector.memset(t[:, 95:96], 1.0)
            hb.append(t)

        gate_all = singles.tile([P, NT * C], BF16)
        zall = singles.tile([P, NT * C], BF16)

        def conv_block(t):
            lt = lhsT[t]
            cvt = [cvps.tile([P, 330], F32, tag=f"cv{j}", name=f"cv{j}")
                   for j in range(3)]
            for j in range(3):
                for i, c in enumerate(PAIRS[j]):
                    nc.tensor.matmul(
                        cvt[j], lt[:, c * P:(c + 1) * P],
                        sb_w[:, (j * 4 + i) * 330:(j * 4 + i + 1) * 330],
                        start=(i == 0), stop=(i == len(PAIRS[j]) - 1))
            h = hb[t % 2]
            for j in range(3):
                nc.vector.tensor_reduce(
                    out=h[:, 15 * j:15 * j + 15],
                    in_=_ap(cvt[j][:, 0:330], [[22, 15], [1, L]]),
                    axis=AX.X, op=ALU.max)
            # avg lanes (col 21 of each 22-block): 2 on DVE, 1 on Act
            nc.vector.tensor_copy(out=h[:, 45:60],
                                  in_=_off(cvt[0], [[22, 15]], 21))
            nc.scalar.copy(out=h[:, 60:75], in_=_off(cvt[1], [[22, 15]], 21))
            nc.vector.tensor_copy(out=h[:, 75:90],
                                  in_=_off(cvt[2], [[22, 15]], 21))
            return h

        def mlp_block(t, h):
            # both MLP branches through 128x128 f32 blocks; PE transposes
            # with ones lanes via h col 95 and the saturated tanh col 127
            trm = ppps.tile([P, P], F32, tag="trm", name="trm")
            nc.tensor.transpose(trm, h, sb_id)
            hT = work.tile([P, P], F32, tag="hT", name="hT")
            nc.scalar.copy(out=hT, in_=trm)
            pp = ppps.tile([P, P], F32, tag="pp", name="pp")
            nc.tensor.matmul(pp, hT, sb_w1, start=True, stop=True)
            t1 = small.tile([P, P], F32, tag="t1", name="t1")
            nc.scalar.activation(out=t1, in_=pp, func=AF.Tanh)
            trm2 = ppps.tile([P, P], F32, tag="trm", name="trm2")
            nc.tensor.transpose(trm2, t1, sb_id)
            t1T = work.tile([P, P], F32, tag="t1T", name="t1T")
            nc.vector.tensor_copy(out=t1T, in_=trm2)
            p2 = ppps.tile([P, 90], F32, tag="pp", name="p2")
            nc.tensor.matmul(p2, t1T, sb_w2, start=True, stop=True)
            z2 = small.tile([P, 90], BF16, tag="z2", name="z2")
            nc.scalar.activation(out=z2, in_=p2, func=AF.Tanh)
            nc.vector.tensor_add(out=zall[:, t * C:(t + 1) * C],
                                 in0=z2[:, 0:45], in1=z2[:, 45:90])

        def softmax_half(lo, nt):
            # segmented softmax over tile blocks [lo, lo+nt) + output DMA
            zs = zall[:, lo * C:(lo + nt) * C]
            zmax = small.tile([P, nt], F32, tag="zmax", name="zmax")
            nc.vector.tensor_reduce(out=zmax, in_=_ap(zs, [[C, nt], [1, C]]),
                                    axis=AX.X, op=ALU.max)
            zmax16 = small.tile([P, nt], BF16, tag="zmax16", name="zmax16")
            nc.vector.tensor_copy(out=zmax16, in_=zmax)
            zsub = work.tile([P, nt * C], BF16, tag="zsub", name="zsub")
            nc.vector.tensor_sub(out=zsub, in0=zs,
                                 in1=_ap(zmax16, [[1, nt], [0, C]]))
            eg = work.tile([P, nt * C], BF16, tag="eg", name="eg")
            nc.scalar.activation(out=eg, in_=zsub, func=AF.Exp)
            ssum = small.tile([P, nt], F32, tag="ssum", name="ssum")
            nc.vector.tensor_reduce(out=ssum, in_=_ap(eg, [[C, nt], [1, C]]),
                                    axis=AX.X, op=ALU.add)
            rs = small.tile([P, nt], BF16, tag="rs", name="rs")
            nc.vector.reciprocal(out=rs, in_=ssum)
            gs = gate_all[:, lo * C:(lo + nt) * C]
            nc.vector.tensor_mul(out=gs, in0=eg,
                                 in1=_ap(rs, [[1, nt], [0, C]]))
            nc.sync.dma_start(out=gate_o[:, lo * C:(lo + nt) * C], in_=gs)

        # software-pipelined emission: engines issue in program order, so
        # interleave conv(t+1) ahead of mlp(t) to let tiles overlap; the
        # first softmax half runs under the back half of the pipeline
        hprev = None
        for t in range(NT + 1):
            if t < NT:
                hcur = conv_block(t)
            if t >= 1:
                mlp_block(t - 1, hprev)
            if t == NT // 2 + 1:
                softmax_half(0, NT // 2)
            hprev = hcur
        softmax_half(NT // 2, NT // 2)
    legalize_sync_waits(nc)
    return nc


# --------------------------------------------------------------------------
# Launch 2: expert attention via fitted moment polynomials
# --------------------------------------------------------------------------

def build_attn_program(degs):
    """degs: per-expert polynomial degree, sorted ascending (len 22)."""
    degs = list(degs)
    assert degs == sorted(degs)
    dmax = max(degs)
    # suffix start index for "experts with degree > k"
    estart = {k: next((i for i in range(E) if degs[i] > k), E)
              for k in range(dmax)}
    # first expert of each degree class (for acc initialization)
    class_start = {}
    for i, d in enumerate(degs):
        class_start.setdefault(d, i)

    nc = bass.Bass()
    u_d = nc.declare_dram_parameter("u", [BC, EL], BF16, isOutput=False)
    phi_d = nc.declare_dram_parameter("phi", [BC, EL], BF16, isOutput=False)
    # e-major transposed u and g*u: 4 chunks of 128 DRAM rows; chunk c rows
    # 0:126 = transposed rows [126c,126c+126) (row = 21*e_perm + l), rest 0
    uT_d = nc.declare_dram_parameter("uT", [4 * P, BC], BF16, isOutput=False)
    guT_d = nc.declare_dram_parameter("guT", [4 * P, BC], BF16,
                                      isOutput=False)
    ind_d = nc.declare_dram_parameter("ind", [P, 4 * E], BF16, isOutput=False)
    gpq_d = nc.declare_dram_parameter("gpq", [BC, 2 * E], BF16,
                                      isOutput=False)
    # per-row numerator coefficient scales: gpk[b, lev*22+e] = gp[b,e]*cn[lev,e]
    gpk_d = nc.declare_dram_parameter("gpk", [BC, (NLEV - 1) * E], BF16,
                                      isOutput=False)
    cd_d = nc.declare_dram_parameter("cd", [1, (NLEV - 1) * NT * E], BF16,
                                     isOutput=False)
    at_o = nc.declare_dram_parameter("at", [BC, EL], BF16, isOutput=True)
    gt_o = nc.declare_dram_parameter("gt", [BC, EL], BF16, isOutput=True)

    def shard_ap(dram, ncols):
        base = dram[:, :]
        return bass.AP(tensor=base.tensor, offset=base.offset,
                       ap=[[ncols, P], [P * ncols, NT], [1, ncols]])

    with tile.TileContext(nc) as tc, ExitStack() as ctx, \
            nc.allow_low_precision(reason="bf16 attn pipeline; 2e-2 tol"):
        sg = ctx.enter_context(tc.tile_pool(name="sg", bufs=1))
        wpool = ctx.enter_context(tc.tile_pool(name="wpool", bufs=1,
                                               space="PSUM"))

        guT = sg.tile([P, 4 * BC], BF16)
        uT = sg.tile([P, 4 * BC], BF16)
        for hh in range(2):
            for dram, tile_ in ((guT_d, guT), (uT_d, uT)):
                bbb = dram[:, :]
                ap = bass.AP(tensor=bbb.tensor, offset=bbb.offset,
                             ap=[[BC, P], [P * BC, 2], [1, BC]])
                ap.offset = ap.offset + hh * 2 * P * BC
                nc.sync.dma_start(
                    out=tile_[:, hh * 2 * BC:(hh + 1) * 2 * BC], in_=ap)
        u = sg.tile([P, WF], BF16)
        nc.sync.dma_start(out=u, in_=shard_ap(u_d, EL))
        phi = sg.tile([P, WF], BF16)
        nc.sync.dma_start(out=phi, in_=shard_ap(phi_d, EL))
        indr = sg.tile([P, 4 * E], BF16)
        nc.sync.dma_start(out=indr, in_=ind_d[:, :])
        ind = sg.tile([P, 4 * E], BF16)
        nc.vector.tensor_copy(out=ind, in_=indr)
        gpq = sg.tile([P, NT * 2 * E], BF16)
        nc.sync.dma_start(out=gpq, in_=shard_ap(gpq_d, 2 * E))

        def bconst(dram, n, nm):
            base = dram[:, :]
            t = sg.tile([P, n], BF16, name=nm)
            nc.sync.dma_start(
                out=t, in_=bass.AP(tensor=base.tensor, offset=base.offset,
                                   ap=[[0, P], [1, n]]))
            return t

        NC_ = NT * E
        cdB = bconst(cd_d, (NLEV - 1) * NC_, "cdB")
        gpk = sg.tile([P, NT * (NLEV - 1) * E], BF16)
        nc.sync.dma_start(out=gpk, in_=shard_ap(gpk_d, (NLEV - 1) * E))

        # Wt layout: col = t*176 + lev*22 + e
        Wt = sg.tile([P, NT * NLEV * E], BF16)

        # slice helpers (l-major: free = (t, l, e); e innermost)
        def full3(tile_, e0=0, ne=E, coloff=0):
            return _off(tile_, [[EL, NT], [E, L], [1, ne]], coloff + e0)

        def coef(ctile, lev, e0=0, ne=E):
            return _off(ctile, [[(NLEV - 1) * E, NT], [0, L], [1, ne]],
                        lev * E + e0)

        # transposed chain: wT = exp(guT); vT_k = vT_{k-1}*uT; chunk c of the
        # free dim holds transposed rows [126c,126c+126) for batch cols
        wT = sg.tile([P, 4 * BC], BF16)
        nc.scalar.activation(out=wT[:, 0:2 * BC], in_=guT[:, 0:2 * BC],
                             func=AF.Exp)
        nc.scalar.activation(out=wT[:, 2 * BC:4 * BC],
                             in_=guT[:, 2 * BC:4 * BC], func=AF.Exp)
        vTa = sg.tile([P, 4 * BC], BF16)
        vTb = sg.tile([P, 4 * BC], BF16)

        # chunk start per level: experts with deg >= lev-1 live in chunks
        # >= estart[lev-2]//6 (extra experts in a straddling chunk are
        # harmless: their cd/cn consts are zero)
        def cstart(lev):
            if lev <= 2:
                return 0
            e0 = estart.get(lev - 2, E)
            return 4 if e0 >= E else e0 // 6

        wps = [wpool.tile([P, NLEV * E], F32, tag=f"wp{t}", name=f"wp{t}")
               for t in range(NT)]

        def moments(lev, vt):
            c0 = cstart(lev)
            for t in range(NT):
                for c in range(c0, 4):
                    nc.tensor.matmul(
                        wps[t][:, lev * E:(lev + 1) * E],
                        vt[0:126, c * BC + t * P:c * BC + (t + 1) * P],
                        ind[0:126, c * E:(c + 1) * E],
                        start=(c == c0), stop=(c == 3))

        moments(0, wT)
        cur, nxt = vTa, vTb
        nc.vector.tensor_mul(out=cur[:, 0:2 * BC], in0=wT[:, 0:2 * BC],
                             in1=uT[:, 0:2 * BC])
        nc.vector.tensor_mul(out=cur[:, 2 * BC:4 * BC],
                             in0=wT[:, 2 * BC:4 * BC],
                             in1=uT[:, 2 * BC:4 * BC])
        moments(1, cur)
        for lev in range(2, NLEV):
            c0 = cstart(lev)
            if c0 >= 4:
                break
            off = c0 * BC
            if c0 < 2:
                nc.vector.tensor_mul(out=nxt[:, off:2 * BC],
                                     in0=cur[:, off:2 * BC],
                                     in1=uT[:, off:2 * BC])
                nc.vector.tensor_mul(out=nxt[:, 2 * BC:4 * BC],
                                     in0=cur[:, 2 * BC:4 * BC],
                                     in1=uT[:, 2 * BC:4 * BC])
            else:
                nc.vector.tensor_mul(out=nxt[:, off:4 * BC],
                                     in0=cur[:, off:4 * BC],
                                     in1=uT[:, off:4 * BC])
            moments(lev, nxt)
            cur, nxt = nxt, cur

        # PSUM -> Wt copies, one per tile block (DVE/Act split)
        for t in range(NT):
            dst = Wt[:, t * NLEV * E:(t + 1) * NLEV * E]
            if t % 2 == 0:
                nc.vector.tensor_copy(out=dst, in_=wps[t])
            else:
                nc.scalar.copy(out=dst, in_=wps[t])

        # Horner coefficient tensors: D_k = W_k*cd_k, N_k = W_{k+1}*cn_k
        NL1 = (NLEV - 1) * E
        Dt = sg.tile([P, NT * NL1], BF16)
        nc.vector.tensor_mul(out=_ap(Dt, [[NL1, NT], [1, NL1]]),
                             in0=_ap(Wt, [[NLEV * E, NT], [1, NL1]]),
                             in1=cdB)
        Nt = sg.tile([P, NT * NL1], BF16)
        nc.vector.tensor_mul(out=_ap(Nt, [[NL1, NT], [1, NL1]]),
                             in0=_off(Wt, [[NLEV * E, NT], [1, NL1]], E),
                             in1=gpk)

        # nested mixed-degree Horner (experts sorted by degree ascending);
        # a class's accumulator starts life fused into its first step:
        # acc = coef(d)*phi + coef(d-1)
        accd = sg.tile([P, WF], BF16)
        accn = sg.tile([P, WF], BF16)
        for k in range(dmax - 1, -1, -1):
            e0 = estart[k]
            ne = E - e0
            cs = class_start.get(k + 1)
            for acc, Ct in ((accd, Dt), (accn, Nt)):
                if cs is not None:
                    ncs = (min([s for dd, s in class_start.items()
                                if dd > k + 1], default=E)) - cs
                    nc.vector.tensor_mul(out=full3(acc, cs, ncs),
                                         in0=coef(Ct, k + 1, cs, ncs),
                                         in1=full3(phi, cs, ncs))
                    e1 = cs + ncs
                    if E - e1 > 0:
                        nc.vector.tensor_mul(out=full3(acc, e1, E - e1),
                                             in0=full3(acc, e1, E - e1),
                                             in1=full3(phi, e1, E - e1))
                else:
                    nc.vector.tensor_mul(out=full3(acc, e0, ne),
                                         in0=full3(acc, e0, ne),
                                         in1=full3(phi, e0, ne))
                nc.vector.tensor_add(out=full3(acc, e0, ne),
                                     in0=full3(acc, e0, ne),
                                     in1=coef(Ct, k, e0, ne))

        # s = num/den; at = s*gp + gq; gt = at*u
        # (tail ops split 16/6 experts across DVE and Pool so they overlap)
        rden = sg.tile([P, WF], BF16)
        nc.vector.reciprocal(out=rden, in_=accd)
        ESP = 16
        at = accn
        nc.vector.tensor_mul(out=full3(at, 0, ESP), in0=full3(accn, 0, ESP),
                             in1=full3(rden, 0, ESP))
        nc.gpsimd.tensor_mul(out=full3(at, ESP, E - ESP),
                             in0=full3(accn, ESP, E - ESP),
                             in1=full3(rden, ESP, E - ESP))
        nc.vector.tensor_add(
            out=full3(at, 0, ESP), in0=full3(at, 0, ESP),
            in1=_off(gpq, [[2 * E, NT], [0, L], [1, ESP]], E))
        nc.gpsimd.tensor_add(
            out=full3(at, ESP, E - ESP), in0=full3(at, ESP, E - ESP),
            in1=_off(gpq, [[2 * E, NT], [0, L], [1, E - ESP]], E + ESP))
        gt = rden
        nc.gpsimd.tensor_mul(out=full3(gt, ESP, E - ESP),
                             in0=full3(at, ESP, E - ESP),
                             in1=full3(u, ESP, E - ESP))
        nc.vector.tensor_mul(out=full3(gt, 0, ESP),
                             in0=full3(at, 0, ESP),
                             in1=full3(u, 0, ESP))
        nc.sync.dma_start(out=shard_ap(at_o, EL), in_=at)
        nc.sync.dma_start(out=shard_ap(gt_o, EL), in_=gt)
    legalize_sync_waits(nc)
    return nc


# --------------------------------------------------------------------------
# Host-side preparation
# --------------------------------------------------------------------------

def _gate_params(inputs):
    gc_w = inputs["gc_w"].astype(np.float64)
    gc_b = inputs["gc_b"].astype(np.float64)
    KC = P * NT
    # full conv weight: rows = (chan,l) + pad + bias lane, cols = (chan, 22)
    wfull = np.zeros((KC, 990), np.float32)
    wavvec = gc_w.mean(0)
    for i in range(C):
        wfull[i * L:(i + 1) * L, i * 22:i * 22 + L] = gc_w.T
        wfull[i * L:(i + 1) * L, i * 22 + L] = wavvec
        wfull[KC - 1, i * 22:i * 22 + L] = gc_b
        wfull[KC - 1, i * 22 + L] = gc_b.mean()
    PAIRS = [[0, 1, 2, 7], [2, 3, 4, 7], [4, 5, 6, 7]]
    wpk = np.zeros((P, 12 * 330), np.float32)
    for j in range(3):
        for i, c in enumerate(PAIRS[j]):
            wpk[:, (j * 4 + i) * 330:(j * 4 + i + 1) * 330] = \
                wfull[c * P:(c + 1) * P, j * 330:(j + 1) * 330]
    # 128x128 MLP layer-1 block: rows = transposed h cols (0:45 mx, 45:90
    # av, 95 = ones), out cols 0:50 = both branch hiddens, col 127 driven to
    # +30 via the ones row so tanh saturates to an exact 1.0 "ones" lane for
    # layer 2; all other cells zero.
    w1blk = np.zeros((128, 128), np.float32)
    w1blk[0:45, 0:25] = inputs["w1"].T
    w1blk[45:90, 25:50] = inputs["w1"].T
    w1blk[95, 0:25] = inputs["b1"]
    w1blk[95, 25:50] = inputs["b1"]
    w1blk[95, 127] = 30.0
    w2blk = np.zeros((128, 90), np.float32)
    w2blk[0:25, 0:45] = inputs["w2"].T
    w2blk[25:50, 45:90] = inputs["w2"].T
    w2blk[127, 0:45] = inputs["b2"]
    w2blk[127, 45:90] = inputs["b2"]
    return wpk, w1blk, w2blk


def _fit_exp(tsamp, K, wsamp=None, ntail=0.5):
    t = np.asarray(tsamp, np.float64)
    w = np.ones_like(t) if wsamp is None else np.asarray(wsamp, np.float64)
    tm = max(np.abs(t).max(), 1e-3)
    textra = np.linspace(-tm, tm, 64)
    t = np.concatenate([t, textra])
    w = np.concatenate([w, np.full(64, ntail * w.mean())])
    V = np.vander(t, K + 1, increasing=True) * w[:, None]
    c, *_ = np.linalg.lstsq(V, np.exp(t) * w, rcond=None)
    return c


_CACHE = {}


def kernel(**inputs):
    inputs = {k: np.ascontiguousarray(np.asarray(v)) for k, v in
              inputs.items()}
    x = inputs["x"].astype(np.float32)            # (B, C, L)
    import ml_dtypes
    bf = ml_dtypes.bfloat16

    wpk, w1blk, w2blk = _gate_params(inputs)
    cores = list(range(NCORES))
    KC = P * NT

    if "gate" not in _CACHE:
        _CACHE["gate"] = build_gate_program()
    nc1 = _CACHE["gate"]
    maps1 = []
    for i in cores:
        xt = np.zeros((KC, KC), np.float32)
        xt[0:CL, :] = x.reshape(B, CL)[i * BC:(i + 1) * BC].T
        xt[KC - 1, :] = 1.0
        maps1.append({"xT": xt.astype(bf), "wpk": wpk.astype(bf),
                      "w1blk": w1blk, "w2blk": w2blk,
                      "ident": np.eye(P, dtype=np.float32)})
    r1 = run_bass_kernel_spmd(nc1, maps1, cores).results
    # gate tiles come back as (P, NT*C): row p, block t -> batch p + t*P
    gate = np.zeros((B, C), np.float32)
    for i in cores:
        g = np.asarray(r1[i]["gate"]).astype(np.float32)
        gate[i * BC:(i + 1) * BC] = \
            g.reshape(P, NT, C).transpose(1, 0, 2).reshape(BC, C)
    mean_gate = gate.astype(np.float64).mean(0)
    sel = np.sort(np.argsort(-mean_gate, kind="stable")[:E])

    # expert scalars
    wq, bq = inputs["wq"], inputs["bq"]
    wk, bk = inputs["wk"], inputs["bk"]
    wv, bv = inputs["wv"], inputs["bv"]
    wo, bo = inputs["wo"], inputs["bo"]
    alpha = (wq * wk).sum(1).astype(np.float32)
    gamma = (bq * wk).sum(1).astype(np.float32)
    pv = (wo * wv).sum(1).astype(np.float32)
    qv = ((wo * bv).sum(1) + bo).astype(np.float32)

    usel = x[:, sel, :]                            # (B, E, L)
    # per-expert tau range -> degree ladder
    phimax = np.abs(alpha[None, :, None] * usel).max(axis=(0, 2))
    umax = np.abs(usel).max(axis=(0, 2))
    taumax = phimax * umax
    degs_raw = np.where(taumax <= 0.35, 1,
                        np.where(taumax <= 0.7, 2,
                                 np.where(taumax <= 1.2, 3,
                                          np.where(taumax <= 2.0, 4, 6))))
    perm = np.argsort(degs_raw, kind="stable")     # experts by degree asc
    degs = degs_raw[perm]

    # coefficient fits per (permuted) expert
    rng = np.random.RandomState(12345)
    cd = np.zeros((NLEV - 1, E), np.float32)
    cn = np.zeros((NLEV - 1, E), np.float32)
    sub = usel[::16]                               # (B/16, E, L) samples
    for j, e in enumerate(perm):
        K = int(degs[j])
        ue = sub[:, e, :].astype(np.float64)
        tau = (alpha[e] * ue[:, :, None] * ue[:, None, :]).ravel()
        uw = np.abs(np.broadcast_to(ue[:, None, :], ue.shape[:1] + (L, L))
                    ).ravel()
        ss = rng.choice(tau.size, min(40000, tau.size), replace=False)
        cd[0:K + 1, j] = _fit_exp(tau[ss], K)
        cn[0:K + 1, j] = _fit_exp(tau[ss], K, wsamp=uw[ss] + 0.1)

    # device tensors (l-major, expert-permuted)
    uselp = usel[:, perm, :]
    u_lm = np.ascontiguousarray(uselp.transpose(0, 2, 1).reshape(B, EL))
    phi_lm = np.ascontiguousarray(
        (alpha[perm][None, :, None] * uselp).transpose(0, 2, 1)
        .reshape(B, EL))
    gusel = gamma[perm][None, :, None] * uselp                  # (B,E,L)

    def emaj_chunks(arr):
        # (B,E,L) -> e-major transposed (462,B) in 4 x 128-row DRAM chunks
        tr = arr.transpose(1, 2, 0).reshape(EL, B)
        out = np.zeros((4 * P, B), np.float32)
        for c in range(4):
            r0, r1 = 126 * c, min(126 * (c + 1), EL)
            out[128 * c:128 * c + (r1 - r0)] = tr[r0:r1]
        return out

    uT_pad = emaj_chunks(uselp)
    guT_pad = emaj_chunks(gusel)
    ind = np.zeros((P, 4 * E), np.float32)
    for c in range(4):
        for p_ in range(126):
            r = 126 * c + p_
            if r < EL:
                ind[p_, c * E + r // L] = 1.0
    gsel = gate[:, sel][:, perm]
    gp = gsel * pv[perm][None, :]
    gq = gsel * qv[perm][None, :]
    gpq = np.concatenate([gp, gq], 1).astype(np.float32)   # (B, 44)
    # fold gp into the numerator coefficient scales per row
    gpk = (gp[:, None, :] * cn[None, :, :]).reshape(
        B, (NLEV - 1) * E).astype(np.float32)

    NC_ = NT * E
    # coefficient layout (t, lev, e) to match the per-tile moment blocks
    cd_full = np.tile(cd[None, :, :], (NT, 1, 1)).reshape(1, (NLEV - 1) * NC_)

    key = (tuple(int(d) for d in degs),)
    if _CACHE.get("attn_key") != key:
        _CACHE["attn"] = build_attn_program([int(d) for d in degs])
        _CACHE["attn_key"] = key
    nc2 = _CACHE["attn"]
    maps2 = [{"u": u_lm[i * BC:(i + 1) * BC].astype(bf),
              "phi": phi_lm[i * BC:(i + 1) * BC].astype(bf),
              "uT": uT_pad[:, i * BC:(i + 1) * BC].astype(bf),
              "guT": guT_pad[:, i * BC:(i + 1) * BC].astype(bf),
              "ind": ind.astype(bf),
              "gpq": gpq[i * BC:(i + 1) * BC].astype(bf),
              "gpk": gpk[i * BC:(i + 1) * BC].astype(bf),
              "cd": cd_full.astype(bf)}
             for i in cores]
    r2 = run_bass_kernel_spmd(nc2, maps2, cores).results
    at = np.concatenate([np.asarray(r["at"]).astype(np.float32)
                         for r in r2], 0)          # (B, 462) l-major perm
    gt = np.concatenate([np.asarray(r["gt"]).astype(np.float32)
                         for r in r2], 0)

    inv = np.argsort(perm)
    at_e = at.reshape(B, L, E).transpose(0, 2, 1)[:, inv, :]   # (B,E,L)
    gt_e = gt.reshape(B, L, E).transpose(0, 2, 1)[:, inv, :]
    A_full = np.zeros((B, C, L), np.float32)
    G_full = np.zeros((B, C, L), np.float32)
    A_full[:, sel, :] = at_e
    G_full[:, sel, :] = gt_e
    return G_full.reshape(B, CL), A_full.reshape(B, CL)


# revision 64
# speedup vs baseline: 1.1979x; 1.0033x over previous
"""Trainium2 Bass kernel for grouped-attention MoE routing.

Math (derived from the nn.Module):
  gate  = softmax(mlp(maxpool(conv(x))) + mlp(avgpool(conv(x))))      (B,45)
  sel   = sorted(top22(mean_b gate))                                  (22,)
  Per expert e with u = x[:, sel[e], :]:
    energy[l,m] = (a_e*u_l + g_e) * u_m   (rank-1; scalars a,g from weights)
    attn = softmax_m(energy);  s_l = sum_m u_m attn[l,m]
    y_l  = P_e*s_l + Q_e;      A[:,sel[e],:] = y * gate[:,sel[e]]
  G = x * A (flat);  return (G, A_flat)

Implementation strategy (v2):
  Launch 1 (gate): bf16 PE conv with bias folded via a 127th ones row and
    the avg-pool folded in as extra matmul columns; both MLP branches run
    through one block-diagonal matmul pair; outputs batched into one DMA.
  Launch 2 (attn): the rank-1 softmax is evaluated without the LxL energy
    tensor.  With w = exp(g_e*u) and phi = a_e*u:
      den(phi_l) = sum_m w_m exp(phi_l u_m) ~= sum_k cd_k phi_l^k W_k
      num(phi_l) = sum_m u_m w_m exp(phi_l u_m) ~= sum_k cn_k phi_l^k W_{k+1}
    where W_k = sum_m w_m u_m^k are on-device moments and cd/cn are host-
    fitted per-expert polynomial coefficients (least squares over the
    empirical tau = phi*u range; numerator fit |u|-weighted).  s = num/den.
    Per-expert degree ladder (2..6) by empirical |tau| range; experts are
    permuted so degree classes are contiguous and the nested Horner only
    touches suffix slices for the high degrees.  Everything bf16, l-major
    (l outer, e inner) so per-(b,e) coefficient broadcasts stay packed.
  Routing (45-float mean-gate reduction) is mediated on host between the
  two launches, equivalent to the all-reduce in the sharding hint.
"""

import math
import numpy as np
from contextlib import ExitStack

import bass_rust
import concourse.bass as bass
import concourse.mybir as mybir
import concourse.tile as tile
from concourse.bass_utils import run_bass_kernel_spmd

_MULTIWAIT_OK = ("InstNoOp", "InstAllEngineBarrier",
                 "InstEventSemaphore", "InstUnconditionalBranch")


def legalize_sync_waits(nc):
    """walrus codegen on this stack rejects >1 sync wait on most
    instructions; hoist extra waits onto same-engine NoOps."""
    for func in nc.m.functions:
        for block in func.blocks:
            il = block.instructions
            out = []
            for inst in il:
                tname = type(inst).__name__
                si = getattr(inst, "sync_info", None)
                waits = list(si.on_wait) if si is not None else []
                if tname not in _MULTIWAIT_OK and len(waits) > 1:
                    for k, w in enumerate(waits):
                        nop = mybir.InstNoOp(
                            name=f"{inst.name}-synop{k}", ins=[], outs=[])
                        nop.engine = inst.engine
                        nop.sync_info = bass_rust.SyncInfo(
                            on_wait=[w], on_update=[])
                        out.append(nop)
                    inst.sync_info = bass_rust.SyncInfo(
                        on_wait=[], on_update=list(inst.sync_info.on_update))
                out.append(inst)
            il.clear()
            il.extend(out)


B, C, L, E = 8192, 45, 21, 22
NCORES = 8
BC = B // NCORES          # rows per core
P = 128                   # SBUF partitions
NT = BC // P              # batch tiles per core
CL = C * L                # 945
EL = E * L                # 462
WF = NT * EL              # 3696 full-shard free width (t, l, e) l-major
F32 = mybir.dt.float32
BF16 = mybir.dt.bfloat16
AF = mybir.ActivationFunctionType
ALU = mybir.AluOpType
AX = mybir.AxisListType

NG = 8                    # conv channel groups (6,6,...,3 channels)
GCH = [list(range(g, min(g + 6, C))) for g in range(0, C, 6)]
NLEV = 8                  # moment levels W_0..W_7 held on device
DEG_MAX = 6


def _ap(base, extra_free):
    """Custom free-dim access pattern on an SBUF tile slice (partition
    dim kept from `base`)."""
    return bass.AP(tensor=base.tensor, offset=base.offset,
                   ap=[base.ap[0]] + extra_free)


def _off(base, extra_free, col_off):
    ap = bass.AP(tensor=base.tensor, offset=base.offset,
                 ap=[base.ap[0]] + extra_free)
    ap.offset = ap.offset + col_off
    return ap


# --------------------------------------------------------------------------
# Launch 1: gating network
# --------------------------------------------------------------------------

def build_gate_program():
    """Gate launch. Host uploads x TRANSPOSED and padded: XT (1024, 1024)
    with rows 0:945 = x[coreshard].T (row = (chan,l)), rows 945:1023 = 0,
    row 1023 = 1 (bias lane).  The conv+avg matmul accumulates chunk-wise
    into 3 PSUM column blocks of 15 channels x 22 outputs; no on-device
    transposes or PSUM->SBUF copies are needed for the conv at all."""
    nc = bass.Bass()
    KC = P * NT                       # 1024 padded contraction rows
    xT = nc.declare_dram_parameter("xT", [KC, KC], BF16, isOutput=False)
    # packed rhs slices for the 12 (chunk, block) matmuls: (128, 12*330)
    wpk = nc.declare_dram_parameter("wpk", [P, 12 * 330], BF16,
                                    isOutput=False)
    w1blk = nc.declare_dram_parameter("w1blk", [P, P], F32, isOutput=False)
    w2blk = nc.declare_dram_parameter("w2blk", [P, 90], F32, isOutput=False)
    ident = nc.declare_dram_parameter("ident", [P, P], F32, isOutput=False)
    gate_o = nc.declare_dram_parameter("gate", [P, NT * C], BF16,
                                       isOutput=True)

    # (chunk, block) pairs: block j covers channels 15j..15j+14 =
    # contraction rows 315j..315j+314, plus the bias lane in chunk 7
    PAIRS = [[0, 1, 2, 7], [2, 3, 4, 7], [4, 5, 6, 7]]

    with tile.TileContext(nc) as tc, ExitStack() as ctx, \
            nc.allow_low_precision(reason="bf16 gate pipeline; 2e-2 tol"):
        singles = ctx.enter_context(tc.tile_pool(name="singles", bufs=1))
        cvps = ctx.enter_context(tc.tile_pool(name="cvps", bufs=2,
                                              space="PSUM"))
        ppps = ctx.enter_context(tc.tile_pool(name="ppps", bufs=1,
                                              space="PSUM"))
        work = ctx.enter_context(tc.tile_pool(name="work", bufs=2))
        small = ctx.enter_context(tc.tile_pool(name="small", bufs=3))

        def dve_const(dram, p, n, dt=BF16):
            raw = singles.tile([p, n], dt, name="raw_" + dram.name)
            nc.sync.dma_start(out=raw, in_=dram[:, :])
            t = singles.tile([p, n], dt, name="sb_" + dram.name)
            nc.vector.tensor_copy(out=t, in_=raw)
            return t

        sb_w = dve_const(wpk, P, 12 * 330)

        # xT loads: per-tile DMA of the (128, 8x128) lhsT panel so tile 0
        # can start early.  lhs[p, c*128+b] = XT[128c+p, 128t+b]
        xb = xT[:, :]
        lhsT = []

        def load_lhsT(t):
            lt = singles.tile([P, NT * P], BF16, name=f"lhsT{t}")
            ap = bass.AP(tensor=xb.tensor, offset=xb.offset,
                         ap=[[KC, P], [P * KC, NT], [1, P]])
            ap.offset = ap.offset + t * P
            nc.sync.dma_start(out=lt, in_=ap)
            lhsT.append(lt)

        load_lhsT(0)
        load_lhsT(1)
        sb_w1 = dve_const(w1blk, P, P, F32)
        sb_w2 = dve_const(w2blk, P, 90, F32)
        sb_id = dve_const(ident, P, P, F32)
        for t in range(2, NT):
            load_lhsT(t)

        # persistent h tiles: cols 0:90 rewritten each use; 90:128 junk is
        # zeroed once (col 95 = ones lane feeding the layer-1 bias row)
        hb = []
        for i in range(2):
            t = singles.tile([P, P], F32, name=f"hb{i}")
            nc.vector.memset(t[:, 90:P], 0.0)
            nc.vector.memset(t[:, 95:96], 1.0)
            hb.append(t)

        gate_all = singles.tile([P, NT * C], BF16)
        zall = singles.tile([P, NT * C], BF16)

        def conv_block(t):
            lt = lhsT[t]
            cvt = [cvps.tile([P, 330], F32, tag=f"cv{j}", name=f"cv{j}")
                   for j in range(3)]
            for j in range(3):
                for i, c in enumerate(PAIRS[j]):
                    nc.tensor.matmul(
                        cvt[j], lt[:, c * P:(c + 1) * P],
                        sb_w[:, (j * 4 + i) * 330:(j * 4 + i + 1) * 330],
                        start=(i == 0), stop=(i == len(PAIRS[j]) - 1))
            h = hb[t % 2]
            for j in range(3):
                nc.vector.tensor_reduce(
                    out=h[:, 15 * j:15 * j + 15],
                    in_=_ap(cvt[j][:, 0:330], [[22, 15], [1, L]]),
                    axis=AX.X, op=ALU.max)
            # avg lanes (col 21 of each 22-block): 2 on DVE, 1 on Act
            nc.vector.tensor_copy(out=h[:, 45:60],
                                  in_=_off(cvt[0], [[22, 15]], 21))
            nc.scalar.copy(out=h[:, 60:75], in_=_off(cvt[1], [[22, 15]], 21))
            nc.vector.tensor_copy(out=h[:, 75:90],
                                  in_=_off(cvt[2], [[22, 15]], 21))
            return h

        def mlp_block(t, h):
            # both MLP branches through 128x128 f32 blocks; PE transposes
            # with ones lanes via h col 95 and the saturated tanh col 127
            trm = ppps.tile([P, P], F32, tag="trm", name="trm")
            nc.tensor.transpose(trm, h, sb_id)
            hT = work.tile([P, P], F32, tag="hT", name="hT")
            nc.scalar.copy(out=hT, in_=trm)
            pp = ppps.tile([P, P], F32, tag="pp", name="pp")
            nc.tensor.matmul(pp, hT, sb_w1, start=True, stop=True)
            t1 = small.tile([P, P], F32, tag="t1", name="t1")
            nc.scalar.activation(out=t1, in_=pp, func=AF.Tanh)
            trm2 = ppps.tile([P, P], F32, tag="trm", name="trm2")
            nc.tensor.transpose(trm2, t1, sb_id)
            t1T = work.tile([P, P], F32, tag="t1T", name="t1T")
            nc.vector.tensor_copy(out=t1T, in_=trm2)
            p2 = ppps.tile([P, 90], F32, tag="pp", name="p2")
            nc.tensor.matmul(p2, t1T, sb_w2, start=True, stop=True)
            z2 = small.tile([P, 90], BF16, tag="z2", name="z2")
            nc.scalar.activation(out=z2, in_=p2, func=AF.Tanh)
            nc.vector.tensor_add(out=zall[:, t * C:(t + 1) * C],
                                 in0=z2[:, 0:45], in1=z2[:, 45:90])

        def softmax_half(lo, nt):
            # segmented softmax over tile blocks [lo, lo+nt) + output DMA
            zs = zall[:, lo * C:(lo + nt) * C]
            zmax = small.tile([P, nt], F32, tag="zmax", name="zmax")
            nc.vector.tensor_reduce(out=zmax, in_=_ap(zs, [[C, nt], [1, C]]),
                                    axis=AX.X, op=ALU.max)
            zmax16 = small.tile([P, nt], BF16, tag="zmax16", name="zmax16")
            nc.vector.tensor_copy(out=zmax16, in_=zmax)
            zsub = work.tile([P, nt * C], BF16, tag="zsub", name="zsub")
            nc.vector.tensor_sub(out=zsub, in0=zs,
                                 in1=_ap(zmax16, [[1, nt], [0, C]]))
            eg = work.tile([P, nt * C], BF16, tag="eg", name="eg")
            nc.scalar.activation(out=eg, in_=zsub, func=AF.Exp)
            ssum = small.tile([P, nt], F32, tag="ssum", name="ssum")
            nc.vector.tensor_reduce(out=ssum, in_=_ap(eg, [[C, nt], [1, C]]),
                                    axis=AX.X, op=ALU.add)
            rs = small.tile([P, nt], BF16, tag="rs", name="rs")
            nc.vector.reciprocal(out=rs, in_=ssum)
            gs = gate_all[:, lo * C:(lo + nt) * C]
            nc.vector.tensor_mul(out=gs, in0=eg,
                                 in1=_ap(rs, [[1, nt], [0, C]]))
            nc.sync.dma_start(out=gate_o[:, lo * C:(lo + nt) * C], in_=gs)

        # software-pipelined emission: engines issue in program order, so
        # interleave conv(t+1) ahead of mlp(t) to let tiles overlap; the
        # first softmax half runs under the back half of the pipeline
        hprev = None
        for t in range(NT + 1):
            if t < NT:
                hcur = conv_block(t)
            if t >= 1:
                mlp_block(t - 1, hprev)
            if t == NT // 2 + 1:
                softmax_half(0, NT // 2)
            hprev = hcur
        softmax_half(NT // 2, NT // 2)
    legalize_sync_waits(nc)
    return nc


# --------------------------------------------------------------------------
# Launch 2: expert attention via fitted moment polynomials
# --------------------------------------------------------------------------

def build_attn_program(degs):
    """degs: per-expert polynomial degree, sorted ascending (len 22)."""
    degs = list(degs)
    assert degs == sorted(degs)
    dmax = max(degs)
    # suffix start index for "experts with degree > k"
    estart = {k: next((i for i in range(E) if degs[i] > k), E)
              for k in range(dmax)}
    # first expert of each degree class (for acc initialization)
    class_start = {}
    for i, d in enumerate(degs):
        class_start.setdefault(d, i)

    nc = bass.Bass()
    u_d = nc.declare_dram_parameter("u", [BC, EL], BF16, isOutput=False)
    phi_d = nc.declare_dram_parameter("phi", [BC, EL], BF16, isOutput=False)
    # e-major transposed u and g*u: 4 chunks of 128 DRAM rows; chunk c rows
    # 0:126 = transposed rows [126c,126c+126) (row = 21*e_perm + l), rest 0
    uT_d = nc.declare_dram_parameter("uT", [4 * P, BC], BF16, isOutput=False)
    guT_d = nc.declare_dram_parameter("guT", [4 * P, BC], BF16,
                                      isOutput=False)
    ind_d = nc.declare_dram_parameter("ind", [P, 4 * E], BF16, isOutput=False)
    gpq_d = nc.declare_dram_parameter("gpq", [BC, 2 * E], BF16,
                                      isOutput=False)
    # per-row numerator coefficient scales: gpk[b, lev*22+e] = gp[b,e]*cn[lev,e]
    gpk_d = nc.declare_dram_parameter("gpk", [BC, (NLEV - 1) * E], BF16,
                                      isOutput=False)
    cd_d = nc.declare_dram_parameter("cd", [1, (NLEV - 1) * NT * E], BF16,
                                     isOutput=False)
    at_o = nc.declare_dram_parameter("at", [BC, EL], BF16, isOutput=True)
    gt_o = nc.declare_dram_parameter("gt", [BC, EL], BF16, isOutput=True)

    def shard_ap(dram, ncols):
        base = dram[:, :]
        return bass.AP(tensor=base.tensor, offset=base.offset,
                       ap=[[ncols, P], [P * ncols, NT], [1, ncols]])

    with tile.TileContext(nc) as tc, ExitStack() as ctx, \
            nc.allow_low_precision(reason="bf16 attn pipeline; 2e-2 tol"):
        sg = ctx.enter_context(tc.tile_pool(name="sg", bufs=1))
        wpool = ctx.enter_context(tc.tile_pool(name="wpool", bufs=1,
                                               space="PSUM"))

        guT = sg.tile([P, 4 * BC], BF16)
        uT = sg.tile([P, 4 * BC], BF16)
        for hh in range(2):
            for dram, tile_ in ((guT_d, guT), (uT_d, uT)):
                bbb = dram[:, :]
                ap = bass.AP(tensor=bbb.tensor, offset=bbb.offset,
                             ap=[[BC, P], [P * BC, 2], [1, BC]])
                ap.offset = ap.offset + hh * 2 * P * BC
                nc.sync.dma_start(
                    out=tile_[:, hh * 2 * BC:(hh + 1) * 2 * BC], in_=ap)
        u = sg.tile([P, WF], BF16)
        nc.sync.dma_start(out=u, in_=shard_ap(u_d, EL))
        phi = sg.tile([P, WF], BF16)
        nc.sync.dma_start(out=phi, in_=shard_ap(phi_d, EL))
        indr = sg.tile([P, 4 * E], BF16)
        nc.sync.dma_start(out=indr, in_=ind_d[:, :])
        ind = sg.tile([P, 4 * E], BF16)
        nc.vector.tensor_copy(out=ind, in_=indr)
        gpq = sg.tile([P, NT * 2 * E], BF16)
        nc.sync.dma_start(out=gpq, in_=shard_ap(gpq_d, 2 * E))

        def bconst(dram, n, nm):
            base = dram[:, :]
            t = sg.tile([P, n], BF16, name=nm)
            nc.sync.dma_start(
                out=t, in_=bass.AP(tensor=base.tensor, offset=base.offset,
                                   ap=[[0, P], [1, n]]))
            return t

        NC_ = NT * E
        cdB = bconst(cd_d, (NLEV - 1) * NC_, "cdB")
        gpk = sg.tile([P, NT * (NLEV - 1) * E], BF16)
        nc.sync.dma_start(out=gpk, in_=shard_ap(gpk_d, (NLEV - 1) * E))

        # Wt layout: col = t*176 + lev*22 + e
        Wt = sg.tile([P, NT * NLEV * E], BF16)

        # slice helpers (l-major: free = (t, l, e); e innermost)
        def full3(tile_, e0=0, ne=E, coloff=0):
            return _off(tile_, [[EL, NT], [E, L], [1, ne]], coloff + e0)

        def coef(ctile, lev, e0=0, ne=E):
            return _off(ctile, [[(NLEV - 1) * E, NT], [0, L], [1, ne]],
                        lev * E + e0)

        # transposed chain: wT = exp(guT); vT_k = vT_{k-1}*uT; chunk c of the
        # free dim holds transposed rows [126c,126c+126) for batch cols
        wT = sg.tile([P, 4 * BC], BF16)
        nc.scalar.activation(out=wT[:, 0:2 * BC], in_=guT[:, 0:2 * BC],
                             func=AF.Exp)
        nc.scalar.activation(out=wT[:, 2 * BC:4 * BC],
                             in_=guT[:, 2 * BC:4 * BC], func=AF.Exp)
        vTa = sg.tile([P, 4 * BC], BF16)
        vTb = sg.tile([P, 4 * BC], BF16)

        # chunk start per level: experts with deg >= lev-1 live in chunks
        # >= estart[lev-2]//6 (extra experts in a straddling chunk are
        # harmless: their cd/cn consts are zero)
        def cstart(lev):
            if lev <= 2:
                return 0
            e0 = estart.get(lev - 2, E)
            return 4 if e0 >= E else e0 // 6

        wps = [wpool.tile([P, NLEV * E], F32, tag=f"wp{t}", name=f"wp{t}")
               for t in range(NT)]

        def moments(lev, vt):
            c0 = cstart(lev)
            for t in range(NT):
                for c in range(c0, 4):
                    nc.tensor.matmul(
                        wps[t][:, lev * E:(lev + 1) * E],
                        vt[0:126, c * BC + t * P:c * BC + (t + 1) * P],
                        ind[0:126, c * E:(c + 1) * E],
                        start=(c == c0), stop=(c == 3))

        moments(0, wT)
        cur, nxt = vTa, vTb
        nc.vector.tensor_mul(out=cur[:, 0:2 * BC], in0=wT[:, 0:2 * BC],
                             in1=uT[:, 0:2 * BC])
        nc.vector.tensor_mul(out=cur[:, 2 * BC:4 * BC],
                             in0=wT[:, 2 * BC:4 * BC],
                             in1=uT[:, 2 * BC:4 * BC])
        moments(1, cur)
        for lev in range(2, NLEV):
            c0 = cstart(lev)
            if c0 >= 4:
                break
            off = c0 * BC
            if c0 < 2:
                nc.vector.tensor_mul(out=nxt[:, off:2 * BC],
                                     in0=cur[:, off:2 * BC],
                                     in1=uT[:, off:2 * BC])
                nc.vector.tensor_mul(out=nxt[:, 2 * BC:4 * BC],
                                     in0=cur[:, 2 * BC:4 * BC],
                                     in1=uT[:, 2 * BC:4 * BC])
            else:
                nc.vector.tensor_mul(out=nxt[:, off:4 * BC],
                                     in0=cur[:, off:4 * BC],
                                     in1=uT[:, off:4 * BC])
            moments(lev, nxt)
            cur, nxt = nxt, cur

        # PSUM -> Wt copies, one per tile block (DVE/Act split)
        for t in range(NT):
            dst = Wt[:, t * NLEV * E:(t + 1) * NLEV * E]
            if t % 2 == 0:
                nc.vector.tensor_copy(out=dst, in_=wps[t])
            else:
                nc.scalar.copy(out=dst, in_=wps[t])

        # Horner coefficient tensors: D_k = W_k*cd_k, N_k = W_{k+1}*cn_k
        NL1 = (NLEV - 1) * E
        Dt = sg.tile([P, NT * NL1], BF16)
        nc.vector.tensor_mul(out=_ap(Dt, [[NL1, NT], [1, NL1]]),
                             in0=_ap(Wt, [[NLEV * E, NT], [1, NL1]]),
                             in1=cdB)
        Nt = sg.tile([P, NT * NL1], BF16)
        nc.vector.tensor_mul(out=_ap(Nt, [[NL1, NT], [1, NL1]]),
                             in0=_off(Wt, [[NLEV * E, NT], [1, NL1]], E),
                             in1=gpk)

        # nested mixed-degree Horner (experts sorted by degree ascending);
        # a class's accumulator starts life fused into its first step:
        # acc = coef(d)*phi + coef(d-1)
        accd = sg.tile([P, WF], BF16)
        accn = sg.tile([P, WF], BF16)
        for k in range(dmax - 1, -1, -1):
            e0 = estart[k]
            ne = E - e0
            cs = class_start.get(k + 1)
            for acc, Ct in ((accd, Dt), (accn, Nt)):
                if cs is not None:
                    ncs = (min([s for dd, s in class_start.items()
                                if dd > k + 1], default=E)) - cs
                    nc.vector.tensor_mul(out=full3(acc, cs, ncs),
                                         in0=coef(Ct, k + 1, cs, ncs),
                                         in1=full3(phi, cs, ncs))
                    e1 = cs + ncs
                    if E - e1 > 0:
                        nc.vector.tensor_mul(out=full3(acc, e1, E - e1),
                                             in0=full3(acc, e1, E - e1),
                                             in1=full3(phi, e1, E - e1))
                else:
                    nc.vector.tensor_mul(out=full3(acc, e0, ne),
                                         in0=full3(acc, e0, ne),
                                         in1=full3(phi, e0, ne))
                nc.vector.tensor_add(out=full3(acc, e0, ne),
                                     in0=full3(acc, e0, ne),
                                     in1=coef(Ct, k, e0, ne))

        # s = num/den; at = s*gp + gq; gt = at*u
        # (tail ops split 16/6 experts across DVE and Pool so they overlap)
        rden = sg.tile([P, WF], BF16)
        nc.vector.reciprocal(out=rden, in_=accd)
        ESP = 16
        at = accn
        nc.vector.tensor_mul(out=full3(at, 0, ESP), in0=full3(accn, 0, ESP),
                             in1=full3(rden, 0, ESP))
        nc.gpsimd.tensor_mul(out=full3(at, ESP, E - ESP),
                             in0=full3(accn, ESP, E - ESP),
                             in1=full3(rden, ESP, E - ESP))
        nc.vector.tensor_add(
            out=full3(at, 0, ESP), in0=full3(at, 0, ESP),
            in1=_off(gpq, [[2 * E, NT], [0, L], [1, ESP]], E))
        nc.gpsimd.tensor_add(
            out=full3(at, ESP, E - ESP), in0=full3(at, ESP, E - ESP),
            in1=_off(gpq, [[2 * E, NT], [0, L], [1, E - ESP]], E + ESP))
        gt = rden
        nc.gpsimd.tensor_mul(out=full3(gt, ESP, E - ESP),
                             in0=full3(at, ESP, E - ESP),
                             in1=full3(u, ESP, E - ESP))
        nc.vector.tensor_mul(out=full3(gt, 0, ESP),
                             in0=full3(at, 0, ESP),
                             in1=full3(u, 0, ESP))
        nc.sync.dma_start(out=shard_ap(at_o, EL), in_=at)
        nc.sync.dma_start(out=shard_ap(gt_o, EL), in_=gt)
    legalize_sync_waits(nc)
    return nc


# --------------------------------------------------------------------------
# Host-side preparation
# --------------------------------------------------------------------------

def _gate_params(inputs):
    gc_w = inputs["gc_w"].astype(np.float64)
    gc_b = inputs["gc_b"].astype(np.float64)
    KC = P * NT
    # full conv weight: rows = (chan,l) + pad + bias lane, cols = (chan, 22)
    wfull = np.zeros((KC, 990), np.float32)
    wavvec = gc_w.mean(0)
    for i in range(C):
        wfull[i * L:(i + 1) * L, i * 22:i * 22 + L] = gc_w.T
        wfull[i * L:(i + 1) * L, i * 22 + L] = wavvec
        wfull[KC - 1, i * 22:i * 22 + L] = gc_b
        wfull[KC - 1, i * 22 + L] = gc_b.mean()
    PAIRS = [[0, 1, 2, 7], [2, 3, 4, 7], [4, 5, 6, 7]]
    wpk = np.zeros((P, 12 * 330), np.float32)
    for j in range(3):
        for i, c in enumerate(PAIRS[j]):
            wpk[:, (j * 4 + i) * 330:(j * 4 + i + 1) * 330] = \
                wfull[c * P:(c + 1) * P, j * 330:(j + 1) * 330]
    # 128x128 MLP layer-1 block: rows = transposed h cols (0:45 mx, 45:90
    # av, 95 = ones), out cols 0:50 = both branch hiddens, col 127 driven to
    # +30 via the ones row so tanh saturates to an exact 1.0 "ones" lane for
    # layer 2; all other cells zero.
    w1blk = np.zeros((128, 128), np.float32)
    w1blk[0:45, 0:25] = inputs["w1"].T
    w1blk[45:90, 25:50] = inputs["w1"].T
    w1blk[95, 0:25] = inputs["b1"]
    w1blk[95, 25:50] = inputs["b1"]
    w1blk[95, 127] = 30.0
    w2blk = np.zeros((128, 90), np.float32)
    w2blk[0:25, 0:45] = inputs["w2"].T
    w2blk[25:50, 45:90] = inputs["w2"].T
    w2blk[127, 0:45] = inputs["b2"]
    w2blk[127, 45:90] = inputs["b2"]
    return wpk, w1blk, w2blk


def _fit_exp(tsamp, K, wsamp=None, ntail=0.5):
    t = np.asarray(tsamp, np.float64)
    w = np.ones_like(t) if wsamp is None else np.asarray(wsamp, np.float64)
    tm = max(np.abs(t).max(), 1e-3)
    textra = np.linspace(-tm, tm, 64)
    t = np.concatenate([t, textra])
    w = np.concatenate([w, np.full(64, ntail * w.mean())])
    V = np.vander(t, K + 1, increasing=True) * w[:, None]
    c, *_ = np.linalg.lstsq(V, np.exp(t) * w, rcond=None)
    return c


_CACHE = {}


def kernel(**inputs):
    inputs = {k: np.ascontiguousarray(np.asarray(v)) for k, v in
              inputs.items()}
    x = inputs["x"].astype(np.float32)            # (B, C, L)
    import ml_dtypes
    bf = ml_dtypes.bfloat16

    wpk, w1blk, w2blk = _gate_params(inputs)
    cores = list(range(NCORES))
    KC = P * NT

    if "gate" not in _CACHE:
        _CACHE["gate"] = build_gate_program()
    nc1 = _CACHE["gate"]
    maps1 = []
    for i in cores:
        xt = np.zeros((KC, KC), np.float32)
        xt[0:CL, :] = x.reshape(B, CL)[i * BC:(i + 1) * BC].T
        xt[KC - 1, :] = 1.0
        maps1.append({"xT": xt.astype(bf), "wpk": wpk.astype(bf),
                      "w1blk": w1blk, "w2blk": w2blk,
                      "ident": np.eye(P, dtype=np.float32)})
    r1 = run_bass_kernel_spmd(nc1, maps1, cores).results
    # gate tiles come back as (P, NT*C): row p, block t -> batch p + t*P
    gate = np.zeros((B, C), np.float32)
    for i in cores:
        g = np.asarray(r1[i]["gate"]).astype(np.float32)
        gate[i * BC:(i + 1) * BC] = \
            g.reshape(P, NT, C).transpose(1, 0, 2).reshape(BC, C)
    mean_gate = gate.astype(np.float64).mean(0)
    sel = np.sort(np.argsort(-mean_gate, kind="stable")[:E])

    # expert scalars
    wq, bq = inputs["wq"], inputs["bq"]
    wk, bk = inputs["wk"], inputs["bk"]
    wv, bv = inputs["wv"], inputs["bv"]
    wo, bo = inputs["wo"], inputs["bo"]
    alpha = (wq * wk).sum(1).astype(np.float32)
    gamma = (bq * wk).sum(1).astype(np.float32)
    pv = (wo * wv).sum(1).astype(np.float32)
    qv = ((wo * bv).sum(1) + bo).astype(np.float32)

    usel = x[:, sel, :]                            # (B, E, L)
    # per-expert tau range -> degree ladder
    phimax = np.abs(alpha[None, :, None] * usel).max(axis=(0, 2))
    umax = np.abs(usel).max(axis=(0, 2))
    taumax = phimax * umax
    degs_raw = np.where(taumax <= 0.5, 1,
                        np.where(taumax <= 0.7, 2,
                                 np.where(taumax <= 1.2, 3,
                                          np.where(taumax <= 2.0, 4, 6))))
    perm = np.argsort(degs_raw, kind="stable")     # experts by degree asc
    degs = degs_raw[perm]

    # coefficient fits per (permuted) expert
    rng = np.random.RandomState(12345)
    cd = np.zeros((NLEV - 1, E), np.float32)
    cn = np.zeros((NLEV - 1, E), np.float32)
    sub = usel[::16]                               # (B/16, E, L) samples
    for j, e in enumerate(perm):
        K = int(degs[j])
        ue = sub[:, e, :].astype(np.float64)
        tau = (alpha[e] * ue[:, :, None] * ue[:, None, :]).ravel()
        uw = np.abs(np.broadcast_to(ue[:, None, :], ue.shape[:1] + (L, L))
                    ).ravel()
        ss = rng.choice(tau.size, min(40000, tau.size), replace=False)
        cd[0:K + 1, j] = _fit_exp(tau[ss], K)
        cn[0:K + 1, j] = _fit_exp(tau[ss], K, wsamp=uw[ss] + 0.1)

    # device tensors (l-major, expert-permuted)
    uselp = usel[:, perm, :]
    u_lm = np.ascontiguousarray(uselp.transpose(0, 2, 1).reshape(B, EL))
    phi_lm = np.ascontiguousarray(
        (alpha[perm][None, :, None] * uselp).transpose(0, 2, 1)
        .reshape(B, EL))
    gusel = gamma[perm][None, :, None] * uselp                  # (B,E,L)

    def emaj_chunks(arr):
        # (B,E,L) -> e-major transposed (462,B) in 4 x 128-row DRAM chunks
        tr = arr.transpose(1, 2, 0).reshape(EL, B)
        out = np.zeros((4 * P, B), np.float32)
        for c in range(4):
            r0, r1 = 126 * c, min(126 * (c + 1), EL)
            out[128 * c:128 * c + (r1 - r0)] = tr[r0:r1]
        return out

    uT_pad = emaj_chunks(uselp)
    guT_pad = emaj_chunks(gusel)
    ind = np.zeros((P, 4 * E), np.float32)
    for c in range(4):
        for p_ in range(126):
            r = 126 * c + p_
            if r < EL:
                ind[p_, c * E + r // L] = 1.0
    gsel = gate[:, sel][:, perm]
    gp = gsel * pv[perm][None, :]
    gq = gsel * qv[perm][None, :]
    gpq = np.concatenate([gp, gq], 1).astype(np.float32)   # (B, 44)
    # fold gp into the numerator coefficient scales per row
    gpk = (gp[:, None, :] * cn[None, :, :]).reshape(
        B, (NLEV - 1) * E).astype(np.float32)

    NC_ = NT * E
    # coefficient layout (t, lev, e) to match the per-tile moment blocks
    cd_full = np.tile(cd[None, :, :], (NT, 1, 1)).reshape(1, (NLEV - 1) * NC_)

    key = (tuple(int(d) for d in degs),)
    if _CACHE.get("attn_key") != key:
        _CACHE["attn"] = build_attn_program([int(d) for d in degs])
        _CACHE["attn_key"] = key
    nc2 = _CACHE["attn"]
    maps2 = [{"u": u_lm[i * BC:(i + 1) * BC].astype(bf),
              "phi": phi_lm[i * BC:(i + 1) * BC].astype(bf),
              "uT": uT_pad[:, i * BC:(i + 1) * BC].astype(bf),
              "guT": guT_pad[:, i * BC:(i + 1) * BC].astype(bf),
              "ind": ind.astype(bf),
              "gpq": gpq[i * BC:(i + 1) * BC].astype(bf),
              "gpk": gpk[i * BC:(i + 1) * BC].astype(bf),
              "cd": cd_full.astype(bf)}
             for i in cores]
    r2 = run_bass_kernel_spmd(nc2, maps2, cores).results
    at = np.concatenate([np.asarray(r["at"]).astype(np.float32)
                         for r in r2], 0)          # (B, 462) l-major perm
    gt = np.concatenate([np.asarray(r["gt"]).astype(np.float32)
                         for r in r2], 0)

    inv = np.argsort(perm)
    at_e = at.reshape(B, L, E).transpose(0, 2, 1)[:, inv, :]   # (B,E,L)
    gt_e = gt.reshape(B, L, E).transpose(0, 2, 1)[:, inv, :]
    A_full = np.zeros((B, C, L), np.float32)
    G_full = np.zeros((B, C, L), np.float32)
    A_full[:, sel, :] = at_e
    G_full[:, sel, :] = gt_e
    return G_full.reshape(B, CL), A_full.reshape(B, CL)
